# revision 8
# baseline (speedup 1.0000x reference)
"""Complex nearest-neighbor 2x spatial upsample on 8 TRN2 NeuronCores.

Reference op: x = x_real + 1j*x_imag, shape [8, 128, 128, 64] (B,H,W,C);
out[b, j, k, c] = x[b, r(j), r(k), c] with
r(j) = clip(round_half_to_even(j/2), 0, 127), output [8, 256, 256, 64]
complex64.

Strategy (batch-sharded, 1 sample per core):
  - Host: interleave real/imag into f32 [H, W, 2C] so a complex "pixel"
    is one contiguous 512B chunk and the complex64 output is a pure view.
  - Device: stage the 8 MiB sample in SBUF (128 rows -> 128 partitions),
    then scatter to the 32 MiB output with strided DMAs.  The
    round-half-to-even gather decomposes exactly into 4 affine families
    per axis, so 4x4 = 16 DRAM-write DMAs with 3-dim access patterns
    (rows, cols, 512B contiguous pixel) cover the whole output.
"""

import numpy as np

_B, _H, _W, _C = 8, 128, 128, 64
_C2 = 2 * _C
_HO, _WO = 2 * _H, 2 * _W
_N_CORES = 8

# Affine families of j -> r(j) = clip(round_half_even(j/2), 0, 127), j in [0,256):
#   j = 2m   -> m      (m = 0..127)
#   j = 4t+1 -> 2t     (t = 0..63)
#   j = 4t+3 -> 2t+2   (t = 0..62)
#   j = 255  -> 127
# Tuples: (dst_start, dst_step, src_start, src_step, count)
_FAMILIES = [
    (0, 2, 0, 1, 128),
    (1, 4, 0, 2, 64),
    (3, 4, 2, 2, 63),
    (255, 1, 127, 1, 1),
]

# Set by test harnesses: TRACE=True makes kernel() profile the run and
# stash the BassKernelResults (incl. exec_time_ns) in LAST_RESULT.
TRACE = False
LAST_RESULT = None

_NC_CACHE = {}


def _ensure_axon_ntff_hook():
    """Provide antenv.axon_hooks when the image ships only the antenv stub.

    concourse.bass_utils imports it for trace=True under axon; the slim
    agent image's boot fails to register the hook because the stub antenv
    package has no axon_hooks submodule.  Recreate the ctypes-based NTFF
    hook against libaxon_pjrt.so (same recipe as trn_agent_boot.trn_boot).
    """
    try:
        import antenv.axon_hooks  # noqa: F401

        return
    except ImportError:
        pass

    import contextlib
    import ctypes
    import sys
    import types

    mod = types.ModuleType("antenv.axon_hooks")
    holder = {"hook": None}

    def set_axon_ntff_profile_hook(hook):
        holder["hook"] = hook

    def get_axon_ntff_profile_hook():
        return holder["hook"]

    mod.set_axon_ntff_profile_hook = set_axon_ntff_profile_hook
    mod.get_axon_ntff_profile_hook = get_axon_ntff_profile_hook
    sys.modules["antenv.axon_hooks"] = mod
    try:
        import antenv

        antenv.axon_hooks = mod
    except ImportError:
        pass

    so_path = "/opt/axon/libaxon_pjrt.so"
    try:
        lib = ctypes.CDLL(so_path)
    except OSError:
        return
    if not hasattr(lib, "axon_start_nrt_profile"):
        return
    lib.axon_start_nrt_profile.argtypes = [
        ctypes.POINTER(ctypes.c_int64),
        ctypes.c_size_t,
    ]
    lib.axon_start_nrt_profile.restype = ctypes.c_int64
    lib.axon_stop_nrt_profile.argtypes = [ctypes.c_char_p]
    lib.axon_stop_nrt_profile.restype = ctypes.c_int64

    @contextlib.contextmanager
    def _hook(output_dir, device_ids):
        import jax

        jax.devices()
        if device_ids:
            ids = (ctypes.c_int64 * len(device_ids))(*device_ids)
            rc = lib.axon_start_nrt_profile(ids, len(device_ids))
        else:
            rc = lib.axon_start_nrt_profile(None, 0)
        if rc != 0:
            raise RuntimeError(f"axon_start_nrt_profile rc={rc}")
        try:
            yield
        finally:
            n = lib.axon_stop_nrt_profile(str(output_dir).encode())
            if n < 0:
                raise RuntimeError(f"axon_stop_nrt_profile rc={n}")

    set_axon_ntff_profile_hook(_hook)


def _sl(start, step, count):
    return slice(start, start + (count - 1) * step + 1, step)


def _build_nc_v1():
    """Pure-DMA scatter: 16 strided DMAs with 512B descriptors.

    Measured 165 us/core: descriptor-rate limited (all 16 SDMA engines
    ~100% busy at ~30 ns per 512B descriptor)."""
    import concourse.bacc as bacc
    import concourse.mybir as mybir
    from concourse.tile import TileContext

    nc = bacc.Bacc()
    x = nc.dram_tensor("x", [_H, _W, _C2], mybir.dt.float32, kind="ExternalInput")
    y = nc.dram_tensor("y", [_HO, _WO, _C2], mybir.dt.float32, kind="ExternalOutput")

    with TileContext(nc) as tc:
        with tc.tile_pool(name="stage", bufs=1) as pool:
            t = pool.tile([_H, _W * _C2], mybir.dt.float32)
            t3 = t[:].rearrange("h (w c) -> h w c", c=_C2)
            # 8 MiB load: one contiguous 64 KiB row per partition.
            nc.sync.dma_start(t[:], x[:].rearrange("h w c -> h (w c)"))
            # 16 strided scatter DMAs, alternating between the two HWDGE
            # rings (sync + scalar) so they drain in parallel.
            engines = [nc.sync, nc.scalar]
            i = 0
            for rd0, rds, rs0, rss, rc in _FAMILIES:
                for cd0, cds, cs0, css, cc in _FAMILIES:
                    eng = engines[i % len(engines)]
                    i += 1
                    eng.dma_start(
                        y[_sl(rd0, rds, rc), _sl(cd0, cds, cc), :],
                        t3[_sl(rs0, rss, rc), _sl(cs0, css, cc), :],
                    )
    nc.compile()
    return nc


def _build_nc_v2():
    """On-chip column expansion + contiguous-row scatter.

    Input rows live one-per-partition.  The vector engine expands the
    column (W) axis into U tiles (64 output cols per quarter, 32 KiB per
    partition), then each quarter is written out with 4 row-family DMAs
    whose descriptors are 32 KiB contiguous — DMA runs at line rate
    instead of the 512B descriptor floor of v1.
    """
    import concourse.bacc as bacc
    import concourse.mybir as mybir
    from concourse.tile import TileContext

    f32 = mybir.dt.float32
    nc = bacc.Bacc()
    x = nc.dram_tensor("x", [_H, _W, _C2], f32, kind="ExternalInput")
    y = nc.dram_tensor("y", [_HO, _WO, _C2], f32, kind="ExternalOutput")

    with TileContext(nc) as tc:
        with (
            tc.tile_pool(name="tin", bufs=1) as tin_pool,
            tc.tile_pool(name="uexp", bufs=3) as u_pool,
        ):
            # Input halves: t_lo = cols 0..64 (65 cols, needed by output
            # quarters 0-1), t_hi = cols 64..127 (needed by quarters 2-3).
            t_lo = tin_pool.tile([_H, 65 * _C2], f32, tag="tlo")
            t_hi = tin_pool.tile([_H, 64 * _C2], f32, tag="thi")
            nc.gpsimd.dma_start(
                t_lo[:].rearrange("h (w c) -> h w c", c=_C2), x[:, 0:65, :]
            )
            nc.gpsimd.dma_start(
                t_hi[:].rearrange("h (w c) -> h w c", c=_C2), x[:, 64:128, :]
            )

            out_engines = [nc.sync, nc.scalar]
            n_out = 0
            for q in range(4):
                t = t_lo if q < 2 else t_hi
                base = 32 * q if q < 2 else 32 * (q - 2)
                t3 = t[:].rearrange("h (w c) -> h w c", c=_C2)
                u = u_pool.tile([_H, 64 * _C2], f32, tag="u")
                u3 = u[:].rearrange("h (w c) -> h w c", c=_C2)
                # Quarter cols j=4t+{0,1,2,3} (t=0..15) read input cols
                # base + {2t, 2t, 2t+1, 2t+2} (locals within t_lo/t_hi).
                # View the 64 quarter cols as 32 pairs: even pairs p=2t are
                # cols (4t, 4t+1), odd pairs cols (4t+2, 4t+3).
                up = u3.rearrange("h (p two) c -> h p two c", two=2)
                # A/B fused: dst pairs (4t, 4t+1) <- src col base+2t twice
                # (stride-0 broadcast of the pair dim).
                nc.vector.tensor_copy(
                    up[:, 0:32:2, :, :],
                    t3[:, _sl(base, 2, 16), :]
                    .unsqueeze(2)
                    .broadcast_to([_H, 16, 2, _C2]),
                )
                # C: dst pairs (4t+2, 4t+3) <- src cols (base+2t+1,
                # base+2t+2) contiguous... except the clipped tail in q3.
                nct = 15 if q == 3 else 16
                nc.vector.tensor_copy(
                    up[:, 1 : 2 * nct : 2, :, :],
                    t3[:, base + 1 : base + 2 * nct + 1, :].rearrange(
                        "h (g two) c -> h g two c", two=2
                    ),
                )
                if q == 3:
                    # cols 254, 255 <- input col 127 (local 63) twice.
                    nc.vector.tensor_copy(
                        u3[:, 62:64, :],
                        t3[:, 63:64, :].broadcast_to([_H, 2, _C2]),
                    )
                # Scatter: 4 row families, 32 KiB contiguous descriptors.
                for rd0, rds, rs0, rss, rcnt in _FAMILIES:
                    eng = out_engines[n_out % len(out_engines)]
                    n_out += 1
                    eng.dma_start(
                        y[_sl(rd0, rds, rcnt), 64 * q : 64 * (q + 1), :],
                        u[_sl(rs0, rss, rcnt), :],
                    )
    nc.compile()
    return nc


def _build_nc_v3():
    """v2 + uniform DMA-engine load.

    v2's HWDGE sync ring fed SDMA engines 0-8 ~2x the descriptors of
    9-15, serializing a long tail.  The SWDGE (gpsimd) queue spreads
    descriptors across all 16 engines evenly (observed), so route every
    DMA through it.  Input is loaded as 4 per-quarter column chunks
    (contiguous per row) so each quarter's expansion only waits for its
    own ~2 MiB load.
    """
    import concourse.bacc as bacc
    import concourse.mybir as mybir
    from concourse.tile import TileContext

    f32 = mybir.dt.float32
    nc = bacc.Bacc()
    x = nc.dram_tensor("x", [_H, _W, _C2], f32, kind="ExternalInput")
    y = nc.dram_tensor("y", [_HO, _WO, _C2], f32, kind="ExternalOutput")

    with TileContext(nc) as tc:
        with (
            tc.tile_pool(name="tin", bufs=1) as tin_pool,
            tc.tile_pool(name="uexp", bufs=3) as u_pool,
        ):
            # Quarter q of the output (cols 64q..64q+64) reads input cols
            # 32q..32q+32 inclusive -> 33-col chunks (32 for q3).
            t_chunks = []
            for q in range(4):
                w0 = 32 * q
                w1 = min(w0 + 33, _W)
                t = tin_pool.tile([_H, (w1 - w0) * _C2], f32, tag=f"t{q}")
                nc.gpsimd.dma_start(
                    t[:].rearrange("h (w c) -> h w c", c=_C2), x[:, w0:w1, :]
                )
                t_chunks.append(t)

            for q in range(4):
                t3 = t_chunks[q][:].rearrange("h (w c) -> h w c", c=_C2)
                u = u_pool.tile([_H, 64 * _C2], f32, tag="u")
                u3 = u[:].rearrange("h (w c) -> h w c", c=_C2)
                up = u3.rearrange("h (p two) c -> h p two c", two=2)
                # A/B fused: dst pairs (4t, 4t+1) <- src local col 2t twice.
                nc.vector.tensor_copy(
                    up[:, 0:32:2, :, :],
                    t3[:, _sl(0, 2, 16), :]
                    .unsqueeze(2)
                    .broadcast_to([_H, 16, 2, _C2]),
                )
                # C: dst pairs (4t+2, 4t+3) <- src local cols (2t+1, 2t+2).
                nct = 15 if q == 3 else 16
                nc.vector.tensor_copy(
                    up[:, 1 : 2 * nct : 2, :, :],
                    t3[:, 1 : 2 * nct + 1, :].rearrange(
                        "h (g two) c -> h g two c", two=2
                    ),
                )
                if q == 3:
                    # cols 254, 255 <- input col 127 (local 31) twice.
                    nc.vector.tensor_copy(
                        u3[:, 62:64, :],
                        t3[:, 31:32, :].broadcast_to([_H, 2, _C2]),
                    )
                for rd0, rds, rs0, rss, rcnt in _FAMILIES:
                    nc.gpsimd.dma_start(
                        y[_sl(rd0, rds, rcnt), 64 * q : 64 * (q + 1), :],
                        u[_sl(rs0, rss, rcnt), :],
                    )
    nc.compile()
    return nc


def _build_nc_v4():
    """v3 + DRAM-friendly write sequencing.

    Measured: concurrent 4-family scatter runs at 232 GB/s vs 337 GB/s
    for <=2 interleaved streams (stride-2 row writes are free).  So:
    pass 1 streams the even output rows (one address stream, quarter by
    quarter as expansions finish), pass 2 writes the odd-row families
    with at most ~2 streams in flight, enforced with explicit dep edges.
    All 4 U quarters stay resident (no pool recycling stalls).
    """
    import concourse.bacc as bacc
    import concourse.mybir as mybir
    from concourse.bass import _add_dep_helper
    from concourse.tile import TileContext

    f32 = mybir.dt.float32
    nc = bacc.Bacc()
    x = nc.dram_tensor("x", [_H, _W, _C2], f32, kind="ExternalInput")
    y = nc.dram_tensor("y", [_HO, _WO, _C2], f32, kind="ExternalOutput")

    with TileContext(nc) as tc:
        with (
            tc.tile_pool(name="tin", bufs=1) as tin_pool,
            tc.tile_pool(name="uexp", bufs=1) as u_pool,
        ):
            t3s, u_tiles = [], []
            for q in range(4):
                w0 = 32 * q
                w1 = min(w0 + 33, _W)
                t = tin_pool.tile([_H, (w1 - w0) * _C2], f32, tag=f"t{q}")
                # 128-partition loads stay on SWDGE: HWDGE splits
                # 128-partition DMAs 2:1 across engines 0-8 vs 9-15.
                nc.gpsimd.dma_start(
                    t[:].rearrange("h (w c) -> h w c", c=_C2), x[:, w0:w1, :]
                )
                t3s.append(t[:].rearrange("h (w c) -> h w c", c=_C2))

            # Expansion (DVE) into 4 resident U quarters.
            for q in range(4):
                t3 = t3s[q]
                u = u_pool.tile([_H, 64 * _C2], f32, tag=f"u{q}")
                u_tiles.append(u)
                u3 = u[:].rearrange("h (w c) -> h w c", c=_C2)
                up = u3.rearrange("h (p two) c -> h p two c", two=2)
                nc.vector.tensor_copy(
                    up[:, 0:32:2, :, :],
                    t3[:, _sl(0, 2, 16), :]
                    .unsqueeze(2)
                    .broadcast_to([_H, 16, 2, _C2]),
                )
                nct = 15 if q == 3 else 16
                nc.vector.tensor_copy(
                    up[:, 1 : 2 * nct : 2, :, :],
                    t3[:, 1 : 2 * nct + 1, :].rearrange(
                        "h (g two) c -> h g two c", two=2
                    ),
                )
                if q == 3:
                    nc.vector.tensor_copy(
                        u3[:, 62:64, :],
                        t3[:, 31:32, :].broadcast_to([_H, 2, _C2]),
                    )

            # Pass 1: even output rows.  No deps — expansion completion
            # staggers the quarters naturally (~2 streams in flight max).
            re_insts = []
            for q in range(4):
                rd0, rds, rs0, rss, rcnt = _FAMILIES[0]
                d = nc.gpsimd.dma_start(
                    y[_sl(rd0, rds, rcnt), 64 * q : 64 * (q + 1), :],
                    u_tiles[q][_sl(rs0, rss, rcnt), :],
                )
                re_insts.append(d.ins)
            # Pass 2 on the two HWDGE rings: RO1 family streams on sync,
            # RO2 on scalar — each ring is FIFO, so each family is one
            # continuous ascending address stream (2-stream mix total).
            # One boundary per ring: its first DMA waits for pass 1.
            for fam, eng in ((1, nc.sync), (2, nc.scalar)):
                rd0, rds, rs0, rss, rcnt = _FAMILIES[fam]
                for q in range(4):
                    d = eng.dma_start(
                        y[_sl(rd0, rds, rcnt), 64 * q : 64 * (q + 1), :],
                        u_tiles[q][_sl(rs0, rss, rcnt), :],
                    )
                    if q == 0:
                        for p in re_insts:
                            _add_dep_helper(d.ins, p, True, "pass1->pass2 boundary")
            # row 255 (tiny), after everything on the sync ring
            for q in range(4):
                rd0, rds, rs0, rss, rcnt = _FAMILIES[3]
                nc.sync.dma_start(
                    y[_sl(rd0, rds, rcnt), 64 * q : 64 * (q + 1), :],
                    u_tiles[q][_sl(rs0, rss, rcnt), :],
                )
    nc.compile()
    return nc


def _build_nc_v5(load_engine_name="gpsimd"):
    """Single SWDGE ring, strict FIFO order, no barriers.

    Trace evidence (v4 @166us): HWDGE rings split descriptors ~2:1 (up
    to 3:1) across SDMA engines 0-8 vs 9-15, so the pass-2 odd-row
    families ran at ~210 GB/s on 9 busy engines while 7 idled; loads
    serialized ahead of fam0 on the SWDGE ring and pass2 sat behind an
    all-pass1 barrier (first write byte ~37us).  SWDGE distributes
    descriptors evenly across all 16 engines, and a single FIFO ring
    is exactly one DRAM address stream at all times: load chunks, then
    even rows quarter-by-quarter (expansions complete while the loads
    drain), then the odd-row families back-to-back.  Floor: 42 MiB at
    ~358 GB/s HBM-per-NC = 118us + startup.
    """
    import concourse.bacc as bacc
    import concourse.mybir as mybir
    from concourse.tile import TileContext

    f32 = mybir.dt.float32
    nc = bacc.Bacc()
    x = nc.dram_tensor("x", [_H, _W, _C2], f32, kind="ExternalInput")
    y = nc.dram_tensor("y", [_HO, _WO, _C2], f32, kind="ExternalOutput")

    with TileContext(nc) as tc:
        with (
            tc.tile_pool(name="tin", bufs=1) as tin_pool,
            tc.tile_pool(name="uexp", bufs=1) as u_pool,
        ):
            load_eng = getattr(nc, load_engine_name)
            t3s, u_tiles = [], []
            for q in range(4):
                w0 = 32 * q
                w1 = min(w0 + 33, _W)
                t = tin_pool.tile([_H, (w1 - w0) * _C2], f32, tag=f"t{q}")
                load_eng.dma_start(
                    t[:].rearrange("h (w c) -> h w c", c=_C2), x[:, w0:w1, :]
                )
                t3s.append(t[:].rearrange("h (w c) -> h w c", c=_C2))

            for q in range(4):
                t3 = t3s[q]
                u = u_pool.tile([_H, 64 * _C2], f32, tag=f"u{q}")
                u_tiles.append(u)
                u3 = u[:].rearrange("h (w c) -> h w c", c=_C2)
                up = u3.rearrange("h (p two) c -> h p two c", two=2)
                nc.vector.tensor_copy(
                    up[:, 0:32:2, :, :],
                    t3[:, _sl(0, 2, 16), :]
                    .unsqueeze(2)
                    .broadcast_to([_H, 16, 2, _C2]),
                )
                nct = 15 if q == 3 else 16
                nc.vector.tensor_copy(
                    up[:, 1 : 2 * nct : 2, :, :],
                    t3[:, 1 : 2 * nct + 1, :].rearrange(
                        "h (g two) c -> h g two c", two=2
                    ),
                )
                if q == 3:
                    nc.vector.tensor_copy(
                        u3[:, 62:64, :],
                        t3[:, 31:32, :].broadcast_to([_H, 2, _C2]),
                    )

            # All writes on the single SWDGE FIFO ring, family-major.
            for fam in range(4):
                rd0, rds, rs0, rss, rcnt = _FAMILIES[fam]
                for q in range(4):
                    nc.gpsimd.dma_start(
                        y[_sl(rd0, rds, rcnt), 64 * q : 64 * (q + 1), :],
                        u_tiles[q][_sl(rs0, rss, rcnt), :],
                    )
    nc.compile()
    return nc


def _build_nc_v6():
    """v5 but loads on the sync HWDGE ring, overlapping the SWDGE write
    stream (writes start ~13us instead of ~29us; costs read/write
    stream mixing during the overlap window)."""
    return _build_nc_v5(load_engine_name="sync")


def _build_nc_v7():
    """v6 + merged odd-row writes at 256 KiB pitch.

    Trace evidence (v6 @139us): fam0 (even rows, descriptor pitch
    256 KiB) sustains ~27.5 GB/s/engine (~440 GB/s aggregate), but the
    separate fam1/fam2 passes (pitch 512 KiB) drop to ~13-18 GB/s per
    engine, and the row-255 writes dribble 2 KiB descriptors for the
    last ~15us.  Fix: pair output rows (4k+3, 4k+5), which share source
    row 2k+2, via a stride-0 free-dim broadcast on the SBUF side — one
    DMA per quarter covers odd rows 3..253 with 32 KiB descriptors
    ascending at 256 KiB pitch, exactly like fam0.  Rows 1 and 255 are
    a single 2-descriptor edge DMA per quarter.
    """
    import concourse.bacc as bacc
    import concourse.mybir as mybir
    from concourse.tile import TileContext

    f32 = mybir.dt.float32
    nc = bacc.Bacc()
    x = nc.dram_tensor("x", [_H, _W, _C2], f32, kind="ExternalInput")
    y = nc.dram_tensor("y", [_HO, _WO, _C2], f32, kind="ExternalOutput")

    with TileContext(nc) as tc:
        with (
            tc.tile_pool(name="tin", bufs=1) as tin_pool,
            tc.tile_pool(name="uexp", bufs=1) as u_pool,
        ):
            t3s, u_tiles = [], []
            for q in range(4):
                w0 = 32 * q
                w1 = min(w0 + 33, _W)
                t = tin_pool.tile([_H, (w1 - w0) * _C2], f32, tag=f"t{q}")
                nc.sync.dma_start(
                    t[:].rearrange("h (w c) -> h w c", c=_C2), x[:, w0:w1, :]
                )
                t3s.append(t[:].rearrange("h (w c) -> h w c", c=_C2))

            for q in range(4):
                t3 = t3s[q]
                u = u_pool.tile([_H, 64 * _C2], f32, tag=f"u{q}")
                u_tiles.append(u)
                u3 = u[:].rearrange("h (w c) -> h w c", c=_C2)
                up = u3.rearrange("h (p two) c -> h p two c", two=2)
                nc.vector.tensor_copy(
                    up[:, 0:32:2, :, :],
                    t3[:, _sl(0, 2, 16), :]
                    .unsqueeze(2)
                    .broadcast_to([_H, 16, 2, _C2]),
                )
                nct = 15 if q == 3 else 16
                nc.vector.tensor_copy(
                    up[:, 1 : 2 * nct : 2, :, :],
                    t3[:, 1 : 2 * nct + 1, :].rearrange(
                        "h (g two) c -> h g two c", two=2
                    ),
                )
                if q == 3:
                    nc.vector.tensor_copy(
                        u3[:, 62:64, :],
                        t3[:, 31:32, :].broadcast_to([_H, 2, _C2]),
                    )

            # All writes on the single SWDGE FIFO ring, quarter-major so
            # quarter q's stream starts as soon as its expansion lands.
            for q in range(4):
                u = u_tiles[q]
                cols = slice(64 * q, 64 * (q + 1))
                # Even rows 0,2,...,254 <- u[0..127]: 128 descs, 256 KiB pitch.
                nc.gpsimd.dma_start(y[_sl(0, 2, 128), cols, :], u[:, :])
                # Odd rows 3..253: pairs (4k+3, 4k+5) <- u[2k+2] twice.
                nc.gpsimd.dma_start(
                    y[_sl(3, 2, 126), cols, :].rearrange(
                        "(k two) w c -> k two w c", two=2
                    ),
                    u[_sl(2, 2, 63), :].unsqueeze(1).broadcast_to([63, 2, 64 * _C2]),
                )
                # Edge rows (1, 255) <- u[(0, 127)]: 2 descs.
                nc.gpsimd.dma_start(
                    y[_sl(1, 254, 2), cols, :], u[_sl(0, 127, 2), :]
                )
    nc.compile()
    return nc


def _build_nc_v9():
    """Column-HALF U tiles -> 64 KiB write descriptors.

    v7 showed stride-0 broadcast source descriptors drain at ~13 GB/s
    per engine — dead end.  Back to v6's two-pass row families, but the
    expanded image is staged as two half-width tiles (u_L = output cols
    0..127, u_R = 128..255; 64 KiB per partition each), so every write
    descriptor is 64 KiB (4x v6) and the whole output takes ~510
    descriptors instead of ~1030.  fam0_L starts once quarters 0-1 are
    expanded (~21us).  All writes on the single SWDGE FIFO ring.
    """
    import concourse.bacc as bacc
    import concourse.mybir as mybir
    from concourse.tile import TileContext

    f32 = mybir.dt.float32
    nc = bacc.Bacc()
    x = nc.dram_tensor("x", [_H, _W, _C2], f32, kind="ExternalInput")
    y = nc.dram_tensor("y", [_HO, _WO, _C2], f32, kind="ExternalOutput")

    with TileContext(nc) as tc:
        with (
            tc.tile_pool(name="tin", bufs=1) as tin_pool,
            tc.tile_pool(name="uexp", bufs=1) as u_pool,
        ):
            t3s = []
            for q in range(4):
                w0 = 32 * q
                w1 = min(w0 + 33, _W)
                t = tin_pool.tile([_H, (w1 - w0) * _C2], f32, tag=f"t{q}")
                nc.sync.dma_start(
                    t[:].rearrange("h (w c) -> h w c", c=_C2), x[:, w0:w1, :]
                )
                t3s.append(t[:].rearrange("h (w c) -> h w c", c=_C2))

            u_halves = [
                u_pool.tile([_H, 128 * _C2], f32, tag="uL", name="uL"),
                u_pool.tile([_H, 128 * _C2], f32, tag="uR", name="uR"),
            ]
            for q in range(4):
                t3 = t3s[q]
                u3h = u_halves[q // 2][:].rearrange("h (w c) -> h w c", c=_C2)
                u3 = u3h[:, 64 * (q % 2) : 64 * (q % 2) + 64, :]
                up = u3.rearrange("h (p two) c -> h p two c", two=2)
                nc.vector.tensor_copy(
                    up[:, 0:32:2, :, :],
                    t3[:, _sl(0, 2, 16), :]
                    .unsqueeze(2)
                    .broadcast_to([_H, 16, 2, _C2]),
                )
                nct = 15 if q == 3 else 16
                nc.vector.tensor_copy(
                    up[:, 1 : 2 * nct : 2, :, :],
                    t3[:, 1 : 2 * nct + 1, :].rearrange(
                        "h (g two) c -> h g two c", two=2
                    ),
                )
                if q == 3:
                    nc.vector.tensor_copy(
                        u3[:, 62:64, :],
                        t3[:, 31:32, :].broadcast_to([_H, 2, _C2]),
                    )

            # Writes: single SWDGE FIFO ring, family-major, halves inner.
            for fam in range(4):
                rd0, rds, rs0, rss, rcnt = _FAMILIES[fam]
                for h in range(2):
                    cols = slice(128 * h, 128 * (h + 1))
                    nc.gpsimd.dma_start(
                        y[_sl(rd0, rds, rcnt), cols, :],
                        u_halves[h][_sl(rs0, rss, rcnt), :],
                    )
    nc.compile()
    return nc


def _build_nc_v10():
    """Loads first on the SWDGE ring + half-width U + odd-pair broadcast.

    v9 lesson: writes starve concurrent HWDGE loads (packet round-robin
    shares engines, bandwidth goes to whoever has descriptors), so late
    chunks -> late expansions -> 14us ring stall.  Put the loads at the
    head of the one SWDGE FIFO ring; all expansions finish while the
    8.6 MB load drains, so the write stream that follows never stalls.
    Writes are 64 KiB descriptors (half-width U tiles): even rows
    (256 KiB pitch), then odd rows 3..253 merged via stride-0 pair
    broadcast (256 KiB pitch), then 2-descriptor edge rows (1, 255).
    """
    import concourse.bacc as bacc
    import concourse.mybir as mybir
    from concourse.tile import TileContext

    f32 = mybir.dt.float32
    nc = bacc.Bacc()
    x = nc.dram_tensor("x", [_H, _W, _C2], f32, kind="ExternalInput")
    y = nc.dram_tensor("y", [_HO, _WO, _C2], f32, kind="ExternalOutput")

    with TileContext(nc) as tc:
        with (
            tc.tile_pool(name="tin", bufs=1) as tin_pool,
            tc.tile_pool(name="uexp", bufs=1) as u_pool,
        ):
            t3s = []
            for q in range(4):
                w0 = 32 * q
                w1 = min(w0 + 33, _W)
                t = tin_pool.tile([_H, (w1 - w0) * _C2], f32, tag=f"t{q}")
                nc.gpsimd.dma_start(
                    t[:].rearrange("h (w c) -> h w c", c=_C2), x[:, w0:w1, :]
                )
                t3s.append(t[:].rearrange("h (w c) -> h w c", c=_C2))

            u_halves = [
                u_pool.tile([_H, 128 * _C2], f32, tag="uL", name="uL"),
                u_pool.tile([_H, 128 * _C2], f32, tag="uR", name="uR"),
            ]
            for q in range(4):
                t3 = t3s[q]
                u3h = u_halves[q // 2][:].rearrange("h (w c) -> h w c", c=_C2)
                u3 = u3h[:, 64 * (q % 2) : 64 * (q % 2) + 64, :]
                up = u3.rearrange("h (p two) c -> h p two c", two=2)
                nc.vector.tensor_copy(
                    up[:, 0:32:2, :, :],
                    t3[:, _sl(0, 2, 16), :]
                    .unsqueeze(2)
                    .broadcast_to([_H, 16, 2, _C2]),
                )
                nct = 15 if q == 3 else 16
                nc.vector.tensor_copy(
                    up[:, 1 : 2 * nct : 2, :, :],
                    t3[:, 1 : 2 * nct + 1, :].rearrange(
                        "h (g two) c -> h g two c", two=2
                    ),
                )
                if q == 3:
                    nc.vector.tensor_copy(
                        u3[:, 62:64, :],
                        t3[:, 31:32, :].broadcast_to([_H, 2, _C2]),
                    )

            for h in range(2):
                u = u_halves[h]
                cols = slice(128 * h, 128 * (h + 1))
                # Even rows 0..254: 128 descs of 64 KiB, 256 KiB pitch.
                nc.gpsimd.dma_start(y[_sl(0, 2, 128), cols, :], u[:, :])
                # Edge rows (1, 255) <- u[(0, 127)]: 2 descs (mid-stream).
                nc.gpsimd.dma_start(y[_sl(1, 254, 2), cols, :], u[_sl(0, 127, 2), :])
                # Odd rows 3..253: pairs (4k+3, 4k+5) <- u[2k+2] twice,
                # 126 descs of 64 KiB, 256 KiB pitch.
                nc.gpsimd.dma_start(
                    y[_sl(3, 2, 126), cols, :].rearrange(
                        "(k two) w c -> k two w c", two=2
                    ),
                    u[_sl(2, 2, 63), :].unsqueeze(1).broadcast_to([63, 2, 128 * _C2]),
                )
    nc.compile()
    return nc


VERSION = 10
_BUILDERS = {
    1: _build_nc_v1,
    2: _build_nc_v2,
    3: _build_nc_v3,
    4: _build_nc_v4,
    5: _build_nc_v5,
    6: _build_nc_v6,
    7: _build_nc_v7,
    9: _build_nc_v9,
    10: _build_nc_v10,
}


def _selftest_families():
    """Host-side check: the family decomposition reproduces the reference
    round-half-to-even nearest index map exactly."""
    idx = np.round(128 * np.arange(256, dtype=np.float64) / 256.0)
    # np.round is round-half-to-even like jnp.round
    idx = np.clip(idx.astype(np.int64), 0, 127)
    recon = np.full(256, -1)
    for d0, ds, s0, ss, c in _FAMILIES:
        for i in range(c):
            assert recon[d0 + ds * i] == -1
            recon[d0 + ds * i] = s0 + ss * i
    assert (recon == idx).all()


_selftest_families()


def _build_nc():
    return _BUILDERS[VERSION]()


def _get_nc():
    if VERSION not in _NC_CACHE:
        _NC_CACHE[VERSION] = _build_nc()
    return _NC_CACHE[VERSION]


def kernel(x_real: np.ndarray, x_imag: np.ndarray) -> np.ndarray:
    global LAST_RESULT
    _ensure_axon_ntff_hook()
    from concourse.bass_utils import run_bass_kernel_spmd

    assert x_real.shape == (_B, _H, _W, _C) and x_imag.shape == (_B, _H, _W, _C)

    # Interleave real/imag channel-wise: f32 [B, H, W, 2C]; pairs
    # (re, im) match the complex64 memory layout.
    xc = np.empty((_B, _H, _W, _C, 2), np.float32)
    xc[..., 0] = x_real
    xc[..., 1] = x_imag
    xc = xc.reshape(_B, _H, _W, _C2)

    nc = _get_nc()
    in_maps = [{"x": xc[b]} for b in range(_B)]
    res = run_bass_kernel_spmd(
        nc,
        in_maps,
        core_ids=list(range(_N_CORES)),
        trace=TRACE,
    )
    LAST_RESULT = res

    out = np.stack([res.results[b]["y"] for b in range(_B)])
    # [B, 256, 256, 128] f32 -> complex64 view [B, 256, 256, 64]
    return out.view(np.complex64)



# revision 10
# speedup vs baseline: 1.1007x; 1.1007x over previous
"""Complex nearest-neighbor 2x spatial upsample on 8 TRN2 NeuronCores.

Reference op: x = x_real + 1j*x_imag, shape [8, 128, 128, 64] (B,H,W,C);
out[b, j, k, c] = x[b, r(j), r(k), c] with
r(j) = clip(round_half_to_even(j/2), 0, 127), output [8, 256, 256, 64]
complex64.

Strategy (batch-sharded, 1 sample per core):
  - Host: interleave real/imag into f32 [H, W, 2C] so a complex "pixel"
    is one contiguous 512B chunk and the complex64 output is a pure view.
  - Device: stage the 8 MiB sample in SBUF (128 rows -> 128 partitions),
    then scatter to the 32 MiB output with strided DMAs.  The
    round-half-to-even gather decomposes exactly into 4 affine families
    per axis, so 4x4 = 16 DRAM-write DMAs with 3-dim access patterns
    (rows, cols, 512B contiguous pixel) cover the whole output.
"""

import numpy as np

_B, _H, _W, _C = 8, 128, 128, 64
_C2 = 2 * _C
_HO, _WO = 2 * _H, 2 * _W
_N_CORES = 8

# Affine families of j -> r(j) = clip(round_half_even(j/2), 0, 127), j in [0,256):
#   j = 2m   -> m      (m = 0..127)
#   j = 4t+1 -> 2t     (t = 0..63)
#   j = 4t+3 -> 2t+2   (t = 0..62)
#   j = 255  -> 127
# Tuples: (dst_start, dst_step, src_start, src_step, count)
_FAMILIES = [
    (0, 2, 0, 1, 128),
    (1, 4, 0, 2, 64),
    (3, 4, 2, 2, 63),
    (255, 1, 127, 1, 1),
]

# Set by test harnesses: TRACE=True makes kernel() profile the run and
# stash the BassKernelResults (incl. exec_time_ns) in LAST_RESULT.
TRACE = False
LAST_RESULT = None

_NC_CACHE = {}


def _ensure_axon_ntff_hook():
    """Provide antenv.axon_hooks when the image ships only the antenv stub.

    concourse.bass_utils imports it for trace=True under axon; the slim
    agent image's boot fails to register the hook because the stub antenv
    package has no axon_hooks submodule.  Recreate the ctypes-based NTFF
    hook against libaxon_pjrt.so (same recipe as trn_agent_boot.trn_boot).
    """
    try:
        import antenv.axon_hooks  # noqa: F401

        return
    except ImportError:
        pass

    import contextlib
    import ctypes
    import sys
    import types

    mod = types.ModuleType("antenv.axon_hooks")
    holder = {"hook": None}

    def set_axon_ntff_profile_hook(hook):
        holder["hook"] = hook

    def get_axon_ntff_profile_hook():
        return holder["hook"]

    mod.set_axon_ntff_profile_hook = set_axon_ntff_profile_hook
    mod.get_axon_ntff_profile_hook = get_axon_ntff_profile_hook
    sys.modules["antenv.axon_hooks"] = mod
    try:
        import antenv

        antenv.axon_hooks = mod
    except ImportError:
        pass

    so_path = "/opt/axon/libaxon_pjrt.so"
    try:
        lib = ctypes.CDLL(so_path)
    except OSError:
        return
    if not hasattr(lib, "axon_start_nrt_profile"):
        return
    lib.axon_start_nrt_profile.argtypes = [
        ctypes.POINTER(ctypes.c_int64),
        ctypes.c_size_t,
    ]
    lib.axon_start_nrt_profile.restype = ctypes.c_int64
    lib.axon_stop_nrt_profile.argtypes = [ctypes.c_char_p]
    lib.axon_stop_nrt_profile.restype = ctypes.c_int64

    @contextlib.contextmanager
    def _hook(output_dir, device_ids):
        import jax

        jax.devices()
        if device_ids:
            ids = (ctypes.c_int64 * len(device_ids))(*device_ids)
            rc = lib.axon_start_nrt_profile(ids, len(device_ids))
        else:
            rc = lib.axon_start_nrt_profile(None, 0)
        if rc != 0:
            raise RuntimeError(f"axon_start_nrt_profile rc={rc}")
        try:
            yield
        finally:
            n = lib.axon_stop_nrt_profile(str(output_dir).encode())
            if n < 0:
                raise RuntimeError(f"axon_stop_nrt_profile rc={n}")

    set_axon_ntff_profile_hook(_hook)


def _sl(start, step, count):
    return slice(start, start + (count - 1) * step + 1, step)


def _build_nc_v1():
    """Pure-DMA scatter: 16 strided DMAs with 512B descriptors.

    Measured 165 us/core: descriptor-rate limited (all 16 SDMA engines
    ~100% busy at ~30 ns per 512B descriptor)."""
    import concourse.bacc as bacc
    import concourse.mybir as mybir
    from concourse.tile import TileContext

    nc = bacc.Bacc()
    x = nc.dram_tensor("x", [_H, _W, _C2], mybir.dt.float32, kind="ExternalInput")
    y = nc.dram_tensor("y", [_HO, _WO, _C2], mybir.dt.float32, kind="ExternalOutput")

    with TileContext(nc) as tc:
        with tc.tile_pool(name="stage", bufs=1) as pool:
            t = pool.tile([_H, _W * _C2], mybir.dt.float32)
            t3 = t[:].rearrange("h (w c) -> h w c", c=_C2)
            # 8 MiB load: one contiguous 64 KiB row per partition.
            nc.sync.dma_start(t[:], x[:].rearrange("h w c -> h (w c)"))
            # 16 strided scatter DMAs, alternating between the two HWDGE
            # rings (sync + scalar) so they drain in parallel.
            engines = [nc.sync, nc.scalar]
            i = 0
            for rd0, rds, rs0, rss, rc in _FAMILIES:
                for cd0, cds, cs0, css, cc in _FAMILIES:
                    eng = engines[i % len(engines)]
                    i += 1
                    eng.dma_start(
                        y[_sl(rd0, rds, rc), _sl(cd0, cds, cc), :],
                        t3[_sl(rs0, rss, rc), _sl(cs0, css, cc), :],
                    )
    nc.compile()
    return nc


def _build_nc_v2():
    """On-chip column expansion + contiguous-row scatter.

    Input rows live one-per-partition.  The vector engine expands the
    column (W) axis into U tiles (64 output cols per quarter, 32 KiB per
    partition), then each quarter is written out with 4 row-family DMAs
    whose descriptors are 32 KiB contiguous — DMA runs at line rate
    instead of the 512B descriptor floor of v1.
    """
    import concourse.bacc as bacc
    import concourse.mybir as mybir
    from concourse.tile import TileContext

    f32 = mybir.dt.float32
    nc = bacc.Bacc()
    x = nc.dram_tensor("x", [_H, _W, _C2], f32, kind="ExternalInput")
    y = nc.dram_tensor("y", [_HO, _WO, _C2], f32, kind="ExternalOutput")

    with TileContext(nc) as tc:
        with (
            tc.tile_pool(name="tin", bufs=1) as tin_pool,
            tc.tile_pool(name="uexp", bufs=3) as u_pool,
        ):
            # Input halves: t_lo = cols 0..64 (65 cols, needed by output
            # quarters 0-1), t_hi = cols 64..127 (needed by quarters 2-3).
            t_lo = tin_pool.tile([_H, 65 * _C2], f32, tag="tlo")
            t_hi = tin_pool.tile([_H, 64 * _C2], f32, tag="thi")
            nc.gpsimd.dma_start(
                t_lo[:].rearrange("h (w c) -> h w c", c=_C2), x[:, 0:65, :]
            )
            nc.gpsimd.dma_start(
                t_hi[:].rearrange("h (w c) -> h w c", c=_C2), x[:, 64:128, :]
            )

            out_engines = [nc.sync, nc.scalar]
            n_out = 0
            for q in range(4):
                t = t_lo if q < 2 else t_hi
                base = 32 * q if q < 2 else 32 * (q - 2)
                t3 = t[:].rearrange("h (w c) -> h w c", c=_C2)
                u = u_pool.tile([_H, 64 * _C2], f32, tag="u")
                u3 = u[:].rearrange("h (w c) -> h w c", c=_C2)
                # Quarter cols j=4t+{0,1,2,3} (t=0..15) read input cols
                # base + {2t, 2t, 2t+1, 2t+2} (locals within t_lo/t_hi).
                # View the 64 quarter cols as 32 pairs: even pairs p=2t are
                # cols (4t, 4t+1), odd pairs cols (4t+2, 4t+3).
                up = u3.rearrange("h (p two) c -> h p two c", two=2)
                # A/B fused: dst pairs (4t, 4t+1) <- src col base+2t twice
                # (stride-0 broadcast of the pair dim).
                nc.vector.tensor_copy(
                    up[:, 0:32:2, :, :],
                    t3[:, _sl(base, 2, 16), :]
                    .unsqueeze(2)
                    .broadcast_to([_H, 16, 2, _C2]),
                )
                # C: dst pairs (4t+2, 4t+3) <- src cols (base+2t+1,
                # base+2t+2) contiguous... except the clipped tail in q3.
                nct = 15 if q == 3 else 16
                nc.vector.tensor_copy(
                    up[:, 1 : 2 * nct : 2, :, :],
                    t3[:, base + 1 : base + 2 * nct + 1, :].rearrange(
                        "h (g two) c -> h g two c", two=2
                    ),
                )
                if q == 3:
                    # cols 254, 255 <- input col 127 (local 63) twice.
                    nc.vector.tensor_copy(
                        u3[:, 62:64, :],
                        t3[:, 63:64, :].broadcast_to([_H, 2, _C2]),
                    )
                # Scatter: 4 row families, 32 KiB contiguous descriptors.
                for rd0, rds, rs0, rss, rcnt in _FAMILIES:
                    eng = out_engines[n_out % len(out_engines)]
                    n_out += 1
                    eng.dma_start(
                        y[_sl(rd0, rds, rcnt), 64 * q : 64 * (q + 1), :],
                        u[_sl(rs0, rss, rcnt), :],
                    )
    nc.compile()
    return nc


def _build_nc_v3():
    """v2 + uniform DMA-engine load.

    v2's HWDGE sync ring fed SDMA engines 0-8 ~2x the descriptors of
    9-15, serializing a long tail.  The SWDGE (gpsimd) queue spreads
    descriptors across all 16 engines evenly (observed), so route every
    DMA through it.  Input is loaded as 4 per-quarter column chunks
    (contiguous per row) so each quarter's expansion only waits for its
    own ~2 MiB load.
    """
    import concourse.bacc as bacc
    import concourse.mybir as mybir
    from concourse.tile import TileContext

    f32 = mybir.dt.float32
    nc = bacc.Bacc()
    x = nc.dram_tensor("x", [_H, _W, _C2], f32, kind="ExternalInput")
    y = nc.dram_tensor("y", [_HO, _WO, _C2], f32, kind="ExternalOutput")

    with TileContext(nc) as tc:
        with (
            tc.tile_pool(name="tin", bufs=1) as tin_pool,
            tc.tile_pool(name="uexp", bufs=3) as u_pool,
        ):
            # Quarter q of the output (cols 64q..64q+64) reads input cols
            # 32q..32q+32 inclusive -> 33-col chunks (32 for q3).
            t_chunks = []
            for q in range(4):
                w0 = 32 * q
                w1 = min(w0 + 33, _W)
                t = tin_pool.tile([_H, (w1 - w0) * _C2], f32, tag=f"t{q}")
                nc.gpsimd.dma_start(
                    t[:].rearrange("h (w c) -> h w c", c=_C2), x[:, w0:w1, :]
                )
                t_chunks.append(t)

            for q in range(4):
                t3 = t_chunks[q][:].rearrange("h (w c) -> h w c", c=_C2)
                u = u_pool.tile([_H, 64 * _C2], f32, tag="u")
                u3 = u[:].rearrange("h (w c) -> h w c", c=_C2)
                up = u3.rearrange("h (p two) c -> h p two c", two=2)
                # A/B fused: dst pairs (4t, 4t+1) <- src local col 2t twice.
                nc.vector.tensor_copy(
                    up[:, 0:32:2, :, :],
                    t3[:, _sl(0, 2, 16), :]
                    .unsqueeze(2)
                    .broadcast_to([_H, 16, 2, _C2]),
                )
                # C: dst pairs (4t+2, 4t+3) <- src local cols (2t+1, 2t+2).
                nct = 15 if q == 3 else 16
                nc.vector.tensor_copy(
                    up[:, 1 : 2 * nct : 2, :, :],
                    t3[:, 1 : 2 * nct + 1, :].rearrange(
                        "h (g two) c -> h g two c", two=2
                    ),
                )
                if q == 3:
                    # cols 254, 255 <- input col 127 (local 31) twice.
                    nc.vector.tensor_copy(
                        u3[:, 62:64, :],
                        t3[:, 31:32, :].broadcast_to([_H, 2, _C2]),
                    )
                for rd0, rds, rs0, rss, rcnt in _FAMILIES:
                    nc.gpsimd.dma_start(
                        y[_sl(rd0, rds, rcnt), 64 * q : 64 * (q + 1), :],
                        u[_sl(rs0, rss, rcnt), :],
                    )
    nc.compile()
    return nc


def _build_nc_v4():
    """v3 + DRAM-friendly write sequencing.

    Measured: concurrent 4-family scatter runs at 232 GB/s vs 337 GB/s
    for <=2 interleaved streams (stride-2 row writes are free).  So:
    pass 1 streams the even output rows (one address stream, quarter by
    quarter as expansions finish), pass 2 writes the odd-row families
    with at most ~2 streams in flight, enforced with explicit dep edges.
    All 4 U quarters stay resident (no pool recycling stalls).
    """
    import concourse.bacc as bacc
    import concourse.mybir as mybir
    from concourse.bass import _add_dep_helper
    from concourse.tile import TileContext

    f32 = mybir.dt.float32
    nc = bacc.Bacc()
    x = nc.dram_tensor("x", [_H, _W, _C2], f32, kind="ExternalInput")
    y = nc.dram_tensor("y", [_HO, _WO, _C2], f32, kind="ExternalOutput")

    with TileContext(nc) as tc:
        with (
            tc.tile_pool(name="tin", bufs=1) as tin_pool,
            tc.tile_pool(name="uexp", bufs=1) as u_pool,
        ):
            t3s, u_tiles = [], []
            for q in range(4):
                w0 = 32 * q
                w1 = min(w0 + 33, _W)
                t = tin_pool.tile([_H, (w1 - w0) * _C2], f32, tag=f"t{q}")
                # 128-partition loads stay on SWDGE: HWDGE splits
                # 128-partition DMAs 2:1 across engines 0-8 vs 9-15.
                nc.gpsimd.dma_start(
                    t[:].rearrange("h (w c) -> h w c", c=_C2), x[:, w0:w1, :]
                )
                t3s.append(t[:].rearrange("h (w c) -> h w c", c=_C2))

            # Expansion (DVE) into 4 resident U quarters.
            for q in range(4):
                t3 = t3s[q]
                u = u_pool.tile([_H, 64 * _C2], f32, tag=f"u{q}")
                u_tiles.append(u)
                u3 = u[:].rearrange("h (w c) -> h w c", c=_C2)
                up = u3.rearrange("h (p two) c -> h p two c", two=2)
                nc.vector.tensor_copy(
                    up[:, 0:32:2, :, :],
                    t3[:, _sl(0, 2, 16), :]
                    .unsqueeze(2)
                    .broadcast_to([_H, 16, 2, _C2]),
                )
                nct = 15 if q == 3 else 16
                nc.vector.tensor_copy(
                    up[:, 1 : 2 * nct : 2, :, :],
                    t3[:, 1 : 2 * nct + 1, :].rearrange(
                        "h (g two) c -> h g two c", two=2
                    ),
                )
                if q == 3:
                    nc.vector.tensor_copy(
                        u3[:, 62:64, :],
                        t3[:, 31:32, :].broadcast_to([_H, 2, _C2]),
                    )

            # Pass 1: even output rows.  No deps — expansion completion
            # staggers the quarters naturally (~2 streams in flight max).
            re_insts = []
            for q in range(4):
                rd0, rds, rs0, rss, rcnt = _FAMILIES[0]
                d = nc.gpsimd.dma_start(
                    y[_sl(rd0, rds, rcnt), 64 * q : 64 * (q + 1), :],
                    u_tiles[q][_sl(rs0, rss, rcnt), :],
                )
                re_insts.append(d.ins)
            # Pass 2 on the two HWDGE rings: RO1 family streams on sync,
            # RO2 on scalar — each ring is FIFO, so each family is one
            # continuous ascending address stream (2-stream mix total).
            # One boundary per ring: its first DMA waits for pass 1.
            for fam, eng in ((1, nc.sync), (2, nc.scalar)):
                rd0, rds, rs0, rss, rcnt = _FAMILIES[fam]
                for q in range(4):
                    d = eng.dma_start(
                        y[_sl(rd0, rds, rcnt), 64 * q : 64 * (q + 1), :],
                        u_tiles[q][_sl(rs0, rss, rcnt), :],
                    )
                    if q == 0:
                        for p in re_insts:
                            _add_dep_helper(d.ins, p, True, "pass1->pass2 boundary")
            # row 255 (tiny), after everything on the sync ring
            for q in range(4):
                rd0, rds, rs0, rss, rcnt = _FAMILIES[3]
                nc.sync.dma_start(
                    y[_sl(rd0, rds, rcnt), 64 * q : 64 * (q + 1), :],
                    u_tiles[q][_sl(rs0, rss, rcnt), :],
                )
    nc.compile()
    return nc


def _build_nc_v5(load_engine_name="gpsimd"):
    """Single SWDGE ring, strict FIFO order, no barriers.

    Trace evidence (v4 @166us): HWDGE rings split descriptors ~2:1 (up
    to 3:1) across SDMA engines 0-8 vs 9-15, so the pass-2 odd-row
    families ran at ~210 GB/s on 9 busy engines while 7 idled; loads
    serialized ahead of fam0 on the SWDGE ring and pass2 sat behind an
    all-pass1 barrier (first write byte ~37us).  SWDGE distributes
    descriptors evenly across all 16 engines, and a single FIFO ring
    is exactly one DRAM address stream at all times: load chunks, then
    even rows quarter-by-quarter (expansions complete while the loads
    drain), then the odd-row families back-to-back.  Floor: 42 MiB at
    ~358 GB/s HBM-per-NC = 118us + startup.
    """
    import concourse.bacc as bacc
    import concourse.mybir as mybir
    from concourse.tile import TileContext

    f32 = mybir.dt.float32
    nc = bacc.Bacc()
    x = nc.dram_tensor("x", [_H, _W, _C2], f32, kind="ExternalInput")
    y = nc.dram_tensor("y", [_HO, _WO, _C2], f32, kind="ExternalOutput")

    with TileContext(nc) as tc:
        with (
            tc.tile_pool(name="tin", bufs=1) as tin_pool,
            tc.tile_pool(name="uexp", bufs=1) as u_pool,
        ):
            load_eng = getattr(nc, load_engine_name)
            t3s, u_tiles = [], []
            for q in range(4):
                w0 = 32 * q
                w1 = min(w0 + 33, _W)
                t = tin_pool.tile([_H, (w1 - w0) * _C2], f32, tag=f"t{q}")
                load_eng.dma_start(
                    t[:].rearrange("h (w c) -> h w c", c=_C2), x[:, w0:w1, :]
                )
                t3s.append(t[:].rearrange("h (w c) -> h w c", c=_C2))

            for q in range(4):
                t3 = t3s[q]
                u = u_pool.tile([_H, 64 * _C2], f32, tag=f"u{q}")
                u_tiles.append(u)
                u3 = u[:].rearrange("h (w c) -> h w c", c=_C2)
                up = u3.rearrange("h (p two) c -> h p two c", two=2)
                nc.vector.tensor_copy(
                    up[:, 0:32:2, :, :],
                    t3[:, _sl(0, 2, 16), :]
                    .unsqueeze(2)
                    .broadcast_to([_H, 16, 2, _C2]),
                )
                nct = 15 if q == 3 else 16
                nc.vector.tensor_copy(
                    up[:, 1 : 2 * nct : 2, :, :],
                    t3[:, 1 : 2 * nct + 1, :].rearrange(
                        "h (g two) c -> h g two c", two=2
                    ),
                )
                if q == 3:
                    nc.vector.tensor_copy(
                        u3[:, 62:64, :],
                        t3[:, 31:32, :].broadcast_to([_H, 2, _C2]),
                    )

            # All writes on the single SWDGE FIFO ring, family-major.
            for fam in range(4):
                rd0, rds, rs0, rss, rcnt = _FAMILIES[fam]
                for q in range(4):
                    nc.gpsimd.dma_start(
                        y[_sl(rd0, rds, rcnt), 64 * q : 64 * (q + 1), :],
                        u_tiles[q][_sl(rs0, rss, rcnt), :],
                    )
    nc.compile()
    return nc


def _build_nc_v6():
    """v5 but loads on the sync HWDGE ring, overlapping the SWDGE write
    stream (writes start ~13us instead of ~29us; costs read/write
    stream mixing during the overlap window)."""
    return _build_nc_v5(load_engine_name="sync")


def _build_nc_v7():
    """v6 + merged odd-row writes at 256 KiB pitch.

    Trace evidence (v6 @139us): fam0 (even rows, descriptor pitch
    256 KiB) sustains ~27.5 GB/s/engine (~440 GB/s aggregate), but the
    separate fam1/fam2 passes (pitch 512 KiB) drop to ~13-18 GB/s per
    engine, and the row-255 writes dribble 2 KiB descriptors for the
    last ~15us.  Fix: pair output rows (4k+3, 4k+5), which share source
    row 2k+2, via a stride-0 free-dim broadcast on the SBUF side — one
    DMA per quarter covers odd rows 3..253 with 32 KiB descriptors
    ascending at 256 KiB pitch, exactly like fam0.  Rows 1 and 255 are
    a single 2-descriptor edge DMA per quarter.
    """
    import concourse.bacc as bacc
    import concourse.mybir as mybir
    from concourse.tile import TileContext

    f32 = mybir.dt.float32
    nc = bacc.Bacc()
    x = nc.dram_tensor("x", [_H, _W, _C2], f32, kind="ExternalInput")
    y = nc.dram_tensor("y", [_HO, _WO, _C2], f32, kind="ExternalOutput")

    with TileContext(nc) as tc:
        with (
            tc.tile_pool(name="tin", bufs=1) as tin_pool,
            tc.tile_pool(name="uexp", bufs=1) as u_pool,
        ):
            t3s, u_tiles = [], []
            for q in range(4):
                w0 = 32 * q
                w1 = min(w0 + 33, _W)
                t = tin_pool.tile([_H, (w1 - w0) * _C2], f32, tag=f"t{q}")
                nc.sync.dma_start(
                    t[:].rearrange("h (w c) -> h w c", c=_C2), x[:, w0:w1, :]
                )
                t3s.append(t[:].rearrange("h (w c) -> h w c", c=_C2))

            for q in range(4):
                t3 = t3s[q]
                u = u_pool.tile([_H, 64 * _C2], f32, tag=f"u{q}")
                u_tiles.append(u)
                u3 = u[:].rearrange("h (w c) -> h w c", c=_C2)
                up = u3.rearrange("h (p two) c -> h p two c", two=2)
                nc.vector.tensor_copy(
                    up[:, 0:32:2, :, :],
                    t3[:, _sl(0, 2, 16), :]
                    .unsqueeze(2)
                    .broadcast_to([_H, 16, 2, _C2]),
                )
                nct = 15 if q == 3 else 16
                nc.vector.tensor_copy(
                    up[:, 1 : 2 * nct : 2, :, :],
                    t3[:, 1 : 2 * nct + 1, :].rearrange(
                        "h (g two) c -> h g two c", two=2
                    ),
                )
                if q == 3:
                    nc.vector.tensor_copy(
                        u3[:, 62:64, :],
                        t3[:, 31:32, :].broadcast_to([_H, 2, _C2]),
                    )

            # All writes on the single SWDGE FIFO ring, quarter-major so
            # quarter q's stream starts as soon as its expansion lands.
            for q in range(4):
                u = u_tiles[q]
                cols = slice(64 * q, 64 * (q + 1))
                # Even rows 0,2,...,254 <- u[0..127]: 128 descs, 256 KiB pitch.
                nc.gpsimd.dma_start(y[_sl(0, 2, 128), cols, :], u[:, :])
                # Odd rows 3..253: pairs (4k+3, 4k+5) <- u[2k+2] twice.
                nc.gpsimd.dma_start(
                    y[_sl(3, 2, 126), cols, :].rearrange(
                        "(k two) w c -> k two w c", two=2
                    ),
                    u[_sl(2, 2, 63), :].unsqueeze(1).broadcast_to([63, 2, 64 * _C2]),
                )
                # Edge rows (1, 255) <- u[(0, 127)]: 2 descs.
                nc.gpsimd.dma_start(
                    y[_sl(1, 254, 2), cols, :], u[_sl(0, 127, 2), :]
                )
    nc.compile()
    return nc


def _build_nc_v9():
    """Column-HALF U tiles -> 64 KiB write descriptors.

    v7 showed stride-0 broadcast source descriptors drain at ~13 GB/s
    per engine — dead end.  Back to v6's two-pass row families, but the
    expanded image is staged as two half-width tiles (u_L = output cols
    0..127, u_R = 128..255; 64 KiB per partition each), so every write
    descriptor is 64 KiB (4x v6) and the whole output takes ~510
    descriptors instead of ~1030.  fam0_L starts once quarters 0-1 are
    expanded (~21us).  All writes on the single SWDGE FIFO ring.
    """
    import concourse.bacc as bacc
    import concourse.mybir as mybir
    from concourse.tile import TileContext

    f32 = mybir.dt.float32
    nc = bacc.Bacc()
    x = nc.dram_tensor("x", [_H, _W, _C2], f32, kind="ExternalInput")
    y = nc.dram_tensor("y", [_HO, _WO, _C2], f32, kind="ExternalOutput")

    with TileContext(nc) as tc:
        with (
            tc.tile_pool(name="tin", bufs=1) as tin_pool,
            tc.tile_pool(name="uexp", bufs=1) as u_pool,
        ):
            t3s = []
            for q in range(4):
                w0 = 32 * q
                w1 = min(w0 + 33, _W)
                t = tin_pool.tile([_H, (w1 - w0) * _C2], f32, tag=f"t{q}")
                nc.sync.dma_start(
                    t[:].rearrange("h (w c) -> h w c", c=_C2), x[:, w0:w1, :]
                )
                t3s.append(t[:].rearrange("h (w c) -> h w c", c=_C2))

            u_halves = [
                u_pool.tile([_H, 128 * _C2], f32, tag="uL", name="uL"),
                u_pool.tile([_H, 128 * _C2], f32, tag="uR", name="uR"),
            ]
            for q in range(4):
                t3 = t3s[q]
                u3h = u_halves[q // 2][:].rearrange("h (w c) -> h w c", c=_C2)
                u3 = u3h[:, 64 * (q % 2) : 64 * (q % 2) + 64, :]
                up = u3.rearrange("h (p two) c -> h p two c", two=2)
                nc.vector.tensor_copy(
                    up[:, 0:32:2, :, :],
                    t3[:, _sl(0, 2, 16), :]
                    .unsqueeze(2)
                    .broadcast_to([_H, 16, 2, _C2]),
                )
                nct = 15 if q == 3 else 16
                nc.vector.tensor_copy(
                    up[:, 1 : 2 * nct : 2, :, :],
                    t3[:, 1 : 2 * nct + 1, :].rearrange(
                        "h (g two) c -> h g two c", two=2
                    ),
                )
                if q == 3:
                    nc.vector.tensor_copy(
                        u3[:, 62:64, :],
                        t3[:, 31:32, :].broadcast_to([_H, 2, _C2]),
                    )

            # Writes: single SWDGE FIFO ring, family-major, halves inner.
            for fam in range(4):
                rd0, rds, rs0, rss, rcnt = _FAMILIES[fam]
                for h in range(2):
                    cols = slice(128 * h, 128 * (h + 1))
                    nc.gpsimd.dma_start(
                        y[_sl(rd0, rds, rcnt), cols, :],
                        u_halves[h][_sl(rs0, rss, rcnt), :],
                    )
    nc.compile()
    return nc


def _build_nc_v10():
    """Loads first on the SWDGE ring + half-width U + odd-pair broadcast.

    v9 lesson: writes starve concurrent HWDGE loads (packet round-robin
    shares engines, bandwidth goes to whoever has descriptors), so late
    chunks -> late expansions -> 14us ring stall.  Put the loads at the
    head of the one SWDGE FIFO ring; all expansions finish while the
    8.6 MB load drains, so the write stream that follows never stalls.
    Writes are 64 KiB descriptors (half-width U tiles): even rows
    (256 KiB pitch), then odd rows 3..253 merged via stride-0 pair
    broadcast (256 KiB pitch), then 2-descriptor edge rows (1, 255).
    """
    import concourse.bacc as bacc
    import concourse.mybir as mybir
    from concourse.tile import TileContext

    f32 = mybir.dt.float32
    nc = bacc.Bacc()
    x = nc.dram_tensor("x", [_H, _W, _C2], f32, kind="ExternalInput")
    y = nc.dram_tensor("y", [_HO, _WO, _C2], f32, kind="ExternalOutput")

    with TileContext(nc) as tc:
        with (
            tc.tile_pool(name="tin", bufs=1) as tin_pool,
            tc.tile_pool(name="uexp", bufs=1) as u_pool,
        ):
            t3s = []
            for q in range(4):
                w0 = 32 * q
                w1 = min(w0 + 33, _W)
                t = tin_pool.tile([_H, (w1 - w0) * _C2], f32, tag=f"t{q}")
                nc.gpsimd.dma_start(
                    t[:].rearrange("h (w c) -> h w c", c=_C2), x[:, w0:w1, :]
                )
                t3s.append(t[:].rearrange("h (w c) -> h w c", c=_C2))

            u_halves = [
                u_pool.tile([_H, 128 * _C2], f32, tag="uL", name="uL"),
                u_pool.tile([_H, 128 * _C2], f32, tag="uR", name="uR"),
            ]
            for q in range(4):
                t3 = t3s[q]
                u3h = u_halves[q // 2][:].rearrange("h (w c) -> h w c", c=_C2)
                u3 = u3h[:, 64 * (q % 2) : 64 * (q % 2) + 64, :]
                up = u3.rearrange("h (p two) c -> h p two c", two=2)
                nc.vector.tensor_copy(
                    up[:, 0:32:2, :, :],
                    t3[:, _sl(0, 2, 16), :]
                    .unsqueeze(2)
                    .broadcast_to([_H, 16, 2, _C2]),
                )
                nct = 15 if q == 3 else 16
                nc.vector.tensor_copy(
                    up[:, 1 : 2 * nct : 2, :, :],
                    t3[:, 1 : 2 * nct + 1, :].rearrange(
                        "h (g two) c -> h g two c", two=2
                    ),
                )
                if q == 3:
                    nc.vector.tensor_copy(
                        u3[:, 62:64, :],
                        t3[:, 31:32, :].broadcast_to([_H, 2, _C2]),
                    )

            for h in range(2):
                u = u_halves[h]
                cols = slice(128 * h, 128 * (h + 1))
                # Even rows 0..254: 128 descs of 64 KiB, 256 KiB pitch.
                nc.gpsimd.dma_start(y[_sl(0, 2, 128), cols, :], u[:, :])
                # Edge rows (1, 255) <- u[(0, 127)]: 2 descs (mid-stream).
                nc.gpsimd.dma_start(y[_sl(1, 254, 2), cols, :], u[_sl(0, 127, 2), :])
                # Odd rows 3..253: pairs (4k+3, 4k+5) <- u[2k+2] twice,
                # 126 descs of 64 KiB, 256 KiB pitch.
                nc.gpsimd.dma_start(
                    y[_sl(3, 2, 126), cols, :].rearrange(
                        "(k two) w c -> k two w c", two=2
                    ),
                    u[_sl(2, 2, 63), :].unsqueeze(1).broadcast_to([63, 2, 128 * _C2]),
                )
    nc.compile()
    return nc


def _v1x_common(nc, mybir, tc, tin_pool, u_pool, f32, x):
    """Shared front half: chunk loads on the SWDGE ring head + DVE
    expansion into two half-width U tiles.  Returns u_halves."""
    t3s = []
    for q in range(4):
        w0 = 32 * q
        w1 = min(w0 + 33, _W)
        t = tin_pool.tile([_H, (w1 - w0) * _C2], f32, tag=f"t{q}", name=f"t{q}")
        nc.gpsimd.dma_start(
            t[:].rearrange("h (w c) -> h w c", c=_C2), x[:, w0:w1, :]
        )
        t3s.append(t[:].rearrange("h (w c) -> h w c", c=_C2))

    u_halves = [
        u_pool.tile([_H, 128 * _C2], f32, tag="uL", name="uL"),
        u_pool.tile([_H, 128 * _C2], f32, tag="uR", name="uR"),
    ]
    for q in range(4):
        t3 = t3s[q]
        u3h = u_halves[q // 2][:].rearrange("h (w c) -> h w c", c=_C2)
        u3 = u3h[:, 64 * (q % 2) : 64 * (q % 2) + 64, :]
        up = u3.rearrange("h (p two) c -> h p two c", two=2)
        nc.vector.tensor_copy(
            up[:, 0:32:2, :, :],
            t3[:, _sl(0, 2, 16), :].unsqueeze(2).broadcast_to([_H, 16, 2, _C2]),
        )
        nct = 15 if q == 3 else 16
        nc.vector.tensor_copy(
            up[:, 1 : 2 * nct : 2, :, :],
            t3[:, 1 : 2 * nct + 1, :].rearrange("h (g two) c -> h g two c", two=2),
        )
        if q == 3:
            nc.vector.tensor_copy(
                u3[:, 62:64, :],
                t3[:, 31:32, :].broadcast_to([_H, 2, _C2]),
            )
    return u_halves


def _build_nc_v11():
    """Loads-first + concurrent 3-ring family writes.

    After the loads drain (~29us, all expansions done), fam0 goes on the
    SWDGE ring while fam1 rides sync and fam2 rides scalar — three
    interleaved streams whose merged address coverage is near-dense
    ascending, testing whether lockstep interleave beats sequential
    strided sweeps."""
    import concourse.bacc as bacc
    import concourse.mybir as mybir
    from concourse.tile import TileContext

    f32 = mybir.dt.float32
    nc = bacc.Bacc()
    x = nc.dram_tensor("x", [_H, _W, _C2], f32, kind="ExternalInput")
    y = nc.dram_tensor("y", [_HO, _WO, _C2], f32, kind="ExternalOutput")

    with TileContext(nc) as tc:
        with (
            tc.tile_pool(name="tin", bufs=1) as tin_pool,
            tc.tile_pool(name="uexp", bufs=1) as u_pool,
        ):
            u_halves = _v1x_common(nc, mybir, tc, tin_pool, u_pool, f32, x)
            for h in range(2):
                u = u_halves[h]
                cols = slice(128 * h, 128 * (h + 1))
                nc.gpsimd.dma_start(y[_sl(0, 2, 128), cols, :], u[:, :])
                nc.gpsimd.dma_start(
                    y[_sl(1, 254, 2), cols, :], u[_sl(0, 127, 2), :]
                )
            for h in range(2):
                cols = slice(128 * h, 128 * (h + 1))
                rd0, rds, rs0, rss, rcnt = _FAMILIES[1]
                nc.sync.dma_start(
                    y[_sl(rd0, rds, rcnt), cols, :],
                    u_halves[h][_sl(rs0, rss, rcnt), :],
                )
                rd0, rds, rs0, rss, rcnt = _FAMILIES[2]
                nc.scalar.dma_start(
                    y[_sl(rd0, rds, rcnt), cols, :],
                    u_halves[h][_sl(rs0, rss, rcnt), :],
                )
    nc.compile()
    return nc


def _build_nc_v12():
    """Loads-first + all-SWDGE with fam1/fam2 interleaved in 1 MiB
    sub-DMAs (16 partitions each), so the FIFO ring's merged odd-row
    stream walks the address space densely at 256 KiB pitch instead of
    two full 512 KiB-pitch passes."""
    import concourse.bacc as bacc
    import concourse.mybir as mybir
    from concourse.tile import TileContext

    f32 = mybir.dt.float32
    nc = bacc.Bacc()
    x = nc.dram_tensor("x", [_H, _W, _C2], f32, kind="ExternalInput")
    y = nc.dram_tensor("y", [_HO, _WO, _C2], f32, kind="ExternalOutput")

    with TileContext(nc) as tc:
        with (
            tc.tile_pool(name="tin", bufs=1) as tin_pool,
            tc.tile_pool(name="uexp", bufs=1) as u_pool,
        ):
            u_halves = _v1x_common(nc, mybir, tc, tin_pool, u_pool, f32, x)
            for h in range(2):
                u = u_halves[h]
                cols = slice(128 * h, 128 * (h + 1))
                nc.gpsimd.dma_start(y[_sl(0, 2, 128), cols, :], u[:, :])
                nc.gpsimd.dma_start(
                    y[_sl(1, 254, 2), cols, :], u[_sl(0, 127, 2), :]
                )
                # Odd rows: alternate fam1/fam2 blocks of 16 partitions
                # (16 x 64 KiB = 1 MiB per sub-DMA) walking forward.
                for b in range(4):
                    # fam1 rows 1+4t for t in [16b, 16b+16)
                    nc.gpsimd.dma_start(
                        y[_sl(1 + 64 * b, 4, 16), cols, :],
                        u[_sl(32 * b, 2, 16), :],
                    )
                    # fam2 rows 3+4t for t in [16b, 16b+16) (15 in last)
                    ncnt = 15 if b == 3 else 16
                    nc.gpsimd.dma_start(
                        y[_sl(3 + 64 * b, 4, ncnt), cols, :],
                        u[_sl(2 + 32 * b, 2, ncnt), :],
                    )
    nc.compile()
    return nc


def _build_nc_v13():
    """Loads-first + one full-row U tile: every write descriptor is
    128 KiB.  Tests whether doubling descriptor size rescues the
    512 KiB-pitch odd families."""
    import concourse.bacc as bacc
    import concourse.mybir as mybir
    from concourse.tile import TileContext

    f32 = mybir.dt.float32
    nc = bacc.Bacc()
    x = nc.dram_tensor("x", [_H, _W, _C2], f32, kind="ExternalInput")
    y = nc.dram_tensor("y", [_HO, _WO, _C2], f32, kind="ExternalOutput")

    with TileContext(nc) as tc:
        with (
            tc.tile_pool(name="tin", bufs=1) as tin_pool,
            tc.tile_pool(name="uexp", bufs=1) as u_pool,
        ):
            t3s = []
            for q in range(4):
                w0 = 32 * q
                w1 = min(w0 + 33, _W)
                t = tin_pool.tile(
                    [_H, (w1 - w0) * _C2], f32, tag=f"t{q}", name=f"t{q}"
                )
                nc.gpsimd.dma_start(
                    t[:].rearrange("h (w c) -> h w c", c=_C2), x[:, w0:w1, :]
                )
                t3s.append(t[:].rearrange("h (w c) -> h w c", c=_C2))

            u = u_pool.tile([_H, 256 * _C2], f32, tag="u", name="u")
            u3f = u[:].rearrange("h (w c) -> h w c", c=_C2)
            for q in range(4):
                t3 = t3s[q]
                u3 = u3f[:, 64 * q : 64 * q + 64, :]
                up = u3.rearrange("h (p two) c -> h p two c", two=2)
                nc.vector.tensor_copy(
                    up[:, 0:32:2, :, :],
                    t3[:, _sl(0, 2, 16), :]
                    .unsqueeze(2)
                    .broadcast_to([_H, 16, 2, _C2]),
                )
                nct = 15 if q == 3 else 16
                nc.vector.tensor_copy(
                    up[:, 1 : 2 * nct : 2, :, :],
                    t3[:, 1 : 2 * nct + 1, :].rearrange(
                        "h (g two) c -> h g two c", two=2
                    ),
                )
                if q == 3:
                    nc.vector.tensor_copy(
                        u3[:, 62:64, :],
                        t3[:, 31:32, :].broadcast_to([_H, 2, _C2]),
                    )

            for fam in range(4):
                rd0, rds, rs0, rss, rcnt = _FAMILIES[fam]
                nc.gpsimd.dma_start(
                    y[_sl(rd0, rds, rcnt), :, :],
                    u[_sl(rs0, rss, rcnt), :],
                )
    nc.compile()
    return nc


VERSION = 11
_BUILDERS = {
    1: _build_nc_v1,
    2: _build_nc_v2,
    3: _build_nc_v3,
    4: _build_nc_v4,
    5: _build_nc_v5,
    6: _build_nc_v6,
    7: _build_nc_v7,
    9: _build_nc_v9,
    10: _build_nc_v10,
    11: _build_nc_v11,
    12: _build_nc_v12,
    13: _build_nc_v13,
}


def _selftest_families():
    """Host-side check: the family decomposition reproduces the reference
    round-half-to-even nearest index map exactly."""
    idx = np.round(128 * np.arange(256, dtype=np.float64) / 256.0)
    # np.round is round-half-to-even like jnp.round
    idx = np.clip(idx.astype(np.int64), 0, 127)
    recon = np.full(256, -1)
    for d0, ds, s0, ss, c in _FAMILIES:
        for i in range(c):
            assert recon[d0 + ds * i] == -1
            recon[d0 + ds * i] = s0 + ss * i
    assert (recon == idx).all()


_selftest_families()


def _build_nc():
    return _BUILDERS[VERSION]()


def _get_nc():
    if VERSION not in _NC_CACHE:
        _NC_CACHE[VERSION] = _build_nc()
    return _NC_CACHE[VERSION]


def kernel(x_real: np.ndarray, x_imag: np.ndarray) -> np.ndarray:
    global LAST_RESULT
    _ensure_axon_ntff_hook()
    from concourse.bass_utils import run_bass_kernel_spmd

    assert x_real.shape == (_B, _H, _W, _C) and x_imag.shape == (_B, _H, _W, _C)

    # Interleave real/imag channel-wise: f32 [B, H, W, 2C]; pairs
    # (re, im) match the complex64 memory layout.
    xc = np.empty((_B, _H, _W, _C, 2), np.float32)
    xc[..., 0] = x_real
    xc[..., 1] = x_imag
    xc = xc.reshape(_B, _H, _W, _C2)

    nc = _get_nc()
    in_maps = [{"x": xc[b]} for b in range(_B)]
    res = run_bass_kernel_spmd(
        nc,
        in_maps,
        core_ids=list(range(_N_CORES)),
        trace=TRACE,
    )
    LAST_RESULT = res

    out = np.stack([res.results[b]["y"] for b in range(_B)])
    # [B, 256, 256, 128] f32 -> complex64 view [B, 256, 256, 64]
    return out.view(np.complex64)



# revision 11
# speedup vs baseline: 1.1544x; 1.0488x over previous
"""Complex nearest-neighbor 2x spatial upsample on 8 TRN2 NeuronCores.

Reference op: x = x_real + 1j*x_imag, shape [8, 128, 128, 64] (B,H,W,C);
out[b, j, k, c] = x[b, r(j), r(k), c] with
r(j) = clip(round_half_to_even(j/2), 0, 127), output [8, 256, 256, 64]
complex64.

Strategy (batch-sharded, 1 sample per core):
  - Host: interleave real/imag into f32 [H, W, 2C] so a complex "pixel"
    is one contiguous 512B chunk and the complex64 output is a pure view.
  - Device: stage the 8 MiB sample in SBUF (128 rows -> 128 partitions),
    then scatter to the 32 MiB output with strided DMAs.  The
    round-half-to-even gather decomposes exactly into 4 affine families
    per axis, so 4x4 = 16 DRAM-write DMAs with 3-dim access patterns
    (rows, cols, 512B contiguous pixel) cover the whole output.
"""

import numpy as np

_B, _H, _W, _C = 8, 128, 128, 64
_C2 = 2 * _C
_HO, _WO = 2 * _H, 2 * _W
_N_CORES = 8

# Affine families of j -> r(j) = clip(round_half_even(j/2), 0, 127), j in [0,256):
#   j = 2m   -> m      (m = 0..127)
#   j = 4t+1 -> 2t     (t = 0..63)
#   j = 4t+3 -> 2t+2   (t = 0..62)
#   j = 255  -> 127
# Tuples: (dst_start, dst_step, src_start, src_step, count)
_FAMILIES = [
    (0, 2, 0, 1, 128),
    (1, 4, 0, 2, 64),
    (3, 4, 2, 2, 63),
    (255, 1, 127, 1, 1),
]

# Set by test harnesses: TRACE=True makes kernel() profile the run and
# stash the BassKernelResults (incl. exec_time_ns) in LAST_RESULT.
TRACE = False
LAST_RESULT = None

_NC_CACHE = {}


def _ensure_axon_ntff_hook():
    """Provide antenv.axon_hooks when the image ships only the antenv stub.

    concourse.bass_utils imports it for trace=True under axon; the slim
    agent image's boot fails to register the hook because the stub antenv
    package has no axon_hooks submodule.  Recreate the ctypes-based NTFF
    hook against libaxon_pjrt.so (same recipe as trn_agent_boot.trn_boot).
    """
    try:
        import antenv.axon_hooks  # noqa: F401

        return
    except ImportError:
        pass

    import contextlib
    import ctypes
    import sys
    import types

    mod = types.ModuleType("antenv.axon_hooks")
    holder = {"hook": None}

    def set_axon_ntff_profile_hook(hook):
        holder["hook"] = hook

    def get_axon_ntff_profile_hook():
        return holder["hook"]

    mod.set_axon_ntff_profile_hook = set_axon_ntff_profile_hook
    mod.get_axon_ntff_profile_hook = get_axon_ntff_profile_hook
    sys.modules["antenv.axon_hooks"] = mod
    try:
        import antenv

        antenv.axon_hooks = mod
    except ImportError:
        pass

    so_path = "/opt/axon/libaxon_pjrt.so"
    try:
        lib = ctypes.CDLL(so_path)
    except OSError:
        return
    if not hasattr(lib, "axon_start_nrt_profile"):
        return
    lib.axon_start_nrt_profile.argtypes = [
        ctypes.POINTER(ctypes.c_int64),
        ctypes.c_size_t,
    ]
    lib.axon_start_nrt_profile.restype = ctypes.c_int64
    lib.axon_stop_nrt_profile.argtypes = [ctypes.c_char_p]
    lib.axon_stop_nrt_profile.restype = ctypes.c_int64

    @contextlib.contextmanager
    def _hook(output_dir, device_ids):
        import jax

        jax.devices()
        if device_ids:
            ids = (ctypes.c_int64 * len(device_ids))(*device_ids)
            rc = lib.axon_start_nrt_profile(ids, len(device_ids))
        else:
            rc = lib.axon_start_nrt_profile(None, 0)
        if rc != 0:
            raise RuntimeError(f"axon_start_nrt_profile rc={rc}")
        try:
            yield
        finally:
            n = lib.axon_stop_nrt_profile(str(output_dir).encode())
            if n < 0:
                raise RuntimeError(f"axon_stop_nrt_profile rc={n}")

    set_axon_ntff_profile_hook(_hook)


def _sl(start, step, count):
    return slice(start, start + (count - 1) * step + 1, step)


def _build_nc_v1():
    """Pure-DMA scatter: 16 strided DMAs with 512B descriptors.

    Measured 165 us/core: descriptor-rate limited (all 16 SDMA engines
    ~100% busy at ~30 ns per 512B descriptor)."""
    import concourse.bacc as bacc
    import concourse.mybir as mybir
    from concourse.tile import TileContext

    nc = bacc.Bacc()
    x = nc.dram_tensor("x", [_H, _W, _C2], mybir.dt.float32, kind="ExternalInput")
    y = nc.dram_tensor("y", [_HO, _WO, _C2], mybir.dt.float32, kind="ExternalOutput")

    with TileContext(nc) as tc:
        with tc.tile_pool(name="stage", bufs=1) as pool:
            t = pool.tile([_H, _W * _C2], mybir.dt.float32)
            t3 = t[:].rearrange("h (w c) -> h w c", c=_C2)
            # 8 MiB load: one contiguous 64 KiB row per partition.
            nc.sync.dma_start(t[:], x[:].rearrange("h w c -> h (w c)"))
            # 16 strided scatter DMAs, alternating between the two HWDGE
            # rings (sync + scalar) so they drain in parallel.
            engines = [nc.sync, nc.scalar]
            i = 0
            for rd0, rds, rs0, rss, rc in _FAMILIES:
                for cd0, cds, cs0, css, cc in _FAMILIES:
                    eng = engines[i % len(engines)]
                    i += 1
                    eng.dma_start(
                        y[_sl(rd0, rds, rc), _sl(cd0, cds, cc), :],
                        t3[_sl(rs0, rss, rc), _sl(cs0, css, cc), :],
                    )
    nc.compile()
    return nc


def _build_nc_v2():
    """On-chip column expansion + contiguous-row scatter.

    Input rows live one-per-partition.  The vector engine expands the
    column (W) axis into U tiles (64 output cols per quarter, 32 KiB per
    partition), then each quarter is written out with 4 row-family DMAs
    whose descriptors are 32 KiB contiguous — DMA runs at line rate
    instead of the 512B descriptor floor of v1.
    """
    import concourse.bacc as bacc
    import concourse.mybir as mybir
    from concourse.tile import TileContext

    f32 = mybir.dt.float32
    nc = bacc.Bacc()
    x = nc.dram_tensor("x", [_H, _W, _C2], f32, kind="ExternalInput")
    y = nc.dram_tensor("y", [_HO, _WO, _C2], f32, kind="ExternalOutput")

    with TileContext(nc) as tc:
        with (
            tc.tile_pool(name="tin", bufs=1) as tin_pool,
            tc.tile_pool(name="uexp", bufs=3) as u_pool,
        ):
            # Input halves: t_lo = cols 0..64 (65 cols, needed by output
            # quarters 0-1), t_hi = cols 64..127 (needed by quarters 2-3).
            t_lo = tin_pool.tile([_H, 65 * _C2], f32, tag="tlo")
            t_hi = tin_pool.tile([_H, 64 * _C2], f32, tag="thi")
            nc.gpsimd.dma_start(
                t_lo[:].rearrange("h (w c) -> h w c", c=_C2), x[:, 0:65, :]
            )
            nc.gpsimd.dma_start(
                t_hi[:].rearrange("h (w c) -> h w c", c=_C2), x[:, 64:128, :]
            )

            out_engines = [nc.sync, nc.scalar]
            n_out = 0
            for q in range(4):
                t = t_lo if q < 2 else t_hi
                base = 32 * q if q < 2 else 32 * (q - 2)
                t3 = t[:].rearrange("h (w c) -> h w c", c=_C2)
                u = u_pool.tile([_H, 64 * _C2], f32, tag="u")
                u3 = u[:].rearrange("h (w c) -> h w c", c=_C2)
                # Quarter cols j=4t+{0,1,2,3} (t=0..15) read input cols
                # base + {2t, 2t, 2t+1, 2t+2} (locals within t_lo/t_hi).
                # View the 64 quarter cols as 32 pairs: even pairs p=2t are
                # cols (4t, 4t+1), odd pairs cols (4t+2, 4t+3).
                up = u3.rearrange("h (p two) c -> h p two c", two=2)
                # A/B fused: dst pairs (4t, 4t+1) <- src col base+2t twice
                # (stride-0 broadcast of the pair dim).
                nc.vector.tensor_copy(
                    up[:, 0:32:2, :, :],
                    t3[:, _sl(base, 2, 16), :]
                    .unsqueeze(2)
                    .broadcast_to([_H, 16, 2, _C2]),
                )
                # C: dst pairs (4t+2, 4t+3) <- src cols (base+2t+1,
                # base+2t+2) contiguous... except the clipped tail in q3.
                nct = 15 if q == 3 else 16
                nc.vector.tensor_copy(
                    up[:, 1 : 2 * nct : 2, :, :],
                    t3[:, base + 1 : base + 2 * nct + 1, :].rearrange(
                        "h (g two) c -> h g two c", two=2
                    ),
                )
                if q == 3:
                    # cols 254, 255 <- input col 127 (local 63) twice.
                    nc.vector.tensor_copy(
                        u3[:, 62:64, :],
                        t3[:, 63:64, :].broadcast_to([_H, 2, _C2]),
                    )
                # Scatter: 4 row families, 32 KiB contiguous descriptors.
                for rd0, rds, rs0, rss, rcnt in _FAMILIES:
                    eng = out_engines[n_out % len(out_engines)]
                    n_out += 1
                    eng.dma_start(
                        y[_sl(rd0, rds, rcnt), 64 * q : 64 * (q + 1), :],
                        u[_sl(rs0, rss, rcnt), :],
                    )
    nc.compile()
    return nc


def _build_nc_v3():
    """v2 + uniform DMA-engine load.

    v2's HWDGE sync ring fed SDMA engines 0-8 ~2x the descriptors of
    9-15, serializing a long tail.  The SWDGE (gpsimd) queue spreads
    descriptors across all 16 engines evenly (observed), so route every
    DMA through it.  Input is loaded as 4 per-quarter column chunks
    (contiguous per row) so each quarter's expansion only waits for its
    own ~2 MiB load.
    """
    import concourse.bacc as bacc
    import concourse.mybir as mybir
    from concourse.tile import TileContext

    f32 = mybir.dt.float32
    nc = bacc.Bacc()
    x = nc.dram_tensor("x", [_H, _W, _C2], f32, kind="ExternalInput")
    y = nc.dram_tensor("y", [_HO, _WO, _C2], f32, kind="ExternalOutput")

    with TileContext(nc) as tc:
        with (
            tc.tile_pool(name="tin", bufs=1) as tin_pool,
            tc.tile_pool(name="uexp", bufs=3) as u_pool,
        ):
            # Quarter q of the output (cols 64q..64q+64) reads input cols
            # 32q..32q+32 inclusive -> 33-col chunks (32 for q3).
            t_chunks = []
            for q in range(4):
                w0 = 32 * q
                w1 = min(w0 + 33, _W)
                t = tin_pool.tile([_H, (w1 - w0) * _C2], f32, tag=f"t{q}")
                nc.gpsimd.dma_start(
                    t[:].rearrange("h (w c) -> h w c", c=_C2), x[:, w0:w1, :]
                )
                t_chunks.append(t)

            for q in range(4):
                t3 = t_chunks[q][:].rearrange("h (w c) -> h w c", c=_C2)
                u = u_pool.tile([_H, 64 * _C2], f32, tag="u")
                u3 = u[:].rearrange("h (w c) -> h w c", c=_C2)
                up = u3.rearrange("h (p two) c -> h p two c", two=2)
                # A/B fused: dst pairs (4t, 4t+1) <- src local col 2t twice.
                nc.vector.tensor_copy(
                    up[:, 0:32:2, :, :],
                    t3[:, _sl(0, 2, 16), :]
                    .unsqueeze(2)
                    .broadcast_to([_H, 16, 2, _C2]),
                )
                # C: dst pairs (4t+2, 4t+3) <- src local cols (2t+1, 2t+2).
                nct = 15 if q == 3 else 16
                nc.vector.tensor_copy(
                    up[:, 1 : 2 * nct : 2, :, :],
                    t3[:, 1 : 2 * nct + 1, :].rearrange(
                        "h (g two) c -> h g two c", two=2
                    ),
                )
                if q == 3:
                    # cols 254, 255 <- input col 127 (local 31) twice.
                    nc.vector.tensor_copy(
                        u3[:, 62:64, :],
                        t3[:, 31:32, :].broadcast_to([_H, 2, _C2]),
                    )
                for rd0, rds, rs0, rss, rcnt in _FAMILIES:
                    nc.gpsimd.dma_start(
                        y[_sl(rd0, rds, rcnt), 64 * q : 64 * (q + 1), :],
                        u[_sl(rs0, rss, rcnt), :],
                    )
    nc.compile()
    return nc


def _build_nc_v4():
    """v3 + DRAM-friendly write sequencing.

    Measured: concurrent 4-family scatter runs at 232 GB/s vs 337 GB/s
    for <=2 interleaved streams (stride-2 row writes are free).  So:
    pass 1 streams the even output rows (one address stream, quarter by
    quarter as expansions finish), pass 2 writes the odd-row families
    with at most ~2 streams in flight, enforced with explicit dep edges.
    All 4 U quarters stay resident (no pool recycling stalls).
    """
    import concourse.bacc as bacc
    import concourse.mybir as mybir
    from concourse.bass import _add_dep_helper
    from concourse.tile import TileContext

    f32 = mybir.dt.float32
    nc = bacc.Bacc()
    x = nc.dram_tensor("x", [_H, _W, _C2], f32, kind="ExternalInput")
    y = nc.dram_tensor("y", [_HO, _WO, _C2], f32, kind="ExternalOutput")

    with TileContext(nc) as tc:
        with (
            tc.tile_pool(name="tin", bufs=1) as tin_pool,
            tc.tile_pool(name="uexp", bufs=1) as u_pool,
        ):
            t3s, u_tiles = [], []
            for q in range(4):
                w0 = 32 * q
                w1 = min(w0 + 33, _W)
                t = tin_pool.tile([_H, (w1 - w0) * _C2], f32, tag=f"t{q}")
                # 128-partition loads stay on SWDGE: HWDGE splits
                # 128-partition DMAs 2:1 across engines 0-8 vs 9-15.
                nc.gpsimd.dma_start(
                    t[:].rearrange("h (w c) -> h w c", c=_C2), x[:, w0:w1, :]
                )
                t3s.append(t[:].rearrange("h (w c) -> h w c", c=_C2))

            # Expansion (DVE) into 4 resident U quarters.
            for q in range(4):
                t3 = t3s[q]
                u = u_pool.tile([_H, 64 * _C2], f32, tag=f"u{q}")
                u_tiles.append(u)
                u3 = u[:].rearrange("h (w c) -> h w c", c=_C2)
                up = u3.rearrange("h (p two) c -> h p two c", two=2)
                nc.vector.tensor_copy(
                    up[:, 0:32:2, :, :],
                    t3[:, _sl(0, 2, 16), :]
                    .unsqueeze(2)
                    .broadcast_to([_H, 16, 2, _C2]),
                )
                nct = 15 if q == 3 else 16
                nc.vector.tensor_copy(
                    up[:, 1 : 2 * nct : 2, :, :],
                    t3[:, 1 : 2 * nct + 1, :].rearrange(
                        "h (g two) c -> h g two c", two=2
                    ),
                )
                if q == 3:
                    nc.vector.tensor_copy(
                        u3[:, 62:64, :],
                        t3[:, 31:32, :].broadcast_to([_H, 2, _C2]),
                    )

            # Pass 1: even output rows.  No deps — expansion completion
            # staggers the quarters naturally (~2 streams in flight max).
            re_insts = []
            for q in range(4):
                rd0, rds, rs0, rss, rcnt = _FAMILIES[0]
                d = nc.gpsimd.dma_start(
                    y[_sl(rd0, rds, rcnt), 64 * q : 64 * (q + 1), :],
                    u_tiles[q][_sl(rs0, rss, rcnt), :],
                )
                re_insts.append(d.ins)
            # Pass 2 on the two HWDGE rings: RO1 family streams on sync,
            # RO2 on scalar — each ring is FIFO, so each family is one
            # continuous ascending address stream (2-stream mix total).
            # One boundary per ring: its first DMA waits for pass 1.
            for fam, eng in ((1, nc.sync), (2, nc.scalar)):
                rd0, rds, rs0, rss, rcnt = _FAMILIES[fam]
                for q in range(4):
                    d = eng.dma_start(
                        y[_sl(rd0, rds, rcnt), 64 * q : 64 * (q + 1), :],
                        u_tiles[q][_sl(rs0, rss, rcnt), :],
                    )
                    if q == 0:
                        for p in re_insts:
                            _add_dep_helper(d.ins, p, True, "pass1->pass2 boundary")
            # row 255 (tiny), after everything on the sync ring
            for q in range(4):
                rd0, rds, rs0, rss, rcnt = _FAMILIES[3]
                nc.sync.dma_start(
                    y[_sl(rd0, rds, rcnt), 64 * q : 64 * (q + 1), :],
                    u_tiles[q][_sl(rs0, rss, rcnt), :],
                )
    nc.compile()
    return nc


def _build_nc_v5(load_engine_name="gpsimd"):
    """Single SWDGE ring, strict FIFO order, no barriers.

    Trace evidence (v4 @166us): HWDGE rings split descriptors ~2:1 (up
    to 3:1) across SDMA engines 0-8 vs 9-15, so the pass-2 odd-row
    families ran at ~210 GB/s on 9 busy engines while 7 idled; loads
    serialized ahead of fam0 on the SWDGE ring and pass2 sat behind an
    all-pass1 barrier (first write byte ~37us).  SWDGE distributes
    descriptors evenly across all 16 engines, and a single FIFO ring
    is exactly one DRAM address stream at all times: load chunks, then
    even rows quarter-by-quarter (expansions complete while the loads
    drain), then the odd-row families back-to-back.  Floor: 42 MiB at
    ~358 GB/s HBM-per-NC = 118us + startup.
    """
    import concourse.bacc as bacc
    import concourse.mybir as mybir
    from concourse.tile import TileContext

    f32 = mybir.dt.float32
    nc = bacc.Bacc()
    x = nc.dram_tensor("x", [_H, _W, _C2], f32, kind="ExternalInput")
    y = nc.dram_tensor("y", [_HO, _WO, _C2], f32, kind="ExternalOutput")

    with TileContext(nc) as tc:
        with (
            tc.tile_pool(name="tin", bufs=1) as tin_pool,
            tc.tile_pool(name="uexp", bufs=1) as u_pool,
        ):
            load_eng = getattr(nc, load_engine_name)
            t3s, u_tiles = [], []
            for q in range(4):
                w0 = 32 * q
                w1 = min(w0 + 33, _W)
                t = tin_pool.tile([_H, (w1 - w0) * _C2], f32, tag=f"t{q}")
                load_eng.dma_start(
                    t[:].rearrange("h (w c) -> h w c", c=_C2), x[:, w0:w1, :]
                )
                t3s.append(t[:].rearrange("h (w c) -> h w c", c=_C2))

            for q in range(4):
                t3 = t3s[q]
                u = u_pool.tile([_H, 64 * _C2], f32, tag=f"u{q}")
                u_tiles.append(u)
                u3 = u[:].rearrange("h (w c) -> h w c", c=_C2)
                up = u3.rearrange("h (p two) c -> h p two c", two=2)
                nc.vector.tensor_copy(
                    up[:, 0:32:2, :, :],
                    t3[:, _sl(0, 2, 16), :]
                    .unsqueeze(2)
                    .broadcast_to([_H, 16, 2, _C2]),
                )
                nct = 15 if q == 3 else 16
                nc.vector.tensor_copy(
                    up[:, 1 : 2 * nct : 2, :, :],
                    t3[:, 1 : 2 * nct + 1, :].rearrange(
                        "h (g two) c -> h g two c", two=2
                    ),
                )
                if q == 3:
                    nc.vector.tensor_copy(
                        u3[:, 62:64, :],
                        t3[:, 31:32, :].broadcast_to([_H, 2, _C2]),
                    )

            # All writes on the single SWDGE FIFO ring, family-major.
            for fam in range(4):
                rd0, rds, rs0, rss, rcnt = _FAMILIES[fam]
                for q in range(4):
                    nc.gpsimd.dma_start(
                        y[_sl(rd0, rds, rcnt), 64 * q : 64 * (q + 1), :],
                        u_tiles[q][_sl(rs0, rss, rcnt), :],
                    )
    nc.compile()
    return nc


def _build_nc_v6():
    """v5 but loads on the sync HWDGE ring, overlapping the SWDGE write
    stream (writes start ~13us instead of ~29us; costs read/write
    stream mixing during the overlap window)."""
    return _build_nc_v5(load_engine_name="sync")


def _build_nc_v7():
    """v6 + merged odd-row writes at 256 KiB pitch.

    Trace evidence (v6 @139us): fam0 (even rows, descriptor pitch
    256 KiB) sustains ~27.5 GB/s/engine (~440 GB/s aggregate), but the
    separate fam1/fam2 passes (pitch 512 KiB) drop to ~13-18 GB/s per
    engine, and the row-255 writes dribble 2 KiB descriptors for the
    last ~15us.  Fix: pair output rows (4k+3, 4k+5), which share source
    row 2k+2, via a stride-0 free-dim broadcast on the SBUF side — one
    DMA per quarter covers odd rows 3..253 with 32 KiB descriptors
    ascending at 256 KiB pitch, exactly like fam0.  Rows 1 and 255 are
    a single 2-descriptor edge DMA per quarter.
    """
    import concourse.bacc as bacc
    import concourse.mybir as mybir
    from concourse.tile import TileContext

    f32 = mybir.dt.float32
    nc = bacc.Bacc()
    x = nc.dram_tensor("x", [_H, _W, _C2], f32, kind="ExternalInput")
    y = nc.dram_tensor("y", [_HO, _WO, _C2], f32, kind="ExternalOutput")

    with TileContext(nc) as tc:
        with (
            tc.tile_pool(name="tin", bufs=1) as tin_pool,
            tc.tile_pool(name="uexp", bufs=1) as u_pool,
        ):
            t3s, u_tiles = [], []
            for q in range(4):
                w0 = 32 * q
                w1 = min(w0 + 33, _W)
                t = tin_pool.tile([_H, (w1 - w0) * _C2], f32, tag=f"t{q}")
                nc.sync.dma_start(
                    t[:].rearrange("h (w c) -> h w c", c=_C2), x[:, w0:w1, :]
                )
                t3s.append(t[:].rearrange("h (w c) -> h w c", c=_C2))

            for q in range(4):
                t3 = t3s[q]
                u = u_pool.tile([_H, 64 * _C2], f32, tag=f"u{q}")
                u_tiles.append(u)
                u3 = u[:].rearrange("h (w c) -> h w c", c=_C2)
                up = u3.rearrange("h (p two) c -> h p two c", two=2)
                nc.vector.tensor_copy(
                    up[:, 0:32:2, :, :],
                    t3[:, _sl(0, 2, 16), :]
                    .unsqueeze(2)
                    .broadcast_to([_H, 16, 2, _C2]),
                )
                nct = 15 if q == 3 else 16
                nc.vector.tensor_copy(
                    up[:, 1 : 2 * nct : 2, :, :],
                    t3[:, 1 : 2 * nct + 1, :].rearrange(
                        "h (g two) c -> h g two c", two=2
                    ),
                )
                if q == 3:
                    nc.vector.tensor_copy(
                        u3[:, 62:64, :],
                        t3[:, 31:32, :].broadcast_to([_H, 2, _C2]),
                    )

            # All writes on the single SWDGE FIFO ring, quarter-major so
            # quarter q's stream starts as soon as its expansion lands.
            for q in range(4):
                u = u_tiles[q]
                cols = slice(64 * q, 64 * (q + 1))
                # Even rows 0,2,...,254 <- u[0..127]: 128 descs, 256 KiB pitch.
                nc.gpsimd.dma_start(y[_sl(0, 2, 128), cols, :], u[:, :])
                # Odd rows 3..253: pairs (4k+3, 4k+5) <- u[2k+2] twice.
                nc.gpsimd.dma_start(
                    y[_sl(3, 2, 126), cols, :].rearrange(
                        "(k two) w c -> k two w c", two=2
                    ),
                    u[_sl(2, 2, 63), :].unsqueeze(1).broadcast_to([63, 2, 64 * _C2]),
                )
                # Edge rows (1, 255) <- u[(0, 127)]: 2 descs.
                nc.gpsimd.dma_start(
                    y[_sl(1, 254, 2), cols, :], u[_sl(0, 127, 2), :]
                )
    nc.compile()
    return nc


def _build_nc_v9():
    """Column-HALF U tiles -> 64 KiB write descriptors.

    v7 showed stride-0 broadcast source descriptors drain at ~13 GB/s
    per engine — dead end.  Back to v6's two-pass row families, but the
    expanded image is staged as two half-width tiles (u_L = output cols
    0..127, u_R = 128..255; 64 KiB per partition each), so every write
    descriptor is 64 KiB (4x v6) and the whole output takes ~510
    descriptors instead of ~1030.  fam0_L starts once quarters 0-1 are
    expanded (~21us).  All writes on the single SWDGE FIFO ring.
    """
    import concourse.bacc as bacc
    import concourse.mybir as mybir
    from concourse.tile import TileContext

    f32 = mybir.dt.float32
    nc = bacc.Bacc()
    x = nc.dram_tensor("x", [_H, _W, _C2], f32, kind="ExternalInput")
    y = nc.dram_tensor("y", [_HO, _WO, _C2], f32, kind="ExternalOutput")

    with TileContext(nc) as tc:
        with (
            tc.tile_pool(name="tin", bufs=1) as tin_pool,
            tc.tile_pool(name="uexp", bufs=1) as u_pool,
        ):
            t3s = []
            for q in range(4):
                w0 = 32 * q
                w1 = min(w0 + 33, _W)
                t = tin_pool.tile([_H, (w1 - w0) * _C2], f32, tag=f"t{q}")
                nc.sync.dma_start(
                    t[:].rearrange("h (w c) -> h w c", c=_C2), x[:, w0:w1, :]
                )
                t3s.append(t[:].rearrange("h (w c) -> h w c", c=_C2))

            u_halves = [
                u_pool.tile([_H, 128 * _C2], f32, tag="uL", name="uL"),
                u_pool.tile([_H, 128 * _C2], f32, tag="uR", name="uR"),
            ]
            for q in range(4):
                t3 = t3s[q]
                u3h = u_halves[q // 2][:].rearrange("h (w c) -> h w c", c=_C2)
                u3 = u3h[:, 64 * (q % 2) : 64 * (q % 2) + 64, :]
                up = u3.rearrange("h (p two) c -> h p two c", two=2)
                nc.vector.tensor_copy(
                    up[:, 0:32:2, :, :],
                    t3[:, _sl(0, 2, 16), :]
                    .unsqueeze(2)
                    .broadcast_to([_H, 16, 2, _C2]),
                )
                nct = 15 if q == 3 else 16
                nc.vector.tensor_copy(
                    up[:, 1 : 2 * nct : 2, :, :],
                    t3[:, 1 : 2 * nct + 1, :].rearrange(
                        "h (g two) c -> h g two c", two=2
                    ),
                )
                if q == 3:
                    nc.vector.tensor_copy(
                        u3[:, 62:64, :],
                        t3[:, 31:32, :].broadcast_to([_H, 2, _C2]),
                    )

            # Writes: single SWDGE FIFO ring, family-major, halves inner.
            for fam in range(4):
                rd0, rds, rs0, rss, rcnt = _FAMILIES[fam]
                for h in range(2):
                    cols = slice(128 * h, 128 * (h + 1))
                    nc.gpsimd.dma_start(
                        y[_sl(rd0, rds, rcnt), cols, :],
                        u_halves[h][_sl(rs0, rss, rcnt), :],
                    )
    nc.compile()
    return nc


def _build_nc_v10():
    """Loads first on the SWDGE ring + half-width U + odd-pair broadcast.

    v9 lesson: writes starve concurrent HWDGE loads (packet round-robin
    shares engines, bandwidth goes to whoever has descriptors), so late
    chunks -> late expansions -> 14us ring stall.  Put the loads at the
    head of the one SWDGE FIFO ring; all expansions finish while the
    8.6 MB load drains, so the write stream that follows never stalls.
    Writes are 64 KiB descriptors (half-width U tiles): even rows
    (256 KiB pitch), then odd rows 3..253 merged via stride-0 pair
    broadcast (256 KiB pitch), then 2-descriptor edge rows (1, 255).
    """
    import concourse.bacc as bacc
    import concourse.mybir as mybir
    from concourse.tile import TileContext

    f32 = mybir.dt.float32
    nc = bacc.Bacc()
    x = nc.dram_tensor("x", [_H, _W, _C2], f32, kind="ExternalInput")
    y = nc.dram_tensor("y", [_HO, _WO, _C2], f32, kind="ExternalOutput")

    with TileContext(nc) as tc:
        with (
            tc.tile_pool(name="tin", bufs=1) as tin_pool,
            tc.tile_pool(name="uexp", bufs=1) as u_pool,
        ):
            t3s = []
            for q in range(4):
                w0 = 32 * q
                w1 = min(w0 + 33, _W)
                t = tin_pool.tile([_H, (w1 - w0) * _C2], f32, tag=f"t{q}")
                nc.gpsimd.dma_start(
                    t[:].rearrange("h (w c) -> h w c", c=_C2), x[:, w0:w1, :]
                )
                t3s.append(t[:].rearrange("h (w c) -> h w c", c=_C2))

            u_halves = [
                u_pool.tile([_H, 128 * _C2], f32, tag="uL", name="uL"),
                u_pool.tile([_H, 128 * _C2], f32, tag="uR", name="uR"),
            ]
            for q in range(4):
                t3 = t3s[q]
                u3h = u_halves[q // 2][:].rearrange("h (w c) -> h w c", c=_C2)
                u3 = u3h[:, 64 * (q % 2) : 64 * (q % 2) + 64, :]
                up = u3.rearrange("h (p two) c -> h p two c", two=2)
                nc.vector.tensor_copy(
                    up[:, 0:32:2, :, :],
                    t3[:, _sl(0, 2, 16), :]
                    .unsqueeze(2)
                    .broadcast_to([_H, 16, 2, _C2]),
                )
                nct = 15 if q == 3 else 16
                nc.vector.tensor_copy(
                    up[:, 1 : 2 * nct : 2, :, :],
                    t3[:, 1 : 2 * nct + 1, :].rearrange(
                        "h (g two) c -> h g two c", two=2
                    ),
                )
                if q == 3:
                    nc.vector.tensor_copy(
                        u3[:, 62:64, :],
                        t3[:, 31:32, :].broadcast_to([_H, 2, _C2]),
                    )

            for h in range(2):
                u = u_halves[h]
                cols = slice(128 * h, 128 * (h + 1))
                # Even rows 0..254: 128 descs of 64 KiB, 256 KiB pitch.
                nc.gpsimd.dma_start(y[_sl(0, 2, 128), cols, :], u[:, :])
                # Edge rows (1, 255) <- u[(0, 127)]: 2 descs (mid-stream).
                nc.gpsimd.dma_start(y[_sl(1, 254, 2), cols, :], u[_sl(0, 127, 2), :])
                # Odd rows 3..253: pairs (4k+3, 4k+5) <- u[2k+2] twice,
                # 126 descs of 64 KiB, 256 KiB pitch.
                nc.gpsimd.dma_start(
                    y[_sl(3, 2, 126), cols, :].rearrange(
                        "(k two) w c -> k two w c", two=2
                    ),
                    u[_sl(2, 2, 63), :].unsqueeze(1).broadcast_to([63, 2, 128 * _C2]),
                )
    nc.compile()
    return nc


def _v1x_common(nc, mybir, tc, tin_pool, u_pool, f32, x):
    """Shared front half: chunk loads on the SWDGE ring head + DVE
    expansion into two half-width U tiles.  Returns u_halves."""
    t3s = []
    for q in range(4):
        w0 = 32 * q
        w1 = min(w0 + 33, _W)
        t = tin_pool.tile([_H, (w1 - w0) * _C2], f32, tag=f"t{q}", name=f"t{q}")
        nc.gpsimd.dma_start(
            t[:].rearrange("h (w c) -> h w c", c=_C2), x[:, w0:w1, :]
        )
        t3s.append(t[:].rearrange("h (w c) -> h w c", c=_C2))

    u_halves = [
        u_pool.tile([_H, 128 * _C2], f32, tag="uL", name="uL"),
        u_pool.tile([_H, 128 * _C2], f32, tag="uR", name="uR"),
    ]
    for q in range(4):
        t3 = t3s[q]
        u3h = u_halves[q // 2][:].rearrange("h (w c) -> h w c", c=_C2)
        u3 = u3h[:, 64 * (q % 2) : 64 * (q % 2) + 64, :]
        up = u3.rearrange("h (p two) c -> h p two c", two=2)
        nc.vector.tensor_copy(
            up[:, 0:32:2, :, :],
            t3[:, _sl(0, 2, 16), :].unsqueeze(2).broadcast_to([_H, 16, 2, _C2]),
        )
        nct = 15 if q == 3 else 16
        nc.vector.tensor_copy(
            up[:, 1 : 2 * nct : 2, :, :],
            t3[:, 1 : 2 * nct + 1, :].rearrange("h (g two) c -> h g two c", two=2),
        )
        if q == 3:
            nc.vector.tensor_copy(
                u3[:, 62:64, :],
                t3[:, 31:32, :].broadcast_to([_H, 2, _C2]),
            )
    return u_halves


def _build_nc_v11():
    """Loads-first + concurrent 3-ring family writes.

    After the loads drain (~29us, all expansions done), fam0 goes on the
    SWDGE ring while fam1 rides sync and fam2 rides scalar — three
    interleaved streams whose merged address coverage is near-dense
    ascending, testing whether lockstep interleave beats sequential
    strided sweeps."""
    import concourse.bacc as bacc
    import concourse.mybir as mybir
    from concourse.tile import TileContext

    f32 = mybir.dt.float32
    nc = bacc.Bacc()
    x = nc.dram_tensor("x", [_H, _W, _C2], f32, kind="ExternalInput")
    y = nc.dram_tensor("y", [_HO, _WO, _C2], f32, kind="ExternalOutput")

    with TileContext(nc) as tc:
        with (
            tc.tile_pool(name="tin", bufs=1) as tin_pool,
            tc.tile_pool(name="uexp", bufs=1) as u_pool,
        ):
            u_halves = _v1x_common(nc, mybir, tc, tin_pool, u_pool, f32, x)
            for h in range(2):
                u = u_halves[h]
                cols = slice(128 * h, 128 * (h + 1))
                nc.gpsimd.dma_start(y[_sl(0, 2, 128), cols, :], u[:, :])
                nc.gpsimd.dma_start(
                    y[_sl(1, 254, 2), cols, :], u[_sl(0, 127, 2), :]
                )
            for h in range(2):
                cols = slice(128 * h, 128 * (h + 1))
                rd0, rds, rs0, rss, rcnt = _FAMILIES[1]
                nc.sync.dma_start(
                    y[_sl(rd0, rds, rcnt), cols, :],
                    u_halves[h][_sl(rs0, rss, rcnt), :],
                )
                rd0, rds, rs0, rss, rcnt = _FAMILIES[2]
                nc.scalar.dma_start(
                    y[_sl(rd0, rds, rcnt), cols, :],
                    u_halves[h][_sl(rs0, rss, rcnt), :],
                )
    nc.compile()
    return nc


def _build_nc_v12():
    """Loads-first + all-SWDGE with fam1/fam2 interleaved in 1 MiB
    sub-DMAs (16 partitions each), so the FIFO ring's merged odd-row
    stream walks the address space densely at 256 KiB pitch instead of
    two full 512 KiB-pitch passes."""
    import concourse.bacc as bacc
    import concourse.mybir as mybir
    from concourse.tile import TileContext

    f32 = mybir.dt.float32
    nc = bacc.Bacc()
    x = nc.dram_tensor("x", [_H, _W, _C2], f32, kind="ExternalInput")
    y = nc.dram_tensor("y", [_HO, _WO, _C2], f32, kind="ExternalOutput")

    with TileContext(nc) as tc:
        with (
            tc.tile_pool(name="tin", bufs=1) as tin_pool,
            tc.tile_pool(name="uexp", bufs=1) as u_pool,
        ):
            u_halves = _v1x_common(nc, mybir, tc, tin_pool, u_pool, f32, x)
            for h in range(2):
                u = u_halves[h]
                cols = slice(128 * h, 128 * (h + 1))
                nc.gpsimd.dma_start(y[_sl(0, 2, 128), cols, :], u[:, :])
                nc.gpsimd.dma_start(
                    y[_sl(1, 254, 2), cols, :], u[_sl(0, 127, 2), :]
                )
                # Odd rows: alternate fam1/fam2 blocks of 16 partitions
                # (16 x 64 KiB = 1 MiB per sub-DMA) walking forward.
                for b in range(4):
                    # fam1 rows 1+4t for t in [16b, 16b+16)
                    nc.gpsimd.dma_start(
                        y[_sl(1 + 64 * b, 4, 16), cols, :],
                        u[_sl(32 * b, 2, 16), :],
                    )
                    # fam2 rows 3+4t for t in [16b, 16b+16) (15 in last)
                    ncnt = 15 if b == 3 else 16
                    nc.gpsimd.dma_start(
                        y[_sl(3 + 64 * b, 4, ncnt), cols, :],
                        u[_sl(2 + 32 * b, 2, ncnt), :],
                    )
    nc.compile()
    return nc


def _build_nc_v13():
    """Loads-first + one full-row U tile: every write descriptor is
    128 KiB.  Tests whether doubling descriptor size rescues the
    512 KiB-pitch odd families."""
    import concourse.bacc as bacc
    import concourse.mybir as mybir
    from concourse.tile import TileContext

    f32 = mybir.dt.float32
    nc = bacc.Bacc()
    x = nc.dram_tensor("x", [_H, _W, _C2], f32, kind="ExternalInput")
    y = nc.dram_tensor("y", [_HO, _WO, _C2], f32, kind="ExternalOutput")

    with TileContext(nc) as tc:
        with (
            tc.tile_pool(name="tin", bufs=1) as tin_pool,
            tc.tile_pool(name="uexp", bufs=1) as u_pool,
        ):
            t3s = []
            for q in range(4):
                w0 = 32 * q
                w1 = min(w0 + 33, _W)
                t = tin_pool.tile(
                    [_H, (w1 - w0) * _C2], f32, tag=f"t{q}", name=f"t{q}"
                )
                nc.gpsimd.dma_start(
                    t[:].rearrange("h (w c) -> h w c", c=_C2), x[:, w0:w1, :]
                )
                t3s.append(t[:].rearrange("h (w c) -> h w c", c=_C2))

            u = u_pool.tile([_H, 256 * _C2], f32, tag="u", name="u")
            u3f = u[:].rearrange("h (w c) -> h w c", c=_C2)
            for q in range(4):
                t3 = t3s[q]
                u3 = u3f[:, 64 * q : 64 * q + 64, :]
                up = u3.rearrange("h (p two) c -> h p two c", two=2)
                nc.vector.tensor_copy(
                    up[:, 0:32:2, :, :],
                    t3[:, _sl(0, 2, 16), :]
                    .unsqueeze(2)
                    .broadcast_to([_H, 16, 2, _C2]),
                )
                nct = 15 if q == 3 else 16
                nc.vector.tensor_copy(
                    up[:, 1 : 2 * nct : 2, :, :],
                    t3[:, 1 : 2 * nct + 1, :].rearrange(
                        "h (g two) c -> h g two c", two=2
                    ),
                )
                if q == 3:
                    nc.vector.tensor_copy(
                        u3[:, 62:64, :],
                        t3[:, 31:32, :].broadcast_to([_H, 2, _C2]),
                    )

            for fam in range(4):
                rd0, rds, rs0, rss, rcnt = _FAMILIES[fam]
                nc.gpsimd.dma_start(
                    y[_sl(rd0, rds, rcnt), :, :],
                    u[_sl(rs0, rss, rcnt), :],
                )
    nc.compile()
    return nc


VERSION = 12
_BUILDERS = {
    1: _build_nc_v1,
    2: _build_nc_v2,
    3: _build_nc_v3,
    4: _build_nc_v4,
    5: _build_nc_v5,
    6: _build_nc_v6,
    7: _build_nc_v7,
    9: _build_nc_v9,
    10: _build_nc_v10,
    11: _build_nc_v11,
    12: _build_nc_v12,
    13: _build_nc_v13,
}


def _selftest_families():
    """Host-side check: the family decomposition reproduces the reference
    round-half-to-even nearest index map exactly."""
    idx = np.round(128 * np.arange(256, dtype=np.float64) / 256.0)
    # np.round is round-half-to-even like jnp.round
    idx = np.clip(idx.astype(np.int64), 0, 127)
    recon = np.full(256, -1)
    for d0, ds, s0, ss, c in _FAMILIES:
        for i in range(c):
            assert recon[d0 + ds * i] == -1
            recon[d0 + ds * i] = s0 + ss * i
    assert (recon == idx).all()


_selftest_families()


def _build_nc():
    return _BUILDERS[VERSION]()


def _get_nc():
    if VERSION not in _NC_CACHE:
        _NC_CACHE[VERSION] = _build_nc()
    return _NC_CACHE[VERSION]


def kernel(x_real: np.ndarray, x_imag: np.ndarray) -> np.ndarray:
    global LAST_RESULT
    _ensure_axon_ntff_hook()
    from concourse.bass_utils import run_bass_kernel_spmd

    assert x_real.shape == (_B, _H, _W, _C) and x_imag.shape == (_B, _H, _W, _C)

    # Interleave real/imag channel-wise: f32 [B, H, W, 2C]; pairs
    # (re, im) match the complex64 memory layout.
    xc = np.empty((_B, _H, _W, _C, 2), np.float32)
    xc[..., 0] = x_real
    xc[..., 1] = x_imag
    xc = xc.reshape(_B, _H, _W, _C2)

    nc = _get_nc()
    in_maps = [{"x": xc[b]} for b in range(_B)]
    res = run_bass_kernel_spmd(
        nc,
        in_maps,
        core_ids=list(range(_N_CORES)),
        trace=TRACE,
    )
    LAST_RESULT = res

    out = np.stack([res.results[b]["y"] for b in range(_B)])
    # [B, 256, 256, 128] f32 -> complex64 view [B, 256, 256, 64]
    return out.view(np.complex64)



# revision 12
# speedup vs baseline: 1.2854x; 1.1135x over previous
"""Complex nearest-neighbor 2x spatial upsample on 8 TRN2 NeuronCores.

Reference op: x = x_real + 1j*x_imag, shape [8, 128, 128, 64] (B,H,W,C);
out[b, j, k, c] = x[b, r(j), r(k), c] with
r(j) = clip(round_half_to_even(j/2), 0, 127), output [8, 256, 256, 64]
complex64.

Strategy (batch-sharded, 1 sample per core):
  - Host: interleave real/imag into f32 [H, W, 2C] so a complex "pixel"
    is one contiguous 512B chunk and the complex64 output is a pure view.
  - Device: stage the 8 MiB sample in SBUF (128 rows -> 128 partitions),
    then scatter to the 32 MiB output with strided DMAs.  The
    round-half-to-even gather decomposes exactly into 4 affine families
    per axis, so 4x4 = 16 DRAM-write DMAs with 3-dim access patterns
    (rows, cols, 512B contiguous pixel) cover the whole output.
"""

import numpy as np

_B, _H, _W, _C = 8, 128, 128, 64
_C2 = 2 * _C
_HO, _WO = 2 * _H, 2 * _W
_N_CORES = 8

# Affine families of j -> r(j) = clip(round_half_even(j/2), 0, 127), j in [0,256):
#   j = 2m   -> m      (m = 0..127)
#   j = 4t+1 -> 2t     (t = 0..63)
#   j = 4t+3 -> 2t+2   (t = 0..62)
#   j = 255  -> 127
# Tuples: (dst_start, dst_step, src_start, src_step, count)
_FAMILIES = [
    (0, 2, 0, 1, 128),
    (1, 4, 0, 2, 64),
    (3, 4, 2, 2, 63),
    (255, 1, 127, 1, 1),
]

# Set by test harnesses: TRACE=True makes kernel() profile the run and
# stash the BassKernelResults (incl. exec_time_ns) in LAST_RESULT.
TRACE = False
LAST_RESULT = None

_NC_CACHE = {}


def _ensure_axon_ntff_hook():
    """Provide antenv.axon_hooks when the image ships only the antenv stub.

    concourse.bass_utils imports it for trace=True under axon; the slim
    agent image's boot fails to register the hook because the stub antenv
    package has no axon_hooks submodule.  Recreate the ctypes-based NTFF
    hook against libaxon_pjrt.so (same recipe as trn_agent_boot.trn_boot).
    """
    try:
        import antenv.axon_hooks  # noqa: F401

        return
    except ImportError:
        pass

    import contextlib
    import ctypes
    import sys
    import types

    mod = types.ModuleType("antenv.axon_hooks")
    holder = {"hook": None}

    def set_axon_ntff_profile_hook(hook):
        holder["hook"] = hook

    def get_axon_ntff_profile_hook():
        return holder["hook"]

    mod.set_axon_ntff_profile_hook = set_axon_ntff_profile_hook
    mod.get_axon_ntff_profile_hook = get_axon_ntff_profile_hook
    sys.modules["antenv.axon_hooks"] = mod
    try:
        import antenv

        antenv.axon_hooks = mod
    except ImportError:
        pass

    so_path = "/opt/axon/libaxon_pjrt.so"
    try:
        lib = ctypes.CDLL(so_path)
    except OSError:
        return
    if not hasattr(lib, "axon_start_nrt_profile"):
        return
    lib.axon_start_nrt_profile.argtypes = [
        ctypes.POINTER(ctypes.c_int64),
        ctypes.c_size_t,
    ]
    lib.axon_start_nrt_profile.restype = ctypes.c_int64
    lib.axon_stop_nrt_profile.argtypes = [ctypes.c_char_p]
    lib.axon_stop_nrt_profile.restype = ctypes.c_int64

    @contextlib.contextmanager
    def _hook(output_dir, device_ids):
        import jax

        jax.devices()
        if device_ids:
            ids = (ctypes.c_int64 * len(device_ids))(*device_ids)
            rc = lib.axon_start_nrt_profile(ids, len(device_ids))
        else:
            rc = lib.axon_start_nrt_profile(None, 0)
        if rc != 0:
            raise RuntimeError(f"axon_start_nrt_profile rc={rc}")
        try:
            yield
        finally:
            n = lib.axon_stop_nrt_profile(str(output_dir).encode())
            if n < 0:
                raise RuntimeError(f"axon_stop_nrt_profile rc={n}")

    set_axon_ntff_profile_hook(_hook)


def _sl(start, step, count):
    return slice(start, start + (count - 1) * step + 1, step)


def _build_nc_v1():
    """Pure-DMA scatter: 16 strided DMAs with 512B descriptors.

    Measured 165 us/core: descriptor-rate limited (all 16 SDMA engines
    ~100% busy at ~30 ns per 512B descriptor)."""
    import concourse.bacc as bacc
    import concourse.mybir as mybir
    from concourse.tile import TileContext

    nc = bacc.Bacc()
    x = nc.dram_tensor("x", [_H, _W, _C2], mybir.dt.float32, kind="ExternalInput")
    y = nc.dram_tensor("y", [_HO, _WO, _C2], mybir.dt.float32, kind="ExternalOutput")

    with TileContext(nc) as tc:
        with tc.tile_pool(name="stage", bufs=1) as pool:
            t = pool.tile([_H, _W * _C2], mybir.dt.float32)
            t3 = t[:].rearrange("h (w c) -> h w c", c=_C2)
            # 8 MiB load: one contiguous 64 KiB row per partition.
            nc.sync.dma_start(t[:], x[:].rearrange("h w c -> h (w c)"))
            # 16 strided scatter DMAs, alternating between the two HWDGE
            # rings (sync + scalar) so they drain in parallel.
            engines = [nc.sync, nc.scalar]
            i = 0
            for rd0, rds, rs0, rss, rc in _FAMILIES:
                for cd0, cds, cs0, css, cc in _FAMILIES:
                    eng = engines[i % len(engines)]
                    i += 1
                    eng.dma_start(
                        y[_sl(rd0, rds, rc), _sl(cd0, cds, cc), :],
                        t3[_sl(rs0, rss, rc), _sl(cs0, css, cc), :],
                    )
    nc.compile()
    return nc


def _build_nc_v2():
    """On-chip column expansion + contiguous-row scatter.

    Input rows live one-per-partition.  The vector engine expands the
    column (W) axis into U tiles (64 output cols per quarter, 32 KiB per
    partition), then each quarter is written out with 4 row-family DMAs
    whose descriptors are 32 KiB contiguous — DMA runs at line rate
    instead of the 512B descriptor floor of v1.
    """
    import concourse.bacc as bacc
    import concourse.mybir as mybir
    from concourse.tile import TileContext

    f32 = mybir.dt.float32
    nc = bacc.Bacc()
    x = nc.dram_tensor("x", [_H, _W, _C2], f32, kind="ExternalInput")
    y = nc.dram_tensor("y", [_HO, _WO, _C2], f32, kind="ExternalOutput")

    with TileContext(nc) as tc:
        with (
            tc.tile_pool(name="tin", bufs=1) as tin_pool,
            tc.tile_pool(name="uexp", bufs=3) as u_pool,
        ):
            # Input halves: t_lo = cols 0..64 (65 cols, needed by output
            # quarters 0-1), t_hi = cols 64..127 (needed by quarters 2-3).
            t_lo = tin_pool.tile([_H, 65 * _C2], f32, tag="tlo")
            t_hi = tin_pool.tile([_H, 64 * _C2], f32, tag="thi")
            nc.gpsimd.dma_start(
                t_lo[:].rearrange("h (w c) -> h w c", c=_C2), x[:, 0:65, :]
            )
            nc.gpsimd.dma_start(
                t_hi[:].rearrange("h (w c) -> h w c", c=_C2), x[:, 64:128, :]
            )

            out_engines = [nc.sync, nc.scalar]
            n_out = 0
            for q in range(4):
                t = t_lo if q < 2 else t_hi
                base = 32 * q if q < 2 else 32 * (q - 2)
                t3 = t[:].rearrange("h (w c) -> h w c", c=_C2)
                u = u_pool.tile([_H, 64 * _C2], f32, tag="u")
                u3 = u[:].rearrange("h (w c) -> h w c", c=_C2)
                # Quarter cols j=4t+{0,1,2,3} (t=0..15) read input cols
                # base + {2t, 2t, 2t+1, 2t+2} (locals within t_lo/t_hi).
                # View the 64 quarter cols as 32 pairs: even pairs p=2t are
                # cols (4t, 4t+1), odd pairs cols (4t+2, 4t+3).
                up = u3.rearrange("h (p two) c -> h p two c", two=2)
                # A/B fused: dst pairs (4t, 4t+1) <- src col base+2t twice
                # (stride-0 broadcast of the pair dim).
                nc.vector.tensor_copy(
                    up[:, 0:32:2, :, :],
                    t3[:, _sl(base, 2, 16), :]
                    .unsqueeze(2)
                    .broadcast_to([_H, 16, 2, _C2]),
                )
                # C: dst pairs (4t+2, 4t+3) <- src cols (base+2t+1,
                # base+2t+2) contiguous... except the clipped tail in q3.
                nct = 15 if q == 3 else 16
                nc.vector.tensor_copy(
                    up[:, 1 : 2 * nct : 2, :, :],
                    t3[:, base + 1 : base + 2 * nct + 1, :].rearrange(
                        "h (g two) c -> h g two c", two=2
                    ),
                )
                if q == 3:
                    # cols 254, 255 <- input col 127 (local 63) twice.
                    nc.vector.tensor_copy(
                        u3[:, 62:64, :],
                        t3[:, 63:64, :].broadcast_to([_H, 2, _C2]),
                    )
                # Scatter: 4 row families, 32 KiB contiguous descriptors.
                for rd0, rds, rs0, rss, rcnt in _FAMILIES:
                    eng = out_engines[n_out % len(out_engines)]
                    n_out += 1
                    eng.dma_start(
                        y[_sl(rd0, rds, rcnt), 64 * q : 64 * (q + 1), :],
                        u[_sl(rs0, rss, rcnt), :],
                    )
    nc.compile()
    return nc


def _build_nc_v3():
    """v2 + uniform DMA-engine load.

    v2's HWDGE sync ring fed SDMA engines 0-8 ~2x the descriptors of
    9-15, serializing a long tail.  The SWDGE (gpsimd) queue spreads
    descriptors across all 16 engines evenly (observed), so route every
    DMA through it.  Input is loaded as 4 per-quarter column chunks
    (contiguous per row) so each quarter's expansion only waits for its
    own ~2 MiB load.
    """
    import concourse.bacc as bacc
    import concourse.mybir as mybir
    from concourse.tile import TileContext

    f32 = mybir.dt.float32
    nc = bacc.Bacc()
    x = nc.dram_tensor("x", [_H, _W, _C2], f32, kind="ExternalInput")
    y = nc.dram_tensor("y", [_HO, _WO, _C2], f32, kind="ExternalOutput")

    with TileContext(nc) as tc:
        with (
            tc.tile_pool(name="tin", bufs=1) as tin_pool,
            tc.tile_pool(name="uexp", bufs=3) as u_pool,
        ):
            # Quarter q of the output (cols 64q..64q+64) reads input cols
            # 32q..32q+32 inclusive -> 33-col chunks (32 for q3).
            t_chunks = []
            for q in range(4):
                w0 = 32 * q
                w1 = min(w0 + 33, _W)
                t = tin_pool.tile([_H, (w1 - w0) * _C2], f32, tag=f"t{q}")
                nc.gpsimd.dma_start(
                    t[:].rearrange("h (w c) -> h w c", c=_C2), x[:, w0:w1, :]
                )
                t_chunks.append(t)

            for q in range(4):
                t3 = t_chunks[q][:].rearrange("h (w c) -> h w c", c=_C2)
                u = u_pool.tile([_H, 64 * _C2], f32, tag="u")
                u3 = u[:].rearrange("h (w c) -> h w c", c=_C2)
                up = u3.rearrange("h (p two) c -> h p two c", two=2)
                # A/B fused: dst pairs (4t, 4t+1) <- src local col 2t twice.
                nc.vector.tensor_copy(
                    up[:, 0:32:2, :, :],
                    t3[:, _sl(0, 2, 16), :]
                    .unsqueeze(2)
                    .broadcast_to([_H, 16, 2, _C2]),
                )
                # C: dst pairs (4t+2, 4t+3) <- src local cols (2t+1, 2t+2).
                nct = 15 if q == 3 else 16
                nc.vector.tensor_copy(
                    up[:, 1 : 2 * nct : 2, :, :],
                    t3[:, 1 : 2 * nct + 1, :].rearrange(
                        "h (g two) c -> h g two c", two=2
                    ),
                )
                if q == 3:
                    # cols 254, 255 <- input col 127 (local 31) twice.
                    nc.vector.tensor_copy(
                        u3[:, 62:64, :],
                        t3[:, 31:32, :].broadcast_to([_H, 2, _C2]),
                    )
                for rd0, rds, rs0, rss, rcnt in _FAMILIES:
                    nc.gpsimd.dma_start(
                        y[_sl(rd0, rds, rcnt), 64 * q : 64 * (q + 1), :],
                        u[_sl(rs0, rss, rcnt), :],
                    )
    nc.compile()
    return nc


def _build_nc_v4():
    """v3 + DRAM-friendly write sequencing.

    Measured: concurrent 4-family scatter runs at 232 GB/s vs 337 GB/s
    for <=2 interleaved streams (stride-2 row writes are free).  So:
    pass 1 streams the even output rows (one address stream, quarter by
    quarter as expansions finish), pass 2 writes the odd-row families
    with at most ~2 streams in flight, enforced with explicit dep edges.
    All 4 U quarters stay resident (no pool recycling stalls).
    """
    import concourse.bacc as bacc
    import concourse.mybir as mybir
    from concourse.bass import _add_dep_helper
    from concourse.tile import TileContext

    f32 = mybir.dt.float32
    nc = bacc.Bacc()
    x = nc.dram_tensor("x", [_H, _W, _C2], f32, kind="ExternalInput")
    y = nc.dram_tensor("y", [_HO, _WO, _C2], f32, kind="ExternalOutput")

    with TileContext(nc) as tc:
        with (
            tc.tile_pool(name="tin", bufs=1) as tin_pool,
            tc.tile_pool(name="uexp", bufs=1) as u_pool,
        ):
            t3s, u_tiles = [], []
            for q in range(4):
                w0 = 32 * q
                w1 = min(w0 + 33, _W)
                t = tin_pool.tile([_H, (w1 - w0) * _C2], f32, tag=f"t{q}")
                # 128-partition loads stay on SWDGE: HWDGE splits
                # 128-partition DMAs 2:1 across engines 0-8 vs 9-15.
                nc.gpsimd.dma_start(
                    t[:].rearrange("h (w c) -> h w c", c=_C2), x[:, w0:w1, :]
                )
                t3s.append(t[:].rearrange("h (w c) -> h w c", c=_C2))

            # Expansion (DVE) into 4 resident U quarters.
            for q in range(4):
                t3 = t3s[q]
                u = u_pool.tile([_H, 64 * _C2], f32, tag=f"u{q}")
                u_tiles.append(u)
                u3 = u[:].rearrange("h (w c) -> h w c", c=_C2)
                up = u3.rearrange("h (p two) c -> h p two c", two=2)
                nc.vector.tensor_copy(
                    up[:, 0:32:2, :, :],
                    t3[:, _sl(0, 2, 16), :]
                    .unsqueeze(2)
                    .broadcast_to([_H, 16, 2, _C2]),
                )
                nct = 15 if q == 3 else 16
                nc.vector.tensor_copy(
                    up[:, 1 : 2 * nct : 2, :, :],
                    t3[:, 1 : 2 * nct + 1, :].rearrange(
                        "h (g two) c -> h g two c", two=2
                    ),
                )
                if q == 3:
                    nc.vector.tensor_copy(
                        u3[:, 62:64, :],
                        t3[:, 31:32, :].broadcast_to([_H, 2, _C2]),
                    )

            # Pass 1: even output rows.  No deps — expansion completion
            # staggers the quarters naturally (~2 streams in flight max).
            re_insts = []
            for q in range(4):
                rd0, rds, rs0, rss, rcnt = _FAMILIES[0]
                d = nc.gpsimd.dma_start(
                    y[_sl(rd0, rds, rcnt), 64 * q : 64 * (q + 1), :],
                    u_tiles[q][_sl(rs0, rss, rcnt), :],
                )
                re_insts.append(d.ins)
            # Pass 2 on the two HWDGE rings: RO1 family streams on sync,
            # RO2 on scalar — each ring is FIFO, so each family is one
            # continuous ascending address stream (2-stream mix total).
            # One boundary per ring: its first DMA waits for pass 1.
            for fam, eng in ((1, nc.sync), (2, nc.scalar)):
                rd0, rds, rs0, rss, rcnt = _FAMILIES[fam]
                for q in range(4):
                    d = eng.dma_start(
                        y[_sl(rd0, rds, rcnt), 64 * q : 64 * (q + 1), :],
                        u_tiles[q][_sl(rs0, rss, rcnt), :],
                    )
                    if q == 0:
                        for p in re_insts:
                            _add_dep_helper(d.ins, p, True, "pass1->pass2 boundary")
            # row 255 (tiny), after everything on the sync ring
            for q in range(4):
                rd0, rds, rs0, rss, rcnt = _FAMILIES[3]
                nc.sync.dma_start(
                    y[_sl(rd0, rds, rcnt), 64 * q : 64 * (q + 1), :],
                    u_tiles[q][_sl(rs0, rss, rcnt), :],
                )
    nc.compile()
    return nc


def _build_nc_v5(load_engine_name="gpsimd"):
    """Single SWDGE ring, strict FIFO order, no barriers.

    Trace evidence (v4 @166us): HWDGE rings split descriptors ~2:1 (up
    to 3:1) across SDMA engines 0-8 vs 9-15, so the pass-2 odd-row
    families ran at ~210 GB/s on 9 busy engines while 7 idled; loads
    serialized ahead of fam0 on the SWDGE ring and pass2 sat behind an
    all-pass1 barrier (first write byte ~37us).  SWDGE distributes
    descriptors evenly across all 16 engines, and a single FIFO ring
    is exactly one DRAM address stream at all times: load chunks, then
    even rows quarter-by-quarter (expansions complete while the loads
    drain), then the odd-row families back-to-back.  Floor: 42 MiB at
    ~358 GB/s HBM-per-NC = 118us + startup.
    """
    import concourse.bacc as bacc
    import concourse.mybir as mybir
    from concourse.tile import TileContext

    f32 = mybir.dt.float32
    nc = bacc.Bacc()
    x = nc.dram_tensor("x", [_H, _W, _C2], f32, kind="ExternalInput")
    y = nc.dram_tensor("y", [_HO, _WO, _C2], f32, kind="ExternalOutput")

    with TileContext(nc) as tc:
        with (
            tc.tile_pool(name="tin", bufs=1) as tin_pool,
            tc.tile_pool(name="uexp", bufs=1) as u_pool,
        ):
            load_eng = getattr(nc, load_engine_name)
            t3s, u_tiles = [], []
            for q in range(4):
                w0 = 32 * q
                w1 = min(w0 + 33, _W)
                t = tin_pool.tile([_H, (w1 - w0) * _C2], f32, tag=f"t{q}")
                load_eng.dma_start(
                    t[:].rearrange("h (w c) -> h w c", c=_C2), x[:, w0:w1, :]
                )
                t3s.append(t[:].rearrange("h (w c) -> h w c", c=_C2))

            for q in range(4):
                t3 = t3s[q]
                u = u_pool.tile([_H, 64 * _C2], f32, tag=f"u{q}")
                u_tiles.append(u)
                u3 = u[:].rearrange("h (w c) -> h w c", c=_C2)
                up = u3.rearrange("h (p two) c -> h p two c", two=2)
                nc.vector.tensor_copy(
                    up[:, 0:32:2, :, :],
                    t3[:, _sl(0, 2, 16), :]
                    .unsqueeze(2)
                    .broadcast_to([_H, 16, 2, _C2]),
                )
                nct = 15 if q == 3 else 16
                nc.vector.tensor_copy(
                    up[:, 1 : 2 * nct : 2, :, :],
                    t3[:, 1 : 2 * nct + 1, :].rearrange(
                        "h (g two) c -> h g two c", two=2
                    ),
                )
                if q == 3:
                    nc.vector.tensor_copy(
                        u3[:, 62:64, :],
                        t3[:, 31:32, :].broadcast_to([_H, 2, _C2]),
                    )

            # All writes on the single SWDGE FIFO ring, family-major.
            for fam in range(4):
                rd0, rds, rs0, rss, rcnt = _FAMILIES[fam]
                for q in range(4):
                    nc.gpsimd.dma_start(
                        y[_sl(rd0, rds, rcnt), 64 * q : 64 * (q + 1), :],
                        u_tiles[q][_sl(rs0, rss, rcnt), :],
                    )
    nc.compile()
    return nc


def _build_nc_v6():
    """v5 but loads on the sync HWDGE ring, overlapping the SWDGE write
    stream (writes start ~13us instead of ~29us; costs read/write
    stream mixing during the overlap window)."""
    return _build_nc_v5(load_engine_name="sync")


def _build_nc_v7():
    """v6 + merged odd-row writes at 256 KiB pitch.

    Trace evidence (v6 @139us): fam0 (even rows, descriptor pitch
    256 KiB) sustains ~27.5 GB/s/engine (~440 GB/s aggregate), but the
    separate fam1/fam2 passes (pitch 512 KiB) drop to ~13-18 GB/s per
    engine, and the row-255 writes dribble 2 KiB descriptors for the
    last ~15us.  Fix: pair output rows (4k+3, 4k+5), which share source
    row 2k+2, via a stride-0 free-dim broadcast on the SBUF side — one
    DMA per quarter covers odd rows 3..253 with 32 KiB descriptors
    ascending at 256 KiB pitch, exactly like fam0.  Rows 1 and 255 are
    a single 2-descriptor edge DMA per quarter.
    """
    import concourse.bacc as bacc
    import concourse.mybir as mybir
    from concourse.tile import TileContext

    f32 = mybir.dt.float32
    nc = bacc.Bacc()
    x = nc.dram_tensor("x", [_H, _W, _C2], f32, kind="ExternalInput")
    y = nc.dram_tensor("y", [_HO, _WO, _C2], f32, kind="ExternalOutput")

    with TileContext(nc) as tc:
        with (
            tc.tile_pool(name="tin", bufs=1) as tin_pool,
            tc.tile_pool(name="uexp", bufs=1) as u_pool,
        ):
            t3s, u_tiles = [], []
            for q in range(4):
                w0 = 32 * q
                w1 = min(w0 + 33, _W)
                t = tin_pool.tile([_H, (w1 - w0) * _C2], f32, tag=f"t{q}")
                nc.sync.dma_start(
                    t[:].rearrange("h (w c) -> h w c", c=_C2), x[:, w0:w1, :]
                )
                t3s.append(t[:].rearrange("h (w c) -> h w c", c=_C2))

            for q in range(4):
                t3 = t3s[q]
                u = u_pool.tile([_H, 64 * _C2], f32, tag=f"u{q}")
                u_tiles.append(u)
                u3 = u[:].rearrange("h (w c) -> h w c", c=_C2)
                up = u3.rearrange("h (p two) c -> h p two c", two=2)
                nc.vector.tensor_copy(
                    up[:, 0:32:2, :, :],
                    t3[:, _sl(0, 2, 16), :]
                    .unsqueeze(2)
                    .broadcast_to([_H, 16, 2, _C2]),
                )
                nct = 15 if q == 3 else 16
                nc.vector.tensor_copy(
                    up[:, 1 : 2 * nct : 2, :, :],
                    t3[:, 1 : 2 * nct + 1, :].rearrange(
                        "h (g two) c -> h g two c", two=2
                    ),
                )
                if q == 3:
                    nc.vector.tensor_copy(
                        u3[:, 62:64, :],
                        t3[:, 31:32, :].broadcast_to([_H, 2, _C2]),
                    )

            # All writes on the single SWDGE FIFO ring, quarter-major so
            # quarter q's stream starts as soon as its expansion lands.
            for q in range(4):
                u = u_tiles[q]
                cols = slice(64 * q, 64 * (q + 1))
                # Even rows 0,2,...,254 <- u[0..127]: 128 descs, 256 KiB pitch.
                nc.gpsimd.dma_start(y[_sl(0, 2, 128), cols, :], u[:, :])
                # Odd rows 3..253: pairs (4k+3, 4k+5) <- u[2k+2] twice.
                nc.gpsimd.dma_start(
                    y[_sl(3, 2, 126), cols, :].rearrange(
                        "(k two) w c -> k two w c", two=2
                    ),
                    u[_sl(2, 2, 63), :].unsqueeze(1).broadcast_to([63, 2, 64 * _C2]),
                )
                # Edge rows (1, 255) <- u[(0, 127)]: 2 descs.
                nc.gpsimd.dma_start(
                    y[_sl(1, 254, 2), cols, :], u[_sl(0, 127, 2), :]
                )
    nc.compile()
    return nc


def _build_nc_v9():
    """Column-HALF U tiles -> 64 KiB write descriptors.

    v7 showed stride-0 broadcast source descriptors drain at ~13 GB/s
    per engine — dead end.  Back to v6's two-pass row families, but the
    expanded image is staged as two half-width tiles (u_L = output cols
    0..127, u_R = 128..255; 64 KiB per partition each), so every write
    descriptor is 64 KiB (4x v6) and the whole output takes ~510
    descriptors instead of ~1030.  fam0_L starts once quarters 0-1 are
    expanded (~21us).  All writes on the single SWDGE FIFO ring.
    """
    import concourse.bacc as bacc
    import concourse.mybir as mybir
    from concourse.tile import TileContext

    f32 = mybir.dt.float32
    nc = bacc.Bacc()
    x = nc.dram_tensor("x", [_H, _W, _C2], f32, kind="ExternalInput")
    y = nc.dram_tensor("y", [_HO, _WO, _C2], f32, kind="ExternalOutput")

    with TileContext(nc) as tc:
        with (
            tc.tile_pool(name="tin", bufs=1) as tin_pool,
            tc.tile_pool(name="uexp", bufs=1) as u_pool,
        ):
            t3s = []
            for q in range(4):
                w0 = 32 * q
                w1 = min(w0 + 33, _W)
                t = tin_pool.tile([_H, (w1 - w0) * _C2], f32, tag=f"t{q}")
                nc.sync.dma_start(
                    t[:].rearrange("h (w c) -> h w c", c=_C2), x[:, w0:w1, :]
                )
                t3s.append(t[:].rearrange("h (w c) -> h w c", c=_C2))

            u_halves = [
                u_pool.tile([_H, 128 * _C2], f32, tag="uL", name="uL"),
                u_pool.tile([_H, 128 * _C2], f32, tag="uR", name="uR"),
            ]
            for q in range(4):
                t3 = t3s[q]
                u3h = u_halves[q // 2][:].rearrange("h (w c) -> h w c", c=_C2)
                u3 = u3h[:, 64 * (q % 2) : 64 * (q % 2) + 64, :]
                up = u3.rearrange("h (p two) c -> h p two c", two=2)
                nc.vector.tensor_copy(
                    up[:, 0:32:2, :, :],
                    t3[:, _sl(0, 2, 16), :]
                    .unsqueeze(2)
                    .broadcast_to([_H, 16, 2, _C2]),
                )
                nct = 15 if q == 3 else 16
                nc.vector.tensor_copy(
                    up[:, 1 : 2 * nct : 2, :, :],
                    t3[:, 1 : 2 * nct + 1, :].rearrange(
                        "h (g two) c -> h g two c", two=2
                    ),
                )
                if q == 3:
                    nc.vector.tensor_copy(
                        u3[:, 62:64, :],
                        t3[:, 31:32, :].broadcast_to([_H, 2, _C2]),
                    )

            # Writes: single SWDGE FIFO ring, family-major, halves inner.
            for fam in range(4):
                rd0, rds, rs0, rss, rcnt = _FAMILIES[fam]
                for h in range(2):
                    cols = slice(128 * h, 128 * (h + 1))
                    nc.gpsimd.dma_start(
                        y[_sl(rd0, rds, rcnt), cols, :],
                        u_halves[h][_sl(rs0, rss, rcnt), :],
                    )
    nc.compile()
    return nc


def _build_nc_v10():
    """Loads first on the SWDGE ring + half-width U + odd-pair broadcast.

    v9 lesson: writes starve concurrent HWDGE loads (packet round-robin
    shares engines, bandwidth goes to whoever has descriptors), so late
    chunks -> late expansions -> 14us ring stall.  Put the loads at the
    head of the one SWDGE FIFO ring; all expansions finish while the
    8.6 MB load drains, so the write stream that follows never stalls.
    Writes are 64 KiB descriptors (half-width U tiles): even rows
    (256 KiB pitch), then odd rows 3..253 merged via stride-0 pair
    broadcast (256 KiB pitch), then 2-descriptor edge rows (1, 255).
    """
    import concourse.bacc as bacc
    import concourse.mybir as mybir
    from concourse.tile import TileContext

    f32 = mybir.dt.float32
    nc = bacc.Bacc()
    x = nc.dram_tensor("x", [_H, _W, _C2], f32, kind="ExternalInput")
    y = nc.dram_tensor("y", [_HO, _WO, _C2], f32, kind="ExternalOutput")

    with TileContext(nc) as tc:
        with (
            tc.tile_pool(name="tin", bufs=1) as tin_pool,
            tc.tile_pool(name="uexp", bufs=1) as u_pool,
        ):
            t3s = []
            for q in range(4):
                w0 = 32 * q
                w1 = min(w0 + 33, _W)
                t = tin_pool.tile([_H, (w1 - w0) * _C2], f32, tag=f"t{q}")
                nc.gpsimd.dma_start(
                    t[:].rearrange("h (w c) -> h w c", c=_C2), x[:, w0:w1, :]
                )
                t3s.append(t[:].rearrange("h (w c) -> h w c", c=_C2))

            u_halves = [
                u_pool.tile([_H, 128 * _C2], f32, tag="uL", name="uL"),
                u_pool.tile([_H, 128 * _C2], f32, tag="uR", name="uR"),
            ]
            for q in range(4):
                t3 = t3s[q]
                u3h = u_halves[q // 2][:].rearrange("h (w c) -> h w c", c=_C2)
                u3 = u3h[:, 64 * (q % 2) : 64 * (q % 2) + 64, :]
                up = u3.rearrange("h (p two) c -> h p two c", two=2)
                nc.vector.tensor_copy(
                    up[:, 0:32:2, :, :],
                    t3[:, _sl(0, 2, 16), :]
                    .unsqueeze(2)
                    .broadcast_to([_H, 16, 2, _C2]),
                )
                nct = 15 if q == 3 else 16
                nc.vector.tensor_copy(
                    up[:, 1 : 2 * nct : 2, :, :],
                    t3[:, 1 : 2 * nct + 1, :].rearrange(
                        "h (g two) c -> h g two c", two=2
                    ),
                )
                if q == 3:
                    nc.vector.tensor_copy(
                        u3[:, 62:64, :],
                        t3[:, 31:32, :].broadcast_to([_H, 2, _C2]),
                    )

            for h in range(2):
                u = u_halves[h]
                cols = slice(128 * h, 128 * (h + 1))
                # Even rows 0..254: 128 descs of 64 KiB, 256 KiB pitch.
                nc.gpsimd.dma_start(y[_sl(0, 2, 128), cols, :], u[:, :])
                # Edge rows (1, 255) <- u[(0, 127)]: 2 descs (mid-stream).
                nc.gpsimd.dma_start(y[_sl(1, 254, 2), cols, :], u[_sl(0, 127, 2), :])
                # Odd rows 3..253: pairs (4k+3, 4k+5) <- u[2k+2] twice,
                # 126 descs of 64 KiB, 256 KiB pitch.
                nc.gpsimd.dma_start(
                    y[_sl(3, 2, 126), cols, :].rearrange(
                        "(k two) w c -> k two w c", two=2
                    ),
                    u[_sl(2, 2, 63), :].unsqueeze(1).broadcast_to([63, 2, 128 * _C2]),
                )
    nc.compile()
    return nc


def _v1x_common(nc, mybir, tc, tin_pool, u_pool, f32, x):
    """Shared front half: chunk loads on the SWDGE ring head + DVE
    expansion into two half-width U tiles.  Returns u_halves."""
    t3s = []
    for q in range(4):
        w0 = 32 * q
        w1 = min(w0 + 33, _W)
        t = tin_pool.tile([_H, (w1 - w0) * _C2], f32, tag=f"t{q}", name=f"t{q}")
        nc.gpsimd.dma_start(
            t[:].rearrange("h (w c) -> h w c", c=_C2), x[:, w0:w1, :]
        )
        t3s.append(t[:].rearrange("h (w c) -> h w c", c=_C2))

    u_halves = [
        u_pool.tile([_H, 128 * _C2], f32, tag="uL", name="uL"),
        u_pool.tile([_H, 128 * _C2], f32, tag="uR", name="uR"),
    ]
    for q in range(4):
        t3 = t3s[q]
        u3h = u_halves[q // 2][:].rearrange("h (w c) -> h w c", c=_C2)
        u3 = u3h[:, 64 * (q % 2) : 64 * (q % 2) + 64, :]
        up = u3.rearrange("h (p two) c -> h p two c", two=2)
        nc.vector.tensor_copy(
            up[:, 0:32:2, :, :],
            t3[:, _sl(0, 2, 16), :].unsqueeze(2).broadcast_to([_H, 16, 2, _C2]),
        )
        nct = 15 if q == 3 else 16
        nc.vector.tensor_copy(
            up[:, 1 : 2 * nct : 2, :, :],
            t3[:, 1 : 2 * nct + 1, :].rearrange("h (g two) c -> h g two c", two=2),
        )
        if q == 3:
            nc.vector.tensor_copy(
                u3[:, 62:64, :],
                t3[:, 31:32, :].broadcast_to([_H, 2, _C2]),
            )
    return u_halves


def _build_nc_v11():
    """Loads-first + concurrent 3-ring family writes.

    After the loads drain (~29us, all expansions done), fam0 goes on the
    SWDGE ring while fam1 rides sync and fam2 rides scalar — three
    interleaved streams whose merged address coverage is near-dense
    ascending, testing whether lockstep interleave beats sequential
    strided sweeps."""
    import concourse.bacc as bacc
    import concourse.mybir as mybir
    from concourse.tile import TileContext

    f32 = mybir.dt.float32
    nc = bacc.Bacc()
    x = nc.dram_tensor("x", [_H, _W, _C2], f32, kind="ExternalInput")
    y = nc.dram_tensor("y", [_HO, _WO, _C2], f32, kind="ExternalOutput")

    with TileContext(nc) as tc:
        with (
            tc.tile_pool(name="tin", bufs=1) as tin_pool,
            tc.tile_pool(name="uexp", bufs=1) as u_pool,
        ):
            u_halves = _v1x_common(nc, mybir, tc, tin_pool, u_pool, f32, x)
            for h in range(2):
                u = u_halves[h]
                cols = slice(128 * h, 128 * (h + 1))
                nc.gpsimd.dma_start(y[_sl(0, 2, 128), cols, :], u[:, :])
                nc.gpsimd.dma_start(
                    y[_sl(1, 254, 2), cols, :], u[_sl(0, 127, 2), :]
                )
            for h in range(2):
                cols = slice(128 * h, 128 * (h + 1))
                rd0, rds, rs0, rss, rcnt = _FAMILIES[1]
                nc.sync.dma_start(
                    y[_sl(rd0, rds, rcnt), cols, :],
                    u_halves[h][_sl(rs0, rss, rcnt), :],
                )
                rd0, rds, rs0, rss, rcnt = _FAMILIES[2]
                nc.scalar.dma_start(
                    y[_sl(rd0, rds, rcnt), cols, :],
                    u_halves[h][_sl(rs0, rss, rcnt), :],
                )
    nc.compile()
    return nc


def _build_nc_v12():
    """Loads-first + all-SWDGE with fam1/fam2 interleaved in 1 MiB
    sub-DMAs (16 partitions each), so the FIFO ring's merged odd-row
    stream walks the address space densely at 256 KiB pitch instead of
    two full 512 KiB-pitch passes."""
    import concourse.bacc as bacc
    import concourse.mybir as mybir
    from concourse.tile import TileContext

    f32 = mybir.dt.float32
    nc = bacc.Bacc()
    x = nc.dram_tensor("x", [_H, _W, _C2], f32, kind="ExternalInput")
    y = nc.dram_tensor("y", [_HO, _WO, _C2], f32, kind="ExternalOutput")

    with TileContext(nc) as tc:
        with (
            tc.tile_pool(name="tin", bufs=1) as tin_pool,
            tc.tile_pool(name="uexp", bufs=1) as u_pool,
        ):
            u_halves = _v1x_common(nc, mybir, tc, tin_pool, u_pool, f32, x)
            for h in range(2):
                u = u_halves[h]
                cols = slice(128 * h, 128 * (h + 1))
                nc.gpsimd.dma_start(y[_sl(0, 2, 128), cols, :], u[:, :])
                nc.gpsimd.dma_start(
                    y[_sl(1, 254, 2), cols, :], u[_sl(0, 127, 2), :]
                )
                # Odd rows: alternate fam1/fam2 blocks of 16 partitions
                # (16 x 64 KiB = 1 MiB per sub-DMA) walking forward.
                for b in range(4):
                    # fam1 rows 1+4t for t in [16b, 16b+16)
                    nc.gpsimd.dma_start(
                        y[_sl(1 + 64 * b, 4, 16), cols, :],
                        u[_sl(32 * b, 2, 16), :],
                    )
                    # fam2 rows 3+4t for t in [16b, 16b+16) (15 in last)
                    ncnt = 15 if b == 3 else 16
                    nc.gpsimd.dma_start(
                        y[_sl(3 + 64 * b, 4, ncnt), cols, :],
                        u[_sl(2 + 32 * b, 2, ncnt), :],
                    )
    nc.compile()
    return nc


def _build_nc_v13():
    """Loads-first + one full-row U tile: every write descriptor is
    128 KiB.  Tests whether doubling descriptor size rescues the
    512 KiB-pitch odd families."""
    import concourse.bacc as bacc
    import concourse.mybir as mybir
    from concourse.tile import TileContext

    f32 = mybir.dt.float32
    nc = bacc.Bacc()
    x = nc.dram_tensor("x", [_H, _W, _C2], f32, kind="ExternalInput")
    y = nc.dram_tensor("y", [_HO, _WO, _C2], f32, kind="ExternalOutput")

    with TileContext(nc) as tc:
        with (
            tc.tile_pool(name="tin", bufs=1) as tin_pool,
            tc.tile_pool(name="uexp", bufs=1) as u_pool,
        ):
            t3s = []
            for q in range(4):
                w0 = 32 * q
                w1 = min(w0 + 33, _W)
                t = tin_pool.tile(
                    [_H, (w1 - w0) * _C2], f32, tag=f"t{q}", name=f"t{q}"
                )
                nc.gpsimd.dma_start(
                    t[:].rearrange("h (w c) -> h w c", c=_C2), x[:, w0:w1, :]
                )
                t3s.append(t[:].rearrange("h (w c) -> h w c", c=_C2))

            u = u_pool.tile([_H, 256 * _C2], f32, tag="u", name="u")
            u3f = u[:].rearrange("h (w c) -> h w c", c=_C2)
            for q in range(4):
                t3 = t3s[q]
                u3 = u3f[:, 64 * q : 64 * q + 64, :]
                up = u3.rearrange("h (p two) c -> h p two c", two=2)
                nc.vector.tensor_copy(
                    up[:, 0:32:2, :, :],
                    t3[:, _sl(0, 2, 16), :]
                    .unsqueeze(2)
                    .broadcast_to([_H, 16, 2, _C2]),
                )
                nct = 15 if q == 3 else 16
                nc.vector.tensor_copy(
                    up[:, 1 : 2 * nct : 2, :, :],
                    t3[:, 1 : 2 * nct + 1, :].rearrange(
                        "h (g two) c -> h g two c", two=2
                    ),
                )
                if q == 3:
                    nc.vector.tensor_copy(
                        u3[:, 62:64, :],
                        t3[:, 31:32, :].broadcast_to([_H, 2, _C2]),
                    )

            for fam in range(4):
                rd0, rds, rs0, rss, rcnt = _FAMILIES[fam]
                nc.gpsimd.dma_start(
                    y[_sl(rd0, rds, rcnt), :, :],
                    u[_sl(rs0, rss, rcnt), :],
                )
    nc.compile()
    return nc


VERSION = 13
_BUILDERS = {
    1: _build_nc_v1,
    2: _build_nc_v2,
    3: _build_nc_v3,
    4: _build_nc_v4,
    5: _build_nc_v5,
    6: _build_nc_v6,
    7: _build_nc_v7,
    9: _build_nc_v9,
    10: _build_nc_v10,
    11: _build_nc_v11,
    12: _build_nc_v12,
    13: _build_nc_v13,
}


def _selftest_families():
    """Host-side check: the family decomposition reproduces the reference
    round-half-to-even nearest index map exactly."""
    idx = np.round(128 * np.arange(256, dtype=np.float64) / 256.0)
    # np.round is round-half-to-even like jnp.round
    idx = np.clip(idx.astype(np.int64), 0, 127)
    recon = np.full(256, -1)
    for d0, ds, s0, ss, c in _FAMILIES:
        for i in range(c):
            assert recon[d0 + ds * i] == -1
            recon[d0 + ds * i] = s0 + ss * i
    assert (recon == idx).all()


_selftest_families()


def _build_nc():
    return _BUILDERS[VERSION]()


def _get_nc():
    if VERSION not in _NC_CACHE:
        _NC_CACHE[VERSION] = _build_nc()
    return _NC_CACHE[VERSION]


def kernel(x_real: np.ndarray, x_imag: np.ndarray) -> np.ndarray:
    global LAST_RESULT
    _ensure_axon_ntff_hook()
    from concourse.bass_utils import run_bass_kernel_spmd

    assert x_real.shape == (_B, _H, _W, _C) and x_imag.shape == (_B, _H, _W, _C)

    # Interleave real/imag channel-wise: f32 [B, H, W, 2C]; pairs
    # (re, im) match the complex64 memory layout.
    xc = np.empty((_B, _H, _W, _C, 2), np.float32)
    xc[..., 0] = x_real
    xc[..., 1] = x_imag
    xc = xc.reshape(_B, _H, _W, _C2)

    nc = _get_nc()
    in_maps = [{"x": xc[b]} for b in range(_B)]
    res = run_bass_kernel_spmd(
        nc,
        in_maps,
        core_ids=list(range(_N_CORES)),
        trace=TRACE,
    )
    LAST_RESULT = res

    out = np.stack([res.results[b]["y"] for b in range(_B)])
    # [B, 256, 256, 128] f32 -> complex64 view [B, 256, 256, 64]
    return out.view(np.complex64)



# revision 14
# speedup vs baseline: 1.3671x; 1.0636x over previous
"""Complex nearest-neighbor 2x spatial upsample on 8 TRN2 NeuronCores.

Reference op: x = x_real + 1j*x_imag, shape [8, 128, 128, 64] (B,H,W,C);
out[b, j, k, c] = x[b, r(j), r(k), c] with
r(j) = clip(round_half_to_even(j/2), 0, 127), output [8, 256, 256, 64]
complex64.

Strategy (batch-sharded, 1 sample per core):
  - Host: interleave real/imag into f32 [H, W, 2C] so a complex "pixel"
    is one contiguous 512B chunk and the complex64 output is a pure view.
  - Device: stage the 8 MiB sample in SBUF (128 rows -> 128 partitions),
    then scatter to the 32 MiB output with strided DMAs.  The
    round-half-to-even gather decomposes exactly into 4 affine families
    per axis, so 4x4 = 16 DRAM-write DMAs with 3-dim access patterns
    (rows, cols, 512B contiguous pixel) cover the whole output.
"""

import numpy as np

_B, _H, _W, _C = 8, 128, 128, 64
_C2 = 2 * _C
_HO, _WO = 2 * _H, 2 * _W
_N_CORES = 8

# Affine families of j -> r(j) = clip(round_half_even(j/2), 0, 127), j in [0,256):
#   j = 2m   -> m      (m = 0..127)
#   j = 4t+1 -> 2t     (t = 0..63)
#   j = 4t+3 -> 2t+2   (t = 0..62)
#   j = 255  -> 127
# Tuples: (dst_start, dst_step, src_start, src_step, count)
_FAMILIES = [
    (0, 2, 0, 1, 128),
    (1, 4, 0, 2, 64),
    (3, 4, 2, 2, 63),
    (255, 1, 127, 1, 1),
]

# Set by test harnesses: TRACE=True makes kernel() profile the run and
# stash the BassKernelResults (incl. exec_time_ns) in LAST_RESULT.
TRACE = False
LAST_RESULT = None

_NC_CACHE = {}


def _ensure_axon_ntff_hook():
    """Provide antenv.axon_hooks when the image ships only the antenv stub.

    concourse.bass_utils imports it for trace=True under axon; the slim
    agent image's boot fails to register the hook because the stub antenv
    package has no axon_hooks submodule.  Recreate the ctypes-based NTFF
    hook against libaxon_pjrt.so (same recipe as trn_agent_boot.trn_boot).
    """
    try:
        import antenv.axon_hooks  # noqa: F401

        return
    except ImportError:
        pass

    import contextlib
    import ctypes
    import sys
    import types

    mod = types.ModuleType("antenv.axon_hooks")
    holder = {"hook": None}

    def set_axon_ntff_profile_hook(hook):
        holder["hook"] = hook

    def get_axon_ntff_profile_hook():
        return holder["hook"]

    mod.set_axon_ntff_profile_hook = set_axon_ntff_profile_hook
    mod.get_axon_ntff_profile_hook = get_axon_ntff_profile_hook
    sys.modules["antenv.axon_hooks"] = mod
    try:
        import antenv

        antenv.axon_hooks = mod
    except ImportError:
        pass

    so_path = "/opt/axon/libaxon_pjrt.so"
    try:
        lib = ctypes.CDLL(so_path)
    except OSError:
        return
    if not hasattr(lib, "axon_start_nrt_profile"):
        return
    lib.axon_start_nrt_profile.argtypes = [
        ctypes.POINTER(ctypes.c_int64),
        ctypes.c_size_t,
    ]
    lib.axon_start_nrt_profile.restype = ctypes.c_int64
    lib.axon_stop_nrt_profile.argtypes = [ctypes.c_char_p]
    lib.axon_stop_nrt_profile.restype = ctypes.c_int64

    @contextlib.contextmanager
    def _hook(output_dir, device_ids):
        import jax

        jax.devices()
        if device_ids:
            ids = (ctypes.c_int64 * len(device_ids))(*device_ids)
            rc = lib.axon_start_nrt_profile(ids, len(device_ids))
        else:
            rc = lib.axon_start_nrt_profile(None, 0)
        if rc != 0:
            raise RuntimeError(f"axon_start_nrt_profile rc={rc}")
        try:
            yield
        finally:
            n = lib.axon_stop_nrt_profile(str(output_dir).encode())
            if n < 0:
                raise RuntimeError(f"axon_stop_nrt_profile rc={n}")

    set_axon_ntff_profile_hook(_hook)


def _sl(start, step, count):
    return slice(start, start + (count - 1) * step + 1, step)


def _build_nc_v1():
    """Pure-DMA scatter: 16 strided DMAs with 512B descriptors.

    Measured 165 us/core: descriptor-rate limited (all 16 SDMA engines
    ~100% busy at ~30 ns per 512B descriptor)."""
    import concourse.bacc as bacc
    import concourse.mybir as mybir
    from concourse.tile import TileContext

    nc = bacc.Bacc()
    x = nc.dram_tensor("x", [_H, _W, _C2], mybir.dt.float32, kind="ExternalInput")
    y = nc.dram_tensor("y", [_HO, _WO, _C2], mybir.dt.float32, kind="ExternalOutput")

    with TileContext(nc) as tc:
        with tc.tile_pool(name="stage", bufs=1) as pool:
            t = pool.tile([_H, _W * _C2], mybir.dt.float32)
            t3 = t[:].rearrange("h (w c) -> h w c", c=_C2)
            # 8 MiB load: one contiguous 64 KiB row per partition.
            nc.sync.dma_start(t[:], x[:].rearrange("h w c -> h (w c)"))
            # 16 strided scatter DMAs, alternating between the two HWDGE
            # rings (sync + scalar) so they drain in parallel.
            engines = [nc.sync, nc.scalar]
            i = 0
            for rd0, rds, rs0, rss, rc in _FAMILIES:
                for cd0, cds, cs0, css, cc in _FAMILIES:
                    eng = engines[i % len(engines)]
                    i += 1
                    eng.dma_start(
                        y[_sl(rd0, rds, rc), _sl(cd0, cds, cc), :],
                        t3[_sl(rs0, rss, rc), _sl(cs0, css, cc), :],
                    )
    nc.compile()
    return nc


def _build_nc_v2():
    """On-chip column expansion + contiguous-row scatter.

    Input rows live one-per-partition.  The vector engine expands the
    column (W) axis into U tiles (64 output cols per quarter, 32 KiB per
    partition), then each quarter is written out with 4 row-family DMAs
    whose descriptors are 32 KiB contiguous — DMA runs at line rate
    instead of the 512B descriptor floor of v1.
    """
    import concourse.bacc as bacc
    import concourse.mybir as mybir
    from concourse.tile import TileContext

    f32 = mybir.dt.float32
    nc = bacc.Bacc()
    x = nc.dram_tensor("x", [_H, _W, _C2], f32, kind="ExternalInput")
    y = nc.dram_tensor("y", [_HO, _WO, _C2], f32, kind="ExternalOutput")

    with TileContext(nc) as tc:
        with (
            tc.tile_pool(name="tin", bufs=1) as tin_pool,
            tc.tile_pool(name="uexp", bufs=3) as u_pool,
        ):
            # Input halves: t_lo = cols 0..64 (65 cols, needed by output
            # quarters 0-1), t_hi = cols 64..127 (needed by quarters 2-3).
            t_lo = tin_pool.tile([_H, 65 * _C2], f32, tag="tlo")
            t_hi = tin_pool.tile([_H, 64 * _C2], f32, tag="thi")
            nc.gpsimd.dma_start(
                t_lo[:].rearrange("h (w c) -> h w c", c=_C2), x[:, 0:65, :]
            )
            nc.gpsimd.dma_start(
                t_hi[:].rearrange("h (w c) -> h w c", c=_C2), x[:, 64:128, :]
            )

            out_engines = [nc.sync, nc.scalar]
            n_out = 0
            for q in range(4):
                t = t_lo if q < 2 else t_hi
                base = 32 * q if q < 2 else 32 * (q - 2)
                t3 = t[:].rearrange("h (w c) -> h w c", c=_C2)
                u = u_pool.tile([_H, 64 * _C2], f32, tag="u")
                u3 = u[:].rearrange("h (w c) -> h w c", c=_C2)
                # Quarter cols j=4t+{0,1,2,3} (t=0..15) read input cols
                # base + {2t, 2t, 2t+1, 2t+2} (locals within t_lo/t_hi).
                # View the 64 quarter cols as 32 pairs: even pairs p=2t are
                # cols (4t, 4t+1), odd pairs cols (4t+2, 4t+3).
                up = u3.rearrange("h (p two) c -> h p two c", two=2)
                # A/B fused: dst pairs (4t, 4t+1) <- src col base+2t twice
                # (stride-0 broadcast of the pair dim).
                nc.vector.tensor_copy(
                    up[:, 0:32:2, :, :],
                    t3[:, _sl(base, 2, 16), :]
                    .unsqueeze(2)
                    .broadcast_to([_H, 16, 2, _C2]),
                )
                # C: dst pairs (4t+2, 4t+3) <- src cols (base+2t+1,
                # base+2t+2) contiguous... except the clipped tail in q3.
                nct = 15 if q == 3 else 16
                nc.vector.tensor_copy(
                    up[:, 1 : 2 * nct : 2, :, :],
                    t3[:, base + 1 : base + 2 * nct + 1, :].rearrange(
                        "h (g two) c -> h g two c", two=2
                    ),
                )
                if q == 3:
                    # cols 254, 255 <- input col 127 (local 63) twice.
                    nc.vector.tensor_copy(
                        u3[:, 62:64, :],
                        t3[:, 63:64, :].broadcast_to([_H, 2, _C2]),
                    )
                # Scatter: 4 row families, 32 KiB contiguous descriptors.
                for rd0, rds, rs0, rss, rcnt in _FAMILIES:
                    eng = out_engines[n_out % len(out_engines)]
                    n_out += 1
                    eng.dma_start(
                        y[_sl(rd0, rds, rcnt), 64 * q : 64 * (q + 1), :],
                        u[_sl(rs0, rss, rcnt), :],
                    )
    nc.compile()
    return nc


def _build_nc_v3():
    """v2 + uniform DMA-engine load.

    v2's HWDGE sync ring fed SDMA engines 0-8 ~2x the descriptors of
    9-15, serializing a long tail.  The SWDGE (gpsimd) queue spreads
    descriptors across all 16 engines evenly (observed), so route every
    DMA through it.  Input is loaded as 4 per-quarter column chunks
    (contiguous per row) so each quarter's expansion only waits for its
    own ~2 MiB load.
    """
    import concourse.bacc as bacc
    import concourse.mybir as mybir
    from concourse.tile import TileContext

    f32 = mybir.dt.float32
    nc = bacc.Bacc()
    x = nc.dram_tensor("x", [_H, _W, _C2], f32, kind="ExternalInput")
    y = nc.dram_tensor("y", [_HO, _WO, _C2], f32, kind="ExternalOutput")

    with TileContext(nc) as tc:
        with (
            tc.tile_pool(name="tin", bufs=1) as tin_pool,
            tc.tile_pool(name="uexp", bufs=3) as u_pool,
        ):
            # Quarter q of the output (cols 64q..64q+64) reads input cols
            # 32q..32q+32 inclusive -> 33-col chunks (32 for q3).
            t_chunks = []
            for q in range(4):
                w0 = 32 * q
                w1 = min(w0 + 33, _W)
                t = tin_pool.tile([_H, (w1 - w0) * _C2], f32, tag=f"t{q}")
                nc.gpsimd.dma_start(
                    t[:].rearrange("h (w c) -> h w c", c=_C2), x[:, w0:w1, :]
                )
                t_chunks.append(t)

            for q in range(4):
                t3 = t_chunks[q][:].rearrange("h (w c) -> h w c", c=_C2)
                u = u_pool.tile([_H, 64 * _C2], f32, tag="u")
                u3 = u[:].rearrange("h (w c) -> h w c", c=_C2)
                up = u3.rearrange("h (p two) c -> h p two c", two=2)
                # A/B fused: dst pairs (4t, 4t+1) <- src local col 2t twice.
                nc.vector.tensor_copy(
                    up[:, 0:32:2, :, :],
                    t3[:, _sl(0, 2, 16), :]
                    .unsqueeze(2)
                    .broadcast_to([_H, 16, 2, _C2]),
                )
                # C: dst pairs (4t+2, 4t+3) <- src local cols (2t+1, 2t+2).
                nct = 15 if q == 3 else 16
                nc.vector.tensor_copy(
                    up[:, 1 : 2 * nct : 2, :, :],
                    t3[:, 1 : 2 * nct + 1, :].rearrange(
                        "h (g two) c -> h g two c", two=2
                    ),
                )
                if q == 3:
                    # cols 254, 255 <- input col 127 (local 31) twice.
                    nc.vector.tensor_copy(
                        u3[:, 62:64, :],
                        t3[:, 31:32, :].broadcast_to([_H, 2, _C2]),
                    )
                for rd0, rds, rs0, rss, rcnt in _FAMILIES:
                    nc.gpsimd.dma_start(
                        y[_sl(rd0, rds, rcnt), 64 * q : 64 * (q + 1), :],
                        u[_sl(rs0, rss, rcnt), :],
                    )
    nc.compile()
    return nc


def _build_nc_v4():
    """v3 + DRAM-friendly write sequencing.

    Measured: concurrent 4-family scatter runs at 232 GB/s vs 337 GB/s
    for <=2 interleaved streams (stride-2 row writes are free).  So:
    pass 1 streams the even output rows (one address stream, quarter by
    quarter as expansions finish), pass 2 writes the odd-row families
    with at most ~2 streams in flight, enforced with explicit dep edges.
    All 4 U quarters stay resident (no pool recycling stalls).
    """
    import concourse.bacc as bacc
    import concourse.mybir as mybir
    from concourse.bass import _add_dep_helper
    from concourse.tile import TileContext

    f32 = mybir.dt.float32
    nc = bacc.Bacc()
    x = nc.dram_tensor("x", [_H, _W, _C2], f32, kind="ExternalInput")
    y = nc.dram_tensor("y", [_HO, _WO, _C2], f32, kind="ExternalOutput")

    with TileContext(nc) as tc:
        with (
            tc.tile_pool(name="tin", bufs=1) as tin_pool,
            tc.tile_pool(name="uexp", bufs=1) as u_pool,
        ):
            t3s, u_tiles = [], []
            for q in range(4):
                w0 = 32 * q
                w1 = min(w0 + 33, _W)
                t = tin_pool.tile([_H, (w1 - w0) * _C2], f32, tag=f"t{q}")
                # 128-partition loads stay on SWDGE: HWDGE splits
                # 128-partition DMAs 2:1 across engines 0-8 vs 9-15.
                nc.gpsimd.dma_start(
                    t[:].rearrange("h (w c) -> h w c", c=_C2), x[:, w0:w1, :]
                )
                t3s.append(t[:].rearrange("h (w c) -> h w c", c=_C2))

            # Expansion (DVE) into 4 resident U quarters.
            for q in range(4):
                t3 = t3s[q]
                u = u_pool.tile([_H, 64 * _C2], f32, tag=f"u{q}")
                u_tiles.append(u)
                u3 = u[:].rearrange("h (w c) -> h w c", c=_C2)
                up = u3.rearrange("h (p two) c -> h p two c", two=2)
                nc.vector.tensor_copy(
                    up[:, 0:32:2, :, :],
                    t3[:, _sl(0, 2, 16), :]
                    .unsqueeze(2)
                    .broadcast_to([_H, 16, 2, _C2]),
                )
                nct = 15 if q == 3 else 16
                nc.vector.tensor_copy(
                    up[:, 1 : 2 * nct : 2, :, :],
                    t3[:, 1 : 2 * nct + 1, :].rearrange(
                        "h (g two) c -> h g two c", two=2
                    ),
                )
                if q == 3:
                    nc.vector.tensor_copy(
                        u3[:, 62:64, :],
                        t3[:, 31:32, :].broadcast_to([_H, 2, _C2]),
                    )

            # Pass 1: even output rows.  No deps — expansion completion
            # staggers the quarters naturally (~2 streams in flight max).
            re_insts = []
            for q in range(4):
                rd0, rds, rs0, rss, rcnt = _FAMILIES[0]
                d = nc.gpsimd.dma_start(
                    y[_sl(rd0, rds, rcnt), 64 * q : 64 * (q + 1), :],
                    u_tiles[q][_sl(rs0, rss, rcnt), :],
                )
                re_insts.append(d.ins)
            # Pass 2 on the two HWDGE rings: RO1 family streams on sync,
            # RO2 on scalar — each ring is FIFO, so each family is one
            # continuous ascending address stream (2-stream mix total).
            # One boundary per ring: its first DMA waits for pass 1.
            for fam, eng in ((1, nc.sync), (2, nc.scalar)):
                rd0, rds, rs0, rss, rcnt = _FAMILIES[fam]
                for q in range(4):
                    d = eng.dma_start(
                        y[_sl(rd0, rds, rcnt), 64 * q : 64 * (q + 1), :],
                        u_tiles[q][_sl(rs0, rss, rcnt), :],
                    )
                    if q == 0:
                        for p in re_insts:
                            _add_dep_helper(d.ins, p, True, "pass1->pass2 boundary")
            # row 255 (tiny), after everything on the sync ring
            for q in range(4):
                rd0, rds, rs0, rss, rcnt = _FAMILIES[3]
                nc.sync.dma_start(
                    y[_sl(rd0, rds, rcnt), 64 * q : 64 * (q + 1), :],
                    u_tiles[q][_sl(rs0, rss, rcnt), :],
                )
    nc.compile()
    return nc


def _build_nc_v5(load_engine_name="gpsimd"):
    """Single SWDGE ring, strict FIFO order, no barriers.

    Trace evidence (v4 @166us): HWDGE rings split descriptors ~2:1 (up
    to 3:1) across SDMA engines 0-8 vs 9-15, so the pass-2 odd-row
    families ran at ~210 GB/s on 9 busy engines while 7 idled; loads
    serialized ahead of fam0 on the SWDGE ring and pass2 sat behind an
    all-pass1 barrier (first write byte ~37us).  SWDGE distributes
    descriptors evenly across all 16 engines, and a single FIFO ring
    is exactly one DRAM address stream at all times: load chunks, then
    even rows quarter-by-quarter (expansions complete while the loads
    drain), then the odd-row families back-to-back.  Floor: 42 MiB at
    ~358 GB/s HBM-per-NC = 118us + startup.
    """
    import concourse.bacc as bacc
    import concourse.mybir as mybir
    from concourse.tile import TileContext

    f32 = mybir.dt.float32
    nc = bacc.Bacc()
    x = nc.dram_tensor("x", [_H, _W, _C2], f32, kind="ExternalInput")
    y = nc.dram_tensor("y", [_HO, _WO, _C2], f32, kind="ExternalOutput")

    with TileContext(nc) as tc:
        with (
            tc.tile_pool(name="tin", bufs=1) as tin_pool,
            tc.tile_pool(name="uexp", bufs=1) as u_pool,
        ):
            load_eng = getattr(nc, load_engine_name)
            t3s, u_tiles = [], []
            for q in range(4):
                w0 = 32 * q
                w1 = min(w0 + 33, _W)
                t = tin_pool.tile([_H, (w1 - w0) * _C2], f32, tag=f"t{q}")
                load_eng.dma_start(
                    t[:].rearrange("h (w c) -> h w c", c=_C2), x[:, w0:w1, :]
                )
                t3s.append(t[:].rearrange("h (w c) -> h w c", c=_C2))

            for q in range(4):
                t3 = t3s[q]
                u = u_pool.tile([_H, 64 * _C2], f32, tag=f"u{q}")
                u_tiles.append(u)
                u3 = u[:].rearrange("h (w c) -> h w c", c=_C2)
                up = u3.rearrange("h (p two) c -> h p two c", two=2)
                nc.vector.tensor_copy(
                    up[:, 0:32:2, :, :],
                    t3[:, _sl(0, 2, 16), :]
                    .unsqueeze(2)
                    .broadcast_to([_H, 16, 2, _C2]),
                )
                nct = 15 if q == 3 else 16
                nc.vector.tensor_copy(
                    up[:, 1 : 2 * nct : 2, :, :],
                    t3[:, 1 : 2 * nct + 1, :].rearrange(
                        "h (g two) c -> h g two c", two=2
                    ),
                )
                if q == 3:
                    nc.vector.tensor_copy(
                        u3[:, 62:64, :],
                        t3[:, 31:32, :].broadcast_to([_H, 2, _C2]),
                    )

            # All writes on the single SWDGE FIFO ring, family-major.
            for fam in range(4):
                rd0, rds, rs0, rss, rcnt = _FAMILIES[fam]
                for q in range(4):
                    nc.gpsimd.dma_start(
                        y[_sl(rd0, rds, rcnt), 64 * q : 64 * (q + 1), :],
                        u_tiles[q][_sl(rs0, rss, rcnt), :],
                    )
    nc.compile()
    return nc


def _build_nc_v6():
    """v5 but loads on the sync HWDGE ring, overlapping the SWDGE write
    stream (writes start ~13us instead of ~29us; costs read/write
    stream mixing during the overlap window)."""
    return _build_nc_v5(load_engine_name="sync")


def _build_nc_v7():
    """v6 + merged odd-row writes at 256 KiB pitch.

    Trace evidence (v6 @139us): fam0 (even rows, descriptor pitch
    256 KiB) sustains ~27.5 GB/s/engine (~440 GB/s aggregate), but the
    separate fam1/fam2 passes (pitch 512 KiB) drop to ~13-18 GB/s per
    engine, and the row-255 writes dribble 2 KiB descriptors for the
    last ~15us.  Fix: pair output rows (4k+3, 4k+5), which share source
    row 2k+2, via a stride-0 free-dim broadcast on the SBUF side — one
    DMA per quarter covers odd rows 3..253 with 32 KiB descriptors
    ascending at 256 KiB pitch, exactly like fam0.  Rows 1 and 255 are
    a single 2-descriptor edge DMA per quarter.
    """
    import concourse.bacc as bacc
    import concourse.mybir as mybir
    from concourse.tile import TileContext

    f32 = mybir.dt.float32
    nc = bacc.Bacc()
    x = nc.dram_tensor("x", [_H, _W, _C2], f32, kind="ExternalInput")
    y = nc.dram_tensor("y", [_HO, _WO, _C2], f32, kind="ExternalOutput")

    with TileContext(nc) as tc:
        with (
            tc.tile_pool(name="tin", bufs=1) as tin_pool,
            tc.tile_pool(name="uexp", bufs=1) as u_pool,
        ):
            t3s, u_tiles = [], []
            for q in range(4):
                w0 = 32 * q
                w1 = min(w0 + 33, _W)
                t = tin_pool.tile([_H, (w1 - w0) * _C2], f32, tag=f"t{q}")
                nc.sync.dma_start(
                    t[:].rearrange("h (w c) -> h w c", c=_C2), x[:, w0:w1, :]
                )
                t3s.append(t[:].rearrange("h (w c) -> h w c", c=_C2))

            for q in range(4):
                t3 = t3s[q]
                u = u_pool.tile([_H, 64 * _C2], f32, tag=f"u{q}")
                u_tiles.append(u)
                u3 = u[:].rearrange("h (w c) -> h w c", c=_C2)
                up = u3.rearrange("h (p two) c -> h p two c", two=2)
                nc.vector.tensor_copy(
                    up[:, 0:32:2, :, :],
                    t3[:, _sl(0, 2, 16), :]
                    .unsqueeze(2)
                    .broadcast_to([_H, 16, 2, _C2]),
                )
                nct = 15 if q == 3 else 16
                nc.vector.tensor_copy(
                    up[:, 1 : 2 * nct : 2, :, :],
                    t3[:, 1 : 2 * nct + 1, :].rearrange(
                        "h (g two) c -> h g two c", two=2
                    ),
                )
                if q == 3:
                    nc.vector.tensor_copy(
                        u3[:, 62:64, :],
                        t3[:, 31:32, :].broadcast_to([_H, 2, _C2]),
                    )

            # All writes on the single SWDGE FIFO ring, quarter-major so
            # quarter q's stream starts as soon as its expansion lands.
            for q in range(4):
                u = u_tiles[q]
                cols = slice(64 * q, 64 * (q + 1))
                # Even rows 0,2,...,254 <- u[0..127]: 128 descs, 256 KiB pitch.
                nc.gpsimd.dma_start(y[_sl(0, 2, 128), cols, :], u[:, :])
                # Odd rows 3..253: pairs (4k+3, 4k+5) <- u[2k+2] twice.
                nc.gpsimd.dma_start(
                    y[_sl(3, 2, 126), cols, :].rearrange(
                        "(k two) w c -> k two w c", two=2
                    ),
                    u[_sl(2, 2, 63), :].unsqueeze(1).broadcast_to([63, 2, 64 * _C2]),
                )
                # Edge rows (1, 255) <- u[(0, 127)]: 2 descs.
                nc.gpsimd.dma_start(
                    y[_sl(1, 254, 2), cols, :], u[_sl(0, 127, 2), :]
                )
    nc.compile()
    return nc


def _build_nc_v9():
    """Column-HALF U tiles -> 64 KiB write descriptors.

    v7 showed stride-0 broadcast source descriptors drain at ~13 GB/s
    per engine — dead end.  Back to v6's two-pass row families, but the
    expanded image is staged as two half-width tiles (u_L = output cols
    0..127, u_R = 128..255; 64 KiB per partition each), so every write
    descriptor is 64 KiB (4x v6) and the whole output takes ~510
    descriptors instead of ~1030.  fam0_L starts once quarters 0-1 are
    expanded (~21us).  All writes on the single SWDGE FIFO ring.
    """
    import concourse.bacc as bacc
    import concourse.mybir as mybir
    from concourse.tile import TileContext

    f32 = mybir.dt.float32
    nc = bacc.Bacc()
    x = nc.dram_tensor("x", [_H, _W, _C2], f32, kind="ExternalInput")
    y = nc.dram_tensor("y", [_HO, _WO, _C2], f32, kind="ExternalOutput")

    with TileContext(nc) as tc:
        with (
            tc.tile_pool(name="tin", bufs=1) as tin_pool,
            tc.tile_pool(name="uexp", bufs=1) as u_pool,
        ):
            t3s = []
            for q in range(4):
                w0 = 32 * q
                w1 = min(w0 + 33, _W)
                t = tin_pool.tile([_H, (w1 - w0) * _C2], f32, tag=f"t{q}")
                nc.sync.dma_start(
                    t[:].rearrange("h (w c) -> h w c", c=_C2), x[:, w0:w1, :]
                )
                t3s.append(t[:].rearrange("h (w c) -> h w c", c=_C2))

            u_halves = [
                u_pool.tile([_H, 128 * _C2], f32, tag="uL", name="uL"),
                u_pool.tile([_H, 128 * _C2], f32, tag="uR", name="uR"),
            ]
            for q in range(4):
                t3 = t3s[q]
                u3h = u_halves[q // 2][:].rearrange("h (w c) -> h w c", c=_C2)
                u3 = u3h[:, 64 * (q % 2) : 64 * (q % 2) + 64, :]
                up = u3.rearrange("h (p two) c -> h p two c", two=2)
                nc.vector.tensor_copy(
                    up[:, 0:32:2, :, :],
                    t3[:, _sl(0, 2, 16), :]
                    .unsqueeze(2)
                    .broadcast_to([_H, 16, 2, _C2]),
                )
                nct = 15 if q == 3 else 16
                nc.vector.tensor_copy(
                    up[:, 1 : 2 * nct : 2, :, :],
                    t3[:, 1 : 2 * nct + 1, :].rearrange(
                        "h (g two) c -> h g two c", two=2
                    ),
                )
                if q == 3:
                    nc.vector.tensor_copy(
                        u3[:, 62:64, :],
                        t3[:, 31:32, :].broadcast_to([_H, 2, _C2]),
                    )

            # Writes: single SWDGE FIFO ring, family-major, halves inner.
            for fam in range(4):
                rd0, rds, rs0, rss, rcnt = _FAMILIES[fam]
                for h in range(2):
                    cols = slice(128 * h, 128 * (h + 1))
                    nc.gpsimd.dma_start(
                        y[_sl(rd0, rds, rcnt), cols, :],
                        u_halves[h][_sl(rs0, rss, rcnt), :],
                    )
    nc.compile()
    return nc


def _build_nc_v10():
    """Loads first on the SWDGE ring + half-width U + odd-pair broadcast.

    v9 lesson: writes starve concurrent HWDGE loads (packet round-robin
    shares engines, bandwidth goes to whoever has descriptors), so late
    chunks -> late expansions -> 14us ring stall.  Put the loads at the
    head of the one SWDGE FIFO ring; all expansions finish while the
    8.6 MB load drains, so the write stream that follows never stalls.
    Writes are 64 KiB descriptors (half-width U tiles): even rows
    (256 KiB pitch), then odd rows 3..253 merged via stride-0 pair
    broadcast (256 KiB pitch), then 2-descriptor edge rows (1, 255).
    """
    import concourse.bacc as bacc
    import concourse.mybir as mybir
    from concourse.tile import TileContext

    f32 = mybir.dt.float32
    nc = bacc.Bacc()
    x = nc.dram_tensor("x", [_H, _W, _C2], f32, kind="ExternalInput")
    y = nc.dram_tensor("y", [_HO, _WO, _C2], f32, kind="ExternalOutput")

    with TileContext(nc) as tc:
        with (
            tc.tile_pool(name="tin", bufs=1) as tin_pool,
            tc.tile_pool(name="uexp", bufs=1) as u_pool,
        ):
            t3s = []
            for q in range(4):
                w0 = 32 * q
                w1 = min(w0 + 33, _W)
                t = tin_pool.tile([_H, (w1 - w0) * _C2], f32, tag=f"t{q}")
                nc.gpsimd.dma_start(
                    t[:].rearrange("h (w c) -> h w c", c=_C2), x[:, w0:w1, :]
                )
                t3s.append(t[:].rearrange("h (w c) -> h w c", c=_C2))

            u_halves = [
                u_pool.tile([_H, 128 * _C2], f32, tag="uL", name="uL"),
                u_pool.tile([_H, 128 * _C2], f32, tag="uR", name="uR"),
            ]
            for q in range(4):
                t3 = t3s[q]
                u3h = u_halves[q // 2][:].rearrange("h (w c) -> h w c", c=_C2)
                u3 = u3h[:, 64 * (q % 2) : 64 * (q % 2) + 64, :]
                up = u3.rearrange("h (p two) c -> h p two c", two=2)
                nc.vector.tensor_copy(
                    up[:, 0:32:2, :, :],
                    t3[:, _sl(0, 2, 16), :]
                    .unsqueeze(2)
                    .broadcast_to([_H, 16, 2, _C2]),
                )
                nct = 15 if q == 3 else 16
                nc.vector.tensor_copy(
                    up[:, 1 : 2 * nct : 2, :, :],
                    t3[:, 1 : 2 * nct + 1, :].rearrange(
                        "h (g two) c -> h g two c", two=2
                    ),
                )
                if q == 3:
                    nc.vector.tensor_copy(
                        u3[:, 62:64, :],
                        t3[:, 31:32, :].broadcast_to([_H, 2, _C2]),
                    )

            for h in range(2):
                u = u_halves[h]
                cols = slice(128 * h, 128 * (h + 1))
                # Even rows 0..254: 128 descs of 64 KiB, 256 KiB pitch.
                nc.gpsimd.dma_start(y[_sl(0, 2, 128), cols, :], u[:, :])
                # Edge rows (1, 255) <- u[(0, 127)]: 2 descs (mid-stream).
                nc.gpsimd.dma_start(y[_sl(1, 254, 2), cols, :], u[_sl(0, 127, 2), :])
                # Odd rows 3..253: pairs (4k+3, 4k+5) <- u[2k+2] twice,
                # 126 descs of 64 KiB, 256 KiB pitch.
                nc.gpsimd.dma_start(
                    y[_sl(3, 2, 126), cols, :].rearrange(
                        "(k two) w c -> k two w c", two=2
                    ),
                    u[_sl(2, 2, 63), :].unsqueeze(1).broadcast_to([63, 2, 128 * _C2]),
                )
    nc.compile()
    return nc


def _v1x_common(nc, mybir, tc, tin_pool, u_pool, f32, x):
    """Shared front half: chunk loads on the SWDGE ring head + DVE
    expansion into two half-width U tiles.  Returns u_halves."""
    t3s = []
    for q in range(4):
        w0 = 32 * q
        w1 = min(w0 + 33, _W)
        t = tin_pool.tile([_H, (w1 - w0) * _C2], f32, tag=f"t{q}", name=f"t{q}")
        nc.gpsimd.dma_start(
            t[:].rearrange("h (w c) -> h w c", c=_C2), x[:, w0:w1, :]
        )
        t3s.append(t[:].rearrange("h (w c) -> h w c", c=_C2))

    u_halves = [
        u_pool.tile([_H, 128 * _C2], f32, tag="uL", name="uL"),
        u_pool.tile([_H, 128 * _C2], f32, tag="uR", name="uR"),
    ]
    for q in range(4):
        t3 = t3s[q]
        u3h = u_halves[q // 2][:].rearrange("h (w c) -> h w c", c=_C2)
        u3 = u3h[:, 64 * (q % 2) : 64 * (q % 2) + 64, :]
        up = u3.rearrange("h (p two) c -> h p two c", two=2)
        nc.vector.tensor_copy(
            up[:, 0:32:2, :, :],
            t3[:, _sl(0, 2, 16), :].unsqueeze(2).broadcast_to([_H, 16, 2, _C2]),
        )
        nct = 15 if q == 3 else 16
        nc.vector.tensor_copy(
            up[:, 1 : 2 * nct : 2, :, :],
            t3[:, 1 : 2 * nct + 1, :].rearrange("h (g two) c -> h g two c", two=2),
        )
        if q == 3:
            nc.vector.tensor_copy(
                u3[:, 62:64, :],
                t3[:, 31:32, :].broadcast_to([_H, 2, _C2]),
            )
    return u_halves


def _build_nc_v11():
    """Loads-first + concurrent 3-ring family writes.

    After the loads drain (~29us, all expansions done), fam0 goes on the
    SWDGE ring while fam1 rides sync and fam2 rides scalar — three
    interleaved streams whose merged address coverage is near-dense
    ascending, testing whether lockstep interleave beats sequential
    strided sweeps."""
    import concourse.bacc as bacc
    import concourse.mybir as mybir
    from concourse.tile import TileContext

    f32 = mybir.dt.float32
    nc = bacc.Bacc()
    x = nc.dram_tensor("x", [_H, _W, _C2], f32, kind="ExternalInput")
    y = nc.dram_tensor("y", [_HO, _WO, _C2], f32, kind="ExternalOutput")

    with TileContext(nc) as tc:
        with (
            tc.tile_pool(name="tin", bufs=1) as tin_pool,
            tc.tile_pool(name="uexp", bufs=1) as u_pool,
        ):
            u_halves = _v1x_common(nc, mybir, tc, tin_pool, u_pool, f32, x)
            for h in range(2):
                u = u_halves[h]
                cols = slice(128 * h, 128 * (h + 1))
                nc.gpsimd.dma_start(y[_sl(0, 2, 128), cols, :], u[:, :])
                nc.gpsimd.dma_start(
                    y[_sl(1, 254, 2), cols, :], u[_sl(0, 127, 2), :]
                )
            for h in range(2):
                cols = slice(128 * h, 128 * (h + 1))
                rd0, rds, rs0, rss, rcnt = _FAMILIES[1]
                nc.sync.dma_start(
                    y[_sl(rd0, rds, rcnt), cols, :],
                    u_halves[h][_sl(rs0, rss, rcnt), :],
                )
                rd0, rds, rs0, rss, rcnt = _FAMILIES[2]
                nc.scalar.dma_start(
                    y[_sl(rd0, rds, rcnt), cols, :],
                    u_halves[h][_sl(rs0, rss, rcnt), :],
                )
    nc.compile()
    return nc


def _build_nc_v12():
    """Loads-first + all-SWDGE with fam1/fam2 interleaved in 1 MiB
    sub-DMAs (16 partitions each), so the FIFO ring's merged odd-row
    stream walks the address space densely at 256 KiB pitch instead of
    two full 512 KiB-pitch passes."""
    import concourse.bacc as bacc
    import concourse.mybir as mybir
    from concourse.tile import TileContext

    f32 = mybir.dt.float32
    nc = bacc.Bacc()
    x = nc.dram_tensor("x", [_H, _W, _C2], f32, kind="ExternalInput")
    y = nc.dram_tensor("y", [_HO, _WO, _C2], f32, kind="ExternalOutput")

    with TileContext(nc) as tc:
        with (
            tc.tile_pool(name="tin", bufs=1) as tin_pool,
            tc.tile_pool(name="uexp", bufs=1) as u_pool,
        ):
            u_halves = _v1x_common(nc, mybir, tc, tin_pool, u_pool, f32, x)
            for h in range(2):
                u = u_halves[h]
                cols = slice(128 * h, 128 * (h + 1))
                nc.gpsimd.dma_start(y[_sl(0, 2, 128), cols, :], u[:, :])
                nc.gpsimd.dma_start(
                    y[_sl(1, 254, 2), cols, :], u[_sl(0, 127, 2), :]
                )
                # Odd rows: alternate fam1/fam2 blocks of 16 partitions
                # (16 x 64 KiB = 1 MiB per sub-DMA) walking forward.
                for b in range(4):
                    # fam1 rows 1+4t for t in [16b, 16b+16)
                    nc.gpsimd.dma_start(
                        y[_sl(1 + 64 * b, 4, 16), cols, :],
                        u[_sl(32 * b, 2, 16), :],
                    )
                    # fam2 rows 3+4t for t in [16b, 16b+16) (15 in last)
                    ncnt = 15 if b == 3 else 16
                    nc.gpsimd.dma_start(
                        y[_sl(3 + 64 * b, 4, ncnt), cols, :],
                        u[_sl(2 + 32 * b, 2, ncnt), :],
                    )
    nc.compile()
    return nc


def _build_nc_v13():
    """Loads-first + one full-row U tile: every write descriptor is
    128 KiB.  Tests whether doubling descriptor size rescues the
    512 KiB-pitch odd families."""
    import concourse.bacc as bacc
    import concourse.mybir as mybir
    from concourse.tile import TileContext

    f32 = mybir.dt.float32
    nc = bacc.Bacc()
    x = nc.dram_tensor("x", [_H, _W, _C2], f32, kind="ExternalInput")
    y = nc.dram_tensor("y", [_HO, _WO, _C2], f32, kind="ExternalOutput")

    with TileContext(nc) as tc:
        with (
            tc.tile_pool(name="tin", bufs=1) as tin_pool,
            tc.tile_pool(name="uexp", bufs=1) as u_pool,
        ):
            t3s = []
            for q in range(4):
                w0 = 32 * q
                w1 = min(w0 + 33, _W)
                t = tin_pool.tile(
                    [_H, (w1 - w0) * _C2], f32, tag=f"t{q}", name=f"t{q}"
                )
                nc.gpsimd.dma_start(
                    t[:].rearrange("h (w c) -> h w c", c=_C2), x[:, w0:w1, :]
                )
                t3s.append(t[:].rearrange("h (w c) -> h w c", c=_C2))

            u = u_pool.tile([_H, 256 * _C2], f32, tag="u", name="u")
            u3f = u[:].rearrange("h (w c) -> h w c", c=_C2)
            for q in range(4):
                t3 = t3s[q]
                u3 = u3f[:, 64 * q : 64 * q + 64, :]
                up = u3.rearrange("h (p two) c -> h p two c", two=2)
                nc.vector.tensor_copy(
                    up[:, 0:32:2, :, :],
                    t3[:, _sl(0, 2, 16), :]
                    .unsqueeze(2)
                    .broadcast_to([_H, 16, 2, _C2]),
                )
                nct = 15 if q == 3 else 16
                nc.vector.tensor_copy(
                    up[:, 1 : 2 * nct : 2, :, :],
                    t3[:, 1 : 2 * nct + 1, :].rearrange(
                        "h (g two) c -> h g two c", two=2
                    ),
                )
                if q == 3:
                    nc.vector.tensor_copy(
                        u3[:, 62:64, :],
                        t3[:, 31:32, :].broadcast_to([_H, 2, _C2]),
                    )

            for fam in range(4):
                rd0, rds, rs0, rss, rcnt = _FAMILIES[fam]
                nc.gpsimd.dma_start(
                    y[_sl(rd0, rds, rcnt), :, :],
                    u[_sl(rs0, rss, rcnt), :],
                )
    nc.compile()
    return nc


def _build_nc_v14():
    """Loads-first + half U tiles + family-major 64 KiB writes.

    v13 lost 15us because fam0 (single full-row tile) waited on the q3
    expansion whose load-completion semaphore lands ~2-4us after the
    bytes.  With L/R halves, fam0_L only needs quarters 0-1 (expanded
    ~28us, exactly when the loads drain) and fam0_R only unblocks once
    the ring reaches it (~49us) — no gap anywhere.  Edge rows ride in
    the middle of the stream, not the tail."""
    import concourse.bacc as bacc
    import concourse.mybir as mybir
    from concourse.tile import TileContext

    f32 = mybir.dt.float32
    nc = bacc.Bacc()
    x = nc.dram_tensor("x", [_H, _W, _C2], f32, kind="ExternalInput")
    y = nc.dram_tensor("y", [_HO, _WO, _C2], f32, kind="ExternalOutput")

    with TileContext(nc) as tc:
        with (
            tc.tile_pool(name="tin", bufs=1) as tin_pool,
            tc.tile_pool(name="uexp", bufs=1) as u_pool,
        ):
            u_halves = _v1x_common(nc, mybir, tc, tin_pool, u_pool, f32, x)
            for fam in (0, 3, 1, 2):
                rd0, rds, rs0, rss, rcnt = _FAMILIES[fam]
                for h in range(2):
                    cols = slice(128 * h, 128 * (h + 1))
                    nc.gpsimd.dma_start(
                        y[_sl(rd0, rds, rcnt), cols, :],
                        u_halves[h][_sl(rs0, rss, rcnt), :],
                    )
    nc.compile()
    return nc


VERSION = 14
_BUILDERS = {
    1: _build_nc_v1,
    2: _build_nc_v2,
    3: _build_nc_v3,
    4: _build_nc_v4,
    5: _build_nc_v5,
    6: _build_nc_v6,
    7: _build_nc_v7,
    9: _build_nc_v9,
    10: _build_nc_v10,
    11: _build_nc_v11,
    12: _build_nc_v12,
    13: _build_nc_v13,
    14: _build_nc_v14,
}


def _selftest_families():
    """Host-side check: the family decomposition reproduces the reference
    round-half-to-even nearest index map exactly."""
    idx = np.round(128 * np.arange(256, dtype=np.float64) / 256.0)
    # np.round is round-half-to-even like jnp.round
    idx = np.clip(idx.astype(np.int64), 0, 127)
    recon = np.full(256, -1)
    for d0, ds, s0, ss, c in _FAMILIES:
        for i in range(c):
            assert recon[d0 + ds * i] == -1
            recon[d0 + ds * i] = s0 + ss * i
    assert (recon == idx).all()


_selftest_families()


def _build_nc():
    return _BUILDERS[VERSION]()


def _get_nc():
    if VERSION not in _NC_CACHE:
        _NC_CACHE[VERSION] = _build_nc()
    return _NC_CACHE[VERSION]


def kernel(x_real: np.ndarray, x_imag: np.ndarray) -> np.ndarray:
    global LAST_RESULT
    _ensure_axon_ntff_hook()
    from concourse.bass_utils import run_bass_kernel_spmd

    assert x_real.shape == (_B, _H, _W, _C) and x_imag.shape == (_B, _H, _W, _C)

    # Interleave real/imag channel-wise: f32 [B, H, W, 2C]; pairs
    # (re, im) match the complex64 memory layout.
    xc = np.empty((_B, _H, _W, _C, 2), np.float32)
    xc[..., 0] = x_real
    xc[..., 1] = x_imag
    xc = xc.reshape(_B, _H, _W, _C2)

    nc = _get_nc()
    in_maps = [{"x": xc[b]} for b in range(_B)]
    res = run_bass_kernel_spmd(
        nc,
        in_maps,
        core_ids=list(range(_N_CORES)),
        trace=TRACE,
    )
    LAST_RESULT = res

    out = np.stack([res.results[b]["y"] for b in range(_B)])
    # [B, 256, 256, 128] f32 -> complex64 view [B, 256, 256, 64]
    return out.view(np.complex64)

